# revision 18
# baseline (speedup 1.0000x reference)
"""Trainium2 Bass kernel for nn_BackMapLayerWithSidechains.

Algorithm: the sequential NeRF atom-placement chain is reformulated as a
product of affine transforms (R, t): each step composes M = Rx(phi) @
Rz(pi - theta) and t += d * R[:, 0].  Composition is associative, so the
147-step backbone chain is evaluated with a blocked scan:
  - within-chunk prefixes (C=13 chunks x L=12 steps) computed serially but
    vectorized over (chunk, sample),
  - chunk prefixes composed serially (13 small composes),
  - one wide "apply" produces every global atom position.
Sidechain frames branch off backbone frames exactly:
  F_side(r) = F_global(3r-1) . Rx(phi_{3r-1})   (r >= 1; r=0 is identity at
  atom 1), then 5 more affine steps per residue, vectorized over
  (residue, sample).  Sidechain rotation state runs in bf16 (short chains,
  error ~6e-4 of output scale), positions accumulate in fp32.

Data parallel over 8 NeuronCores: 2048 samples/core.  Host-side numpy only
reshapes/pads inputs into the on-chip layouts (no compute).
"""

import math
from contextlib import ExitStack

import numpy as np

import concourse.bacc as bacc
import concourse.bass as bass
import concourse.mybir as mybir
import concourse.tile as tile
from concourse.bass_utils import run_bass_kernel_spmd

F32 = mybir.dt.float32
BF16 = mybir.dt.bfloat16
F16 = mybir.dt.float16
AF = mybir.ActivationFunctionType

P = 128          # partitions
J = 16           # samples per partition (per core: 2048 = 128*16)
S = P * J        # samples per core
NCORES = 8
B = S * NCORES   # 16384

N_RES = 50
V = 4
NB = 150
NS = 250
K = 147          # real backbone steps
C = 13           # chunks
L = 12           # steps/chunk
KP = C * L       # 156 padded steps
NA = 159         # padded backbone atom count (3 + KP)
CHI0 = 2.0943951023931953
PI = math.pi

_cache = {}
TRACE = False


def _bcast_m(ap3):
    """[p, X, 16] -> [p, 3, X, 16] via 0-stride comp dim."""
    return ap3.unsqueeze(1).broadcast_to([P, 3] + list(ap3.shape[1:]))


def _emit(tc, io):
    nc = tc.nc
    V3 = nc.vector
    GP = nc.gpsimd

    TH_d, PH_d, DD_d, SEED_d, SD_d, SA_d, SPH_d, OUT_d = (
        io["TH"], io["PH"], io["DD"], io["SEED"], io["SD"], io["SA"],
        io["SPH"], io["OUT"])

    # Pools: strict LIFO release order.  Open order (outer->inner):
    # perm, side1, w, wt, trig1, dd, c3, thph
    es = ExitStack()
    p_perm = es.enter_context(tc.tile_pool(name="perm", bufs=1))
    p_side1 = es.enter_context(tc.tile_pool(name="side1", bufs=1))
    x_w = ExitStack()
    p_w = x_w.enter_context(tc.tile_pool(name="w", bufs=1))
    x_wt = ExitStack()
    p_wt = x_wt.enter_context(tc.tile_pool(name="wt", bufs=1))
    x_trig1 = ExitStack()
    p_trig1 = x_trig1.enter_context(tc.tile_pool(name="trig1", bufs=1))
    x_dd = ExitStack()
    p_dd = x_dd.enter_context(tc.tile_pool(name="dd", bufs=1))
    x_c3 = ExitStack()
    p_c3 = x_c3.enter_context(tc.tile_pool(name="c3", bufs=1))
    p_tmp = x_c3.enter_context(tc.tile_pool(name="wtmp", bufs=4))
    x_thph = ExitStack()
    p_thph = x_thph.enter_context(tc.tile_pool(name="thph", bufs=1))

    consts = {}

    def cst(val):
        if val not in consts:
            t = p_perm.tile([P, 1], F32, tag=f"cst{len(consts)}",
                            name=f"cst{len(consts)}")
            V3.memset(t[:], val)
            consts[val] = t[:]
        return consts[val]

    SEED = p_perm.tile([P, 48], F32, tag="SEED")
    CP = p_perm.tile([P, KP * J], F16, tag="CP")
    SP = p_perm.tile([P, KP * J], F16, tag="SP")
    A_all = p_perm.tile([P, C * 192], F32, tag="A_all")

    RS_a = [p_side1.tile([P, 3 * N_RES * J], F16, tag=f"RSa{i}",
                         name=f"RSa{i}") for i in range(3)]
    T0 = p_side1.tile([P, 3 * N_RES * J], F32, tag="T0")

    WC1 = p_w.tile([P, 3 * KP * J], F16, tag="WC1")
    WC2 = p_w.tile([P, 3 * KP * J], F16, tag="WC2")
    WT = p_wt.tile([P, 3 * KP * J], F16, tag="WT")
    CA_ = p_trig1.tile([P, KP * J], F16, tag="CA_")
    SA = p_trig1.tile([P, KP * J], F16, tag="SA")
    DD = p_dd.tile([P, KP * J], F16, tag="DD")
    c3t = [p_c3.tile([P, 3 * C * J], F16, tag=f"c3{i}", name=f"c3{i}")
           for i in range(2)]
    TH = p_thph.tile([P, KP * J], F16, tag="TH")
    PH = p_thph.tile([P, KP * J], F16, tag="PH")

    # ---------------- P0: input DMAs (backbone) ----------------
    nc.sync.dma_start(TH[:], TH_d.ap())
    nc.sync.dma_start(PH[:], PH_d.ap())
    nc.sync.dma_start(DD[:], DD_d.ap())
    nc.sync.dma_start(SEED[:], SEED_d.ap())

    # ---------------- P1: backbone trig (ACT), fp16 outputs ----------
    nc.scalar.activation(CA_[:], TH[:], AF.Sin, bias=cst(-PI / 2))
    nc.scalar.activation(SA[:], TH[:], AF.Sin)
    nc.scalar.activation(SP[:], PH[:], AF.Sin)
    nc.scalar.activation(PH[:], PH[:], AF.Abs)
    nc.scalar.activation(CP[:], PH[:], AF.Sin, scale=-1.0, bias=cst(PI / 2))
    x_thph.close()

    def wv(t):
        return t[:].rearrange("p (m k j) -> p m k j", m=3, k=KP, j=J)

    def wl(t, l):
        return t[:].rearrange("p (m c l j) -> p m c l j",
                              m=3, c=C, l=L, j=J)[:, :, :, l, :]

    def trig_l(t, l, bcast=True):
        s = t[:].rearrange("p (c l j) -> p c l j", c=C, l=L, j=J)[:, :, l, :]
        return _bcast_m(s) if bcast else s

    def c3v(i):
        return c3t[i][:].rearrange("p (m c j) -> p m c j", m=3, c=C, j=J)

    # ---------------- P2a: within-chunk init (l = 0) ----------------
    w1_0, w2_0 = wl(WC1, 0), wl(WC2, 0)
    ca0, sa0 = trig_l(CA_, 0, False), trig_l(SA, 0, False)
    cp0, sp0 = trig_l(CP, 0, False), trig_l(SP, 0, False)
    nc.scalar.copy(w1_0[:, 0], ca0)
    V3.tensor_mul(w1_0[:, 1], sa0, cp0)
    V3.tensor_mul(w1_0[:, 2], sa0, sp0)
    V3.tensor_scalar_mul(w2_0[:, 0], sa0, -1.0)
    V3.tensor_mul(w2_0[:, 1], ca0, cp0)
    V3.tensor_mul(w2_0[:, 2], ca0, sp0)
    c3i = c3v(0)
    V3.memset(c3i[:, 0], 0.0)
    V3.tensor_scalar_mul(c3i[:, 1], sp0, -1.0)
    nc.scalar.copy(c3i[:, 2], cp0)
    V3.tensor_mul(wl(WT, 0), trig_l(DD, 0), wl(WC1, 0))

    # ---------------- P2b: within-chunk serial scan ----------------
    for l in range(1, L):
        cpl, spl = trig_l(CP, l), trig_l(SP, l)
        cal, sal = trig_l(CA_, l), trig_l(SA, l)
        p1, p2 = wl(WC1, l - 1), wl(WC2, l - 1)
        p3 = c3v((l - 1) % 2)
        c3c = c3v(l % 2)
        sh = [P, 3 * C * J]

        def tt(tag):
            return p_tmp.tile(sh, F16, tag=tag, name=tag)[:].rearrange(
                "p (m c j) -> p m c j", m=3, c=C, j=J)

        def ttf(tag):
            return p_tmp.tile(sh, F32, tag=tag, name=tag)[:].rearrange(
                "p (m c j) -> p m c j", m=3, c=C, j=J)

        t1, t2, c2x = tt("t1"), tt("t2"), tt("c2x")
        V3.tensor_mul(t1, cpl, p2)
        V3.tensor_mul(t2, spl, p3)
        V3.tensor_add(c2x, t1, t2)
        t1, t2 = tt("t1"), tt("t2")
        V3.tensor_mul(t1, cpl, p3)
        V3.tensor_mul(t2, spl, p2)
        V3.tensor_sub(c3c, t1, t2)
        t1, t2 = tt("t1"), tt("t2")
        V3.tensor_mul(t1, cal, p1)
        V3.tensor_mul(t2, sal, c2x)
        V3.tensor_add(wl(WC1, l), t1, t2)
        t1, t2 = tt("t1"), tt("t2")
        V3.tensor_mul(t1, cal, c2x)
        V3.tensor_mul(t2, sal, p1)
        V3.tensor_sub(wl(WC2, l), t1, t2)
        t9 = tt("t9")
        V3.tensor_mul(t9, trig_l(DD, l), wl(WC1, l))
        V3.tensor_add(wl(WT, l), wl(WT, l - 1), t9)

    # ---------------- P3a: chunk totals T ----------------
    # T lives in its own pool opened after closing c3/dd/trig1? T build
    # reads c3 -> keep c3 open until the copies below are emitted.
    x_T = ExitStack()
    p_T = x_T.enter_context(tc.tile_pool(name="Tp", bufs=1))
    p_u = x_T.enter_context(tc.tile_pool(name="up", bufs=2))
    T_all = p_T.tile([P, C * 192], F32, tag="T_all")

    def anv(t):
        return t[:].rearrange("p (c n m j) -> p c n m j", c=C, n=4, m=3, j=J)

    Tv, Av = anv(T_all), anv(A_all)
    nc.scalar.copy(Tv[:, :, 0], wl(WC1, L - 1).transpose([0, 2, 1, 3]))
    nc.scalar.copy(Tv[:, :, 1], wl(WC2, L - 1).transpose([0, 2, 1, 3]))
    nc.scalar.copy(Tv[:, :, 2], c3v((L - 1) % 2).transpose([0, 2, 1, 3]))
    nc.scalar.copy(Tv[:, :, 3], wl(WT, L - 1).transpose([0, 2, 1, 3]))

    # ---------------- P3b: seed A_0 = F0 ----------------
    sdv = SEED[:].rearrange("p (q j) -> p q j", q=3, j=J)
    cd0, cd1, ca0s = sdv[:, 0], sdv[:, 1], sdv[:, 2]
    sca0 = p_T.tile([P, J], F32, tag="sca0", name="sca0")[:]
    cca0 = p_T.tile([P, J], F32, tag="cca0", name="cca0")[:]
    nc.scalar.activation(sca0, ca0s, AF.Sin)
    nc.scalar.activation(cca0, ca0s, AF.Sin, bias=cst(-PI / 2))
    V3.memset(Av[:, 0], 0.0)
    nc.scalar.copy(Av[:, 0, 0, 0], cca0)
    nc.scalar.copy(Av[:, 0, 0, 1], sca0)
    V3.tensor_scalar_mul(Av[:, 0, 1, 0], sca0, -1.0)
    nc.scalar.copy(Av[:, 0, 1, 1], cca0)
    V3.memset(Av[:, 0, 2, 2], 1.0)
    tm0 = p_T.tile([P, J], F32, tag="tm0", name="tm0")[:]
    V3.tensor_mul(tm0, cd1, cca0)
    V3.tensor_add(Av[:, 0, 3, 0], cd0, tm0)
    V3.tensor_mul(Av[:, 0, 3, 1], cd1, sca0)

    # ---------------- P3c: chunk-prefix chain ----------------
    for c in range(1, C):
        acm = [Av[:, c - 1, m].unsqueeze(1).broadcast_to([P, 4, 3, J])
               for m in range(3)]
        trm = [Tv[:, c - 1, :, m].unsqueeze(2).broadcast_to([P, 4, 3, J])
               for m in range(3)]

        def uu(tag):
            return p_u.tile([P, 192], F32, tag=tag, name=tag)[:].rearrange(
                "p (n m j) -> p n m j", n=4, m=3, j=J)

        u1, u2, u3 = uu("u1"), uu("u2"), uu("u3")
        V3.tensor_mul(u1, acm[0], trm[0])
        V3.tensor_mul(u2, acm[1], trm[1])
        V3.tensor_add(u3, u1, u2)
        u1 = uu("u1")
        V3.tensor_mul(u1, acm[2], trm[2])
        V3.tensor_add(Av[:, c], u3, u1)
        V3.tensor_add(Av[:, c, 3], Av[:, c, 3], Av[:, c - 1, 3])

    # close inner pools (LIFO): Tp/up, then c3, dd, trig1
    x_T.close()
    x_c3.close()
    x_dd.close()
    x_trig1.close()

    # ---------------- P4: apply -> backbone AoS ----------------
    x_bbaos = ExitStack()
    p_bb = x_bbaos.enter_context(tc.tile_pool(name="bbaos", bufs=1))
    BBAOS = p_bb.tile([P, J * NA * 3], F32, tag="BBAOS")

    def bbv():
        return BBAOS[:].rearrange("p (j a m) -> p j a m", j=J, a=NA, m=3)

    x_apply = ExitStack()
    p_ap = x_apply.enter_context(tc.tile_pool(name="applyp", bufs=2))
    wtv = WT[:].rearrange("p (m c l j) -> p m c l j", m=3, c=C, l=L, j=J)
    for c in range(C):
        # [p, comp, L, J] per chunk (3 free dims: walrus TENSOR3D limit)
        aw = [Av[:, c, m].unsqueeze(2).broadcast_to([P, 3, L, J])
              for m in range(3)]
        at = Av[:, c, 3].unsqueeze(2).broadcast_to([P, 3, L, J])
        wtm = [wtv[:, m, c].unsqueeze(1).broadcast_to([P, 3, L, J])
               for m in range(3)]

        EE = GP if c >= 10 else V3

        def qq(tag):
            return p_ap.tile([P, 3 * L * J], F16, tag=tag,
                             name=f"{tag}_{c}")[:].rearrange(
                "p (m l j) -> p m l j", m=3, l=L, j=J)

        q1, q2 = qq("q1"), qq("q2")
        EE.tensor_mul(q1, aw[0], wtm[0])
        EE.tensor_mul(q2, aw[1], wtm[1])
        EE.tensor_add(q1, q1, q2)
        q2 = qq("q2")
        EE.tensor_mul(q2, aw[2], wtm[2])
        EE.tensor_add(q1, q1, q2)
        outv = bbv()[:, :, 3 + c * L:3 + (c + 1) * L, :].transpose(
            [0, 3, 2, 1])
        EE.tensor_add(outv, q1, at)
    x_apply.close()

    # ---------------- P5: atoms 0,1,2 ----------------
    bb = bbv()
    V3.memset(bb[:, :, 0:2, :], 0.0)
    nc.scalar.copy(bb[:, :, 1, 0], cd0)
    V3.memset(bb[:, :, 2, 2], 0.0)
    nc.scalar.copy(bb[:, :, 2, 0], Av[:, 0, 3, 0])
    nc.scalar.copy(bb[:, :, 2, 1], Av[:, 0, 3, 1])

    # ---------------- P6: backbone DMA out + T0 gather ----------------
    outdv = OUT_d.ap().rearrange("p (j a m) -> p j a m", j=J, a=400, m=3)
    nc.sync.dma_start(outdv[:, :, 0:NB, :], bb[:, :, 0:NB, :])
    t0v = T0[:].rearrange("p (m r j) -> p m r j", m=3, r=N_RES, j=J)
    nc.scalar.copy(t0v, bb[:, :, 1:NB:3, :].transpose([0, 3, 2, 1]))
    x_bbaos.close()
    x_wt.close()

    # ---------------- P8: sidechain seeds ----------------
    x_seed = ExitStack()
    p_sd = x_seed.enter_context(tc.tile_pool(name="seedp", bufs=1))

    def rsv(t):
        return t[:].rearrange("p (m r j) -> p m r j", m=3, r=N_RES, j=J)

    AEX = [p_sd.tile([P, 3 * 49 * J], F16, tag=f"AEX{m}", name=f"AEX{m}")
           for m in range(3)]

    def aexv(t):
        return t[:].rearrange("p (m r j) -> p m r j", m=3, r=49, j=J)

    for m in range(3):
        dst = aexv(AEX[m])
        for cq in range(12):
            nc.scalar.copy(
                dst[:, :, 4 * cq:4 * cq + 4, :],
                Av[:, cq, m].unsqueeze(2).broadcast_to([P, 3, 4, J]))
        nc.scalar.copy(dst[:, :, 48, :], Av[:, 12, m])

    w1g = wv(WC1)[:, :, 1:146:3, :]
    w2g = wv(WC2)[:, :, 1:146:3, :]
    C2AW = p_sd.tile([P, 3 * 49 * J], F16, tag="C2AW")
    C3AW = p_sd.tile([P, 3 * 49 * J], F16, tag="C3AW")
    c2awv, c3awv = aexv(C2AW), aexv(C3AW)
    sA = p_sd.tile([P, 3 * 49 * J], F16, tag="sA")
    sB = p_sd.tile([P, 3 * 49 * J], F16, tag="sB")
    sAv, sBv = aexv(sA), aexv(sB)

    def apply_R(wg, outv):
        V3.tensor_mul(sAv, aexv(AEX[0]),
                      wg[:, 0].unsqueeze(1).broadcast_to([P, 3, 49, J]))
        V3.tensor_mul(sBv, aexv(AEX[1]),
                      wg[:, 1].unsqueeze(1).broadcast_to([P, 3, 49, J]))
        V3.tensor_add(sAv, sAv, sBv)
        V3.tensor_mul(sBv, aexv(AEX[2]),
                      wg[:, 2].unsqueeze(1).broadcast_to([P, 3, 49, J]))
        V3.tensor_add(outv, sAv, sBv)

    rs1a = rsv(RS_a[0])
    apply_R(w1g, rs1a[:, :, 1:, :])
    apply_R(w2g, c2awv)
    for mo in range(3):
        m1, m2 = (mo + 1) % 3, (mo + 2) % 3
        V3.tensor_mul(sAv[:, 0], rs1a[:, m1, 1:, :], c2awv[:, m2])
        V3.tensor_mul(sBv[:, 0], rs1a[:, m2, 1:, :], c2awv[:, m1])
        V3.tensor_sub(c3awv[:, mo], sAv[:, 0], sBv[:, 0])
    kj = lambda t: t[:].rearrange("p (k j) -> p k j", k=KP, j=J)
    cpb = _bcast_m(kj(CP)[:, 2:147:3, :])
    spb = _bcast_m(kj(SP)[:, 2:147:3, :])
    rs2a, rs3a = rsv(RS_a[1]), rsv(RS_a[2])
    sC = p_sd.tile([P, 3 * 49 * J], F16, tag="sC")
    sD = p_sd.tile([P, 3 * 49 * J], F16, tag="sD")
    sCv, sDv = aexv(sC), aexv(sD)
    GP.tensor_mul(sCv, cpb, c2awv)
    GP.tensor_mul(sDv, spb, c3awv)
    GP.tensor_add(rs2a[:, :, 1:, :], sCv, sDv)
    V3.tensor_mul(sAv, cpb, c3awv)
    V3.tensor_mul(sBv, spb, c2awv)
    V3.tensor_sub(rs3a[:, :, 1:, :], sAv, sBv)
    for m in range(3):
        rm = rsv(RS_a[m])
        V3.memset(rm[:, :, 0, :], 0.0)
        V3.memset(rm[:, m, 0, :], 1.0)
    x_seed.close()
    x_w.close()

    # ---------------- P7: sidechain inputs + trig ----------------
    x_side2 = ExitStack()
    p_side2 = x_side2.enter_context(tc.tile_pool(name="side2", bufs=1))
    RS_b = [p_side2.tile([P, 3 * N_RES * J], F16, tag=f"RSb{i}",
                         name=f"RSb{i}") for i in range(3)]
    SIDEAOS = p_side2.tile([P, J * NS * 3], F32, tag="SIDEAOS")
    SD = p_side2.tile([P, 5 * N_RES * J], F16, tag="SDb")
    SCA = p_side2.tile([P, 5 * N_RES * J], F16, tag="SCA")
    SSA = p_side2.tile([P, 5 * N_RES * J], F16, tag="SSA")
    SCP = p_side2.tile([P, 5 * N_RES * J], F16, tag="SCP")
    SSP = p_side2.tile([P, 5 * N_RES * J], F16, tag="SSP")
    x_sin = ExitStack()
    p_si = x_sin.enter_context(tc.tile_pool(name="sidein", bufs=1))
    SAs = p_si.tile([P, 5 * N_RES * J], F16, tag="SAs")
    SPHs = p_si.tile([P, 5 * N_RES * J], F16, tag="SPHs")
    nc.sync.dma_start(SD[:], SD_d.ap())
    nc.sync.dma_start(SAs[:], SA_d.ap())
    nc.sync.dma_start(SPHs[:], SPH_d.ap())
    nc.scalar.activation(SCA[:], SAs[:], AF.Sin, bias=cst(-PI / 2))
    nc.scalar.activation(SSA[:], SAs[:], AF.Sin)
    nc.scalar.activation(SSP[:], SPHs[:], AF.Sin)
    nc.scalar.activation(SPHs[:], SPHs[:], AF.Abs)
    nc.scalar.activation(SCP[:], SPHs[:], AF.Sin, scale=-1.0,
                         bias=cst(PI / 2))
    x_sin.close()

    # ---------------- P9: sidechain serial chain ----------------
    sav = SIDEAOS[:].rearrange("p (j r v m) -> p j r v m",
                               j=J, r=N_RES, v=5, m=3)
    p_st = x_side2.enter_context(tc.tile_pool(name="sidetmp", bufs=3))

    def sl(t, v):
        s = t[:].rearrange("p (v r j) -> p v r j", v=5, r=N_RES, j=J)[:, v]
        return _bcast_m(s)

    cur, nxt = RS_a, RS_b
    for v in range(5):
        cpl, spl = sl(SCP, v), sl(SSP, v)
        cal, sal = sl(SCA, v), sl(SSA, v)
        r1p, r2p, r3p = rsv(cur[0]), rsv(cur[1]), rsv(cur[2])
        r1c, r2c, r3c = rsv(nxt[0]), rsv(nxt[1]), rsv(nxt[2])
        shb = [P, 3 * N_RES * J]

        def st(tag):
            return p_st.tile(shb, F16, tag=tag, name=tag)[:].rearrange(
                "p (m r j) -> p m r j", m=3, r=N_RES, j=J)

        t1, t2, c2x = st("s1"), st("s2"), st("sc2x")
        V3.tensor_mul(t1, cpl, r2p)
        V3.tensor_mul(t2, spl, r3p)
        V3.tensor_add(c2x, t1, t2)
        if v < 4:
            t1, t2 = st("s1"), st("s2")
            V3.tensor_mul(t1, cpl, r3p)
            V3.tensor_mul(t2, spl, r2p)
            V3.tensor_sub(r3c, t1, t2)
        t1, t2 = st("s1"), st("s2")
        V3.tensor_mul(t1, cal, r1p)
        V3.tensor_mul(t2, sal, c2x)
        V3.tensor_add(r1c, t1, t2)
        if v < 4:
            t1, t2 = st("s1"), st("s2")
            V3.tensor_mul(t1, cal, c2x)
            V3.tensor_mul(t2, sal, r1p)
            V3.tensor_sub(r2c, t1, t2)
        t9 = p_st.tile(shb, F16, tag="st9", name="st9")[:].rearrange(
            "p (m r j) -> p m r j", m=3, r=N_RES, j=J)
        V3.tensor_mul(t9, sl(SD, v), r1c)
        out = sav[:, :, :, v, :].transpose([0, 3, 2, 1])
        if v == 0:
            V3.tensor_add(out, t0v, t9)
        else:
            prev = sav[:, :, :, v - 1, :].transpose([0, 3, 2, 1])
            V3.tensor_add(out, prev, t9)
        cur, nxt = nxt, cur

    # ---------------- P10: sidechain DMA out ----------------
    sflat = SIDEAOS[:].rearrange("p (j x) -> p j x", j=J, x=NS * 3)
    dflat = OUT_d.ap().rearrange("p (j a m) -> p j a m", j=J, a=400, m=3)[
        :, :, NB:400, :].rearrange("p j a m -> p j (a m)")
    nc.sync.dma_start(dflat, sflat)

    x_side2.close()
    es.close()


def _build():
    if "nc" in _cache:
        return _cache["nc"]
    nc = bacc.Bacc("TRN2", target_bir_lowering=False, debug=False,
                   num_devices=NCORES)
    io = {
        "TH": nc.dram_tensor("TH", [P, KP * J], F16, kind="ExternalInput"),
        "PH": nc.dram_tensor("PH", [P, KP * J], F16, kind="ExternalInput"),
        "DD": nc.dram_tensor("DD", [P, KP * J], F16, kind="ExternalInput"),
        "SEED": nc.dram_tensor("SEED", [P, 48], F32, kind="ExternalInput"),
        "SD": nc.dram_tensor("SD", [P, 5 * N_RES * J], F16,
                             kind="ExternalInput"),
        "SA": nc.dram_tensor("SA", [P, 5 * N_RES * J], F16,
                             kind="ExternalInput"),
        "SPH": nc.dram_tensor("SPH", [P, 5 * N_RES * J], F16,
                              kind="ExternalInput"),
        "OUT": nc.dram_tensor("OUT", [P, J * 400 * 3], F32,
                              kind="ExternalOutput"),
    }
    with tile.TileContext(nc) as tc:
        _emit(tc, io)
    nc.compile()
    _cache["nc"] = nc
    return nc


def _marshal_core(cd, ca, ct, sd, sa, st):
    """Per-core (S samples) numpy repack into [128, free] layouts."""
    n = cd.shape[0]

    def tp(arr, ncol):  # (n, ncol) -> [128, ncol*16] k-major
        return np.ascontiguousarray(
            arr.reshape(P, J, ncol).transpose(0, 2, 1).reshape(P, ncol * J))

    th = np.full((n, KP), PI, np.float32)
    th[:, :K] = ca[:, 1:148]
    ph = np.zeros((n, KP), np.float32)
    ph[:, :K] = ct[:, :K]
    dd = np.zeros((n, KP), np.float32)
    dd[:, :K] = cd[:, 2:149]
    seed = np.stack([cd[:, 0], cd[:, 1], ca[:, 0]], axis=1)

    def tps(arr):  # (n, 50, 5) -> [128, 4000] (v, r, j)
        return np.ascontiguousarray(
            arr.reshape(P, J, N_RES, 5).transpose(0, 3, 2, 1).reshape(P, 4000))

    sph = np.empty((n, N_RES, 5), np.float32)
    sph[:, :, 0] = CHI0
    sph[:, :, 1:] = st.reshape(n, N_RES, V)
    return {
        "TH": tp(th, KP).astype(np.float16),
        "PH": tp(ph, KP).astype(np.float16),
        "DD": tp(dd, KP).astype(np.float16),
        "SEED": tp(seed, 3),
        "SD": tps(sd.reshape(n, N_RES, 5)).astype(np.float16),
        "SA": tps(sa.reshape(n, N_RES, 5)).astype(np.float16),
        "SPH": tps(sph).astype(np.float16),
    }


def kernel(central_distances, central_angles, central_dihedrals,
           side_distances, side_angles, side_dihedrals, **kw):
    nc = _build()
    in_maps = []
    for i in range(NCORES):
        sl_ = slice(i * S, (i + 1) * S)
        in_maps.append(_marshal_core(
            central_distances[sl_], central_angles[sl_],
            central_dihedrals[sl_], side_distances[sl_],
            side_angles[sl_], side_dihedrals[sl_]))
    res = run_bass_kernel_spmd(nc, in_maps, core_ids=list(range(NCORES)),
                               trace=TRACE, stitch_traces=False)
    _cache["last_results"] = res
    outs = []
    for i in range(NCORES):
        o = res.results[i]["OUT"].reshape(P, J, 400, 3).reshape(S, 400, 3)
        outs.append(o)
    return np.ascontiguousarray(np.concatenate(outs, axis=0))


# revision 24
# speedup vs baseline: 1.0965x; 1.0965x over previous
"""Trainium2 Bass kernel for nn_BackMapLayerWithSidechains.

Algorithm: the sequential NeRF atom-placement chain is reformulated as a
product of affine transforms (R, t): each step composes M = Rx(phi) @
Rz(pi - theta) and t += d * R[:, 0].  Composition is associative, so the
147-step backbone chain is evaluated with a blocked scan:
  - within-chunk prefixes (C=13 chunks x L=12 steps) computed serially but
    vectorized over (chunk, sample),
  - chunk prefixes composed serially (13 small composes),
  - one wide "apply" produces every global atom position.
Sidechain frames branch off backbone frames exactly:
  F_side(r) = F_global(3r-1) . Rx(phi_{3r-1})   (r >= 1; r=0 is identity at
  atom 1), then 5 more affine steps per residue, vectorized over
  (residue, sample).  Sidechain rotation state runs in bf16 (short chains,
  error ~6e-4 of output scale), positions accumulate in fp32.

Data parallel over 8 NeuronCores: 2048 samples/core.  Host-side numpy only
reshapes/pads inputs into the on-chip layouts (no compute).
"""

import math
from contextlib import ExitStack

import numpy as np

import concourse.bacc as bacc
import concourse.bass as bass
import concourse.mybir as mybir
import concourse.tile as tile
from concourse.bass_utils import run_bass_kernel_spmd

F32 = mybir.dt.float32
BF16 = mybir.dt.bfloat16
F16 = mybir.dt.float16
AF = mybir.ActivationFunctionType

P = 128          # partitions
J = 16           # samples per partition (per core: 2048 = 128*16)
S = P * J        # samples per core
NCORES = 8
B = S * NCORES   # 16384

N_RES = 50
V = 4
NB = 150
NS = 250
K = 147          # real backbone steps
C = 13           # chunks
L = 12           # steps/chunk
KP = C * L       # 156 padded steps
NA = 159         # padded backbone atom count (3 + KP)
CHI0 = 2.0943951023931953
PI = math.pi

_cache = {}
TRACE = False


def _bcast_m(ap3):
    """[p, X, 16] -> [p, 3, X, 16] via 0-stride comp dim."""
    return ap3.unsqueeze(1).broadcast_to([P, 3] + list(ap3.shape[1:]))


def _emit(tc, io):
    nc = tc.nc
    V3 = nc.vector
    GP = nc.gpsimd

    TH_d, PH_d, DD_d, SEED_d, SD_d, SA_d, SPH_d, OUT_d = (
        io["TH"], io["PH"], io["DD"], io["SEED"], io["SD"], io["SA"],
        io["SPH"], io["OUT"])

    # Pools: strict LIFO release order.  Open order (outer->inner):
    # perm, side1, w, wt, trig1, dd, c3, thph
    es = ExitStack()
    p_perm = es.enter_context(tc.tile_pool(name="perm", bufs=1))
    p_side1 = es.enter_context(tc.tile_pool(name="side1", bufs=1))
    x_w = ExitStack()
    p_w = x_w.enter_context(tc.tile_pool(name="w", bufs=1))
    x_wt = ExitStack()
    p_wt = x_wt.enter_context(tc.tile_pool(name="wt", bufs=1))
    x_trig1 = ExitStack()
    p_trig1 = x_trig1.enter_context(tc.tile_pool(name="trig1", bufs=1))
    x_dd = ExitStack()
    p_dd = x_dd.enter_context(tc.tile_pool(name="dd", bufs=1))
    x_c3 = ExitStack()
    p_c3 = x_c3.enter_context(tc.tile_pool(name="c3", bufs=1))
    p_tmp = x_c3.enter_context(tc.tile_pool(name="wtmp", bufs=4))
    x_thph = ExitStack()
    p_thph = x_thph.enter_context(tc.tile_pool(name="thph", bufs=1))

    consts = {}

    def cst(val):
        if val not in consts:
            t = p_perm.tile([P, 1], F32, tag=f"cst{len(consts)}",
                            name=f"cst{len(consts)}")
            V3.memset(t[:], val)
            consts[val] = t[:]
        return consts[val]

    SEED = p_perm.tile([P, 48], F32, tag="SEED")
    CP = p_perm.tile([P, KP * J], F16, tag="CP")
    SP = p_perm.tile([P, KP * J], F16, tag="SP")
    A_all = p_perm.tile([P, C * 192], F32, tag="A_all")

    RS_a = [p_side1.tile([P, 3 * N_RES * J], F16, tag=f"RSa{i}",
                         name=f"RSa{i}") for i in range(3)]
    T0 = p_side1.tile([P, 3 * N_RES * J], F32, tag="T0")

    WC1 = p_w.tile([P, 3 * KP * J], F16, tag="WC1")
    WC2 = p_w.tile([P, 3 * KP * J], F16, tag="WC2")
    WT = p_wt.tile([P, 3 * KP * J], F16, tag="WT")
    CA_ = p_trig1.tile([P, KP * J], F16, tag="CA_")
    SA = p_trig1.tile([P, KP * J], F16, tag="SA")
    DD = p_dd.tile([P, KP * J], F16, tag="DD")
    c3t = [p_c3.tile([P, 3 * C * J], F16, tag=f"c3{i}", name=f"c3{i}")
           for i in range(2)]
    TH = p_thph.tile([P, KP * J], F16, tag="TH")
    PH = p_thph.tile([P, KP * J], F16, tag="PH")

    # ---------------- P0: input DMAs (backbone) ----------------
    nc.sync.dma_start(TH[:], TH_d.ap())
    nc.sync.dma_start(PH[:], PH_d.ap())
    nc.sync.dma_start(DD[:], DD_d.ap())
    nc.sync.dma_start(SEED[:], SEED_d.ap())

    # ---------------- P1: backbone trig (ACT), fp16 outputs ----------
    nc.scalar.activation(CA_[:], TH[:], AF.Sin, bias=cst(-PI / 2))
    nc.scalar.activation(SA[:], TH[:], AF.Sin)
    nc.scalar.activation(SP[:], PH[:], AF.Sin)
    nc.scalar.activation(PH[:], PH[:], AF.Abs)
    nc.scalar.activation(CP[:], PH[:], AF.Sin, scale=-1.0, bias=cst(PI / 2))
    x_thph.close()

    def wv(t):
        return t[:].rearrange("p (m k j) -> p m k j", m=3, k=KP, j=J)

    def wl(t, l):
        return t[:].rearrange("p (m c l j) -> p m c l j",
                              m=3, c=C, l=L, j=J)[:, :, :, l, :]

    def trig_l(t, l, bcast=True):
        s = t[:].rearrange("p (c l j) -> p c l j", c=C, l=L, j=J)[:, :, l, :]
        return _bcast_m(s) if bcast else s

    def c3v(i):
        return c3t[i][:].rearrange("p (m c j) -> p m c j", m=3, c=C, j=J)

    # ---------------- P2a: within-chunk init (l = 0) ----------------
    w1_0, w2_0 = wl(WC1, 0), wl(WC2, 0)
    ca0, sa0 = trig_l(CA_, 0, False), trig_l(SA, 0, False)
    cp0, sp0 = trig_l(CP, 0, False), trig_l(SP, 0, False)
    nc.scalar.copy(w1_0[:, 0], ca0)
    V3.tensor_mul(w1_0[:, 1], sa0, cp0)
    V3.tensor_mul(w1_0[:, 2], sa0, sp0)
    V3.tensor_scalar_mul(w2_0[:, 0], sa0, -1.0)
    V3.tensor_mul(w2_0[:, 1], ca0, cp0)
    V3.tensor_mul(w2_0[:, 2], ca0, sp0)
    c3i = c3v(0)
    V3.memset(c3i[:, 0], 0.0)
    V3.tensor_scalar_mul(c3i[:, 1], sp0, -1.0)
    nc.scalar.copy(c3i[:, 2], cp0)
    GP.tensor_mul(wl(WT, 0), trig_l(DD, 0), wl(WC1, 0))

    # ---------------- P2b: within-chunk serial scan ----------------
    for l in range(1, L):
        cpl, spl = trig_l(CP, l), trig_l(SP, l)
        cal, sal = trig_l(CA_, l), trig_l(SA, l)
        p1, p2 = wl(WC1, l - 1), wl(WC2, l - 1)
        p3 = c3v((l - 1) % 2)
        c3c = c3v(l % 2)
        sh = [P, 3 * C * J]

        def tt(tag):
            return p_tmp.tile(sh, F16, tag=tag, name=tag)[:].rearrange(
                "p (m c j) -> p m c j", m=3, c=C, j=J)

        def ttf(tag):
            return p_tmp.tile(sh, F32, tag=tag, name=tag)[:].rearrange(
                "p (m c j) -> p m c j", m=3, c=C, j=J)

        t1, t2, c2x = tt("t1"), tt("t2"), tt("c2x")
        V3.tensor_mul(t1, cpl, p2)
        V3.tensor_mul(t2, spl, p3)
        V3.tensor_add(c2x, t1, t2)
        t1, t2 = tt("t1"), tt("t2")
        V3.tensor_mul(t1, cpl, p3)
        V3.tensor_mul(t2, spl, p2)
        V3.tensor_sub(c3c, t1, t2)
        t1, t2 = tt("t1"), tt("t2")
        V3.tensor_mul(t1, cal, p1)
        V3.tensor_mul(t2, sal, c2x)
        V3.tensor_add(wl(WC1, l), t1, t2)
        t1, t2 = tt("t1"), tt("t2")
        V3.tensor_mul(t1, cal, c2x)
        V3.tensor_mul(t2, sal, p1)
        V3.tensor_sub(wl(WC2, l), t1, t2)
        t9 = tt("t9")
        GP.tensor_mul(t9, trig_l(DD, l), wl(WC1, l))
        GP.tensor_add(wl(WT, l), wl(WT, l - 1), t9)

    # ---------------- P3a: chunk totals T ----------------
    # T lives in its own pool opened after closing c3/dd/trig1? T build
    # reads c3 -> keep c3 open until the copies below are emitted.
    x_T = ExitStack()
    p_T = x_T.enter_context(tc.tile_pool(name="Tp", bufs=1))
    p_u = x_T.enter_context(tc.tile_pool(name="up", bufs=2))
    T_all = p_T.tile([P, C * 192], F32, tag="T_all")

    def anv(t):
        return t[:].rearrange("p (c n m j) -> p c n m j", c=C, n=4, m=3, j=J)

    Tv, Av = anv(T_all), anv(A_all)
    nc.scalar.copy(Tv[:, :, 0], wl(WC1, L - 1).transpose([0, 2, 1, 3]))
    nc.scalar.copy(Tv[:, :, 1], wl(WC2, L - 1).transpose([0, 2, 1, 3]))
    nc.scalar.copy(Tv[:, :, 2], c3v((L - 1) % 2).transpose([0, 2, 1, 3]))
    nc.scalar.copy(Tv[:, :, 3], wl(WT, L - 1).transpose([0, 2, 1, 3]))

    # ---------------- P3b: seed A_0 = F0 ----------------
    sdv = SEED[:].rearrange("p (q j) -> p q j", q=3, j=J)
    cd0, cd1, ca0s = sdv[:, 0], sdv[:, 1], sdv[:, 2]
    sca0 = p_T.tile([P, J], F32, tag="sca0", name="sca0")[:]
    cca0 = p_T.tile([P, J], F32, tag="cca0", name="cca0")[:]
    nc.scalar.activation(sca0, ca0s, AF.Sin)
    nc.scalar.activation(cca0, ca0s, AF.Sin, bias=cst(-PI / 2))
    V3.memset(Av[:, 0], 0.0)
    nc.scalar.copy(Av[:, 0, 0, 0], cca0)
    nc.scalar.copy(Av[:, 0, 0, 1], sca0)
    V3.tensor_scalar_mul(Av[:, 0, 1, 0], sca0, -1.0)
    nc.scalar.copy(Av[:, 0, 1, 1], cca0)
    V3.memset(Av[:, 0, 2, 2], 1.0)
    tm0 = p_T.tile([P, J], F32, tag="tm0", name="tm0")[:]
    V3.tensor_mul(tm0, cd1, cca0)
    V3.tensor_add(Av[:, 0, 3, 0], cd0, tm0)
    V3.tensor_mul(Av[:, 0, 3, 1], cd1, sca0)

    # ---------------- P3c: chunk-prefix chain ----------------
    JD = 12  # sample-lane split: j 0..11 DVE, 12..15 GPSIMD (independent)
    for c in range(1, C):
        for EE, jl, jh, sfx in ((V3, 0, JD, "d"), (GP, JD, J, "g")):
            nj = jh - jl
            acm = [Av[:, c - 1, m, :, jl:jh].unsqueeze(1).broadcast_to(
                [P, 4, 3, nj]) for m in range(3)]
            trm = [Tv[:, c - 1, :, m, jl:jh].unsqueeze(2).broadcast_to(
                [P, 4, 3, nj]) for m in range(3)]

            def uu(tag):
                return p_u.tile([P, 12 * nj], F32, tag=tag + sfx,
                                name=tag + sfx)[:].rearrange(
                    "p (n m j) -> p n m j", n=4, m=3, j=nj)

            u1, u2, u3 = uu("u1"), uu("u2"), uu("u3")
            EE.tensor_mul(u1, acm[0], trm[0])
            EE.tensor_mul(u2, acm[1], trm[1])
            EE.tensor_add(u3, u1, u2)
            u1 = uu("u1")
            EE.tensor_mul(u1, acm[2], trm[2])
            EE.tensor_add(Av[:, c, :, :, jl:jh], u3, u1)
            EE.tensor_add(Av[:, c, 3, :, jl:jh], Av[:, c, 3, :, jl:jh],
                          Av[:, c - 1, 3, :, jl:jh])

    # close inner pools (LIFO): Tp/up, then c3, dd, trig1
    x_T.close()
    x_c3.close()
    x_dd.close()
    x_trig1.close()

    # ---------------- P4: apply -> backbone AoS ----------------
    x_bbaos = ExitStack()
    p_bb = x_bbaos.enter_context(tc.tile_pool(name="bbaos", bufs=1))
    BBAOS = p_bb.tile([P, J * NA * 3], F32, tag="BBAOS")

    def bbv():
        return BBAOS[:].rearrange("p (j a m) -> p j a m", j=J, a=NA, m=3)

    x_apply = ExitStack()
    p_ap = x_apply.enter_context(tc.tile_pool(name="applyp", bufs=2))
    wtv = WT[:].rearrange("p (m c l j) -> p m c l j", m=3, c=C, l=L, j=J)
    for c in range(C):
        # [p, comp, L, J] per chunk (3 free dims: walrus TENSOR3D limit)
        aw = [Av[:, c, m].unsqueeze(2).broadcast_to([P, 3, L, J])
              for m in range(3)]
        at = Av[:, c, 3].unsqueeze(2).broadcast_to([P, 3, L, J])
        wtm = [wtv[:, m, c].unsqueeze(1).broadcast_to([P, 3, L, J])
               for m in range(3)]

        EE = GP if c >= 10 else V3

        def qq(tag):
            return p_ap.tile([P, 3 * L * J], F16, tag=tag,
                             name=f"{tag}_{c}")[:].rearrange(
                "p (m l j) -> p m l j", m=3, l=L, j=J)

        q1, q2 = qq("q1"), qq("q2")
        EE.tensor_mul(q1, aw[0], wtm[0])
        EE.tensor_mul(q2, aw[1], wtm[1])
        EE.tensor_add(q1, q1, q2)
        q2 = qq("q2")
        EE.tensor_mul(q2, aw[2], wtm[2])
        EE.tensor_add(q1, q1, q2)
        outv = bbv()[:, :, 3 + c * L:3 + (c + 1) * L, :].transpose(
            [0, 3, 2, 1])
        EE.tensor_add(outv, q1, at)
    x_apply.close()

    # ---------------- P5: atoms 0,1,2 ----------------
    bb = bbv()
    V3.memset(bb[:, :, 0:2, :], 0.0)
    nc.scalar.copy(bb[:, :, 1, 0], cd0)
    V3.memset(bb[:, :, 2, 2], 0.0)
    nc.scalar.copy(bb[:, :, 2, 0], Av[:, 0, 3, 0])
    nc.scalar.copy(bb[:, :, 2, 1], Av[:, 0, 3, 1])

    # ---------------- P6: backbone DMA out + T0 gather ----------------
    outdv = OUT_d.ap().rearrange("p (j a m) -> p j a m", j=J, a=400, m=3)
    nc.sync.dma_start(outdv[:, :, 0:NB, :], bb[:, :, 0:NB, :])
    t0v = T0[:].rearrange("p (m r j) -> p m r j", m=3, r=N_RES, j=J)
    nc.scalar.copy(t0v, bb[:, :, 1:NB:3, :].transpose([0, 3, 2, 1]))
    x_bbaos.close()
    x_wt.close()

    # ---------------- P8: sidechain seeds ----------------
    x_seed = ExitStack()
    p_sd = x_seed.enter_context(tc.tile_pool(name="seedp", bufs=1))

    def rsv(t):
        return t[:].rearrange("p (m r j) -> p m r j", m=3, r=N_RES, j=J)

    AEX = [p_sd.tile([P, 3 * 49 * J], F16, tag=f"AEX{m}", name=f"AEX{m}")
           for m in range(3)]

    def aexv(t):
        return t[:].rearrange("p (m r j) -> p m r j", m=3, r=49, j=J)

    for m in range(3):
        dst = aexv(AEX[m])
        for cq in range(12):
            nc.scalar.copy(
                dst[:, :, 4 * cq:4 * cq + 4, :],
                Av[:, cq, m].unsqueeze(2).broadcast_to([P, 3, 4, J]))
        nc.scalar.copy(dst[:, :, 48, :], Av[:, 12, m])

    w1g = wv(WC1)[:, :, 1:146:3, :]
    w2g = wv(WC2)[:, :, 1:146:3, :]
    C2AW = p_sd.tile([P, 3 * 49 * J], F16, tag="C2AW")
    C3AW = p_sd.tile([P, 3 * 49 * J], F16, tag="C3AW")
    c2awv, c3awv = aexv(C2AW), aexv(C3AW)
    kj = lambda t: t[:].rearrange("p (k j) -> p k j", k=KP, j=J)
    cpb = _bcast_m(kj(CP)[:, 2:147:3, :])
    spb = _bcast_m(kj(SP)[:, 2:147:3, :])
    rs1a, rs2a, rs3a = rsv(RS_a[0]), rsv(RS_a[1]), rsv(RS_a[2])
    RSD = 37  # gather-index split: [0:RSD] DVE, [RSD:49] GPSIMD

    for EE, lo, hi, sfx in ((V3, 0, RSD, "d"), (GP, RSD, 49, "g")):
        n = hi - lo

        def sdt(tag):
            return p_sd.tile([P, 3 * n * J], F16, tag=tag + sfx,
                             name=tag + sfx)[:].rearrange(
                "p (m r j) -> p m r j", m=3, r=n, j=J)

        def rs(ap):
            return ap[:, :, lo:hi, :]

        def rsm(ap, m):
            return ap[:, m, lo:hi, :].unsqueeze(1).broadcast_to(
                [P, 3, n, J])

        sAv, sBv = sdt("sA"), sdt("sB")

        def apply_R(wg, outv):
            EE.tensor_mul(sAv, rs(aexv(AEX[0])), rsm(wg, 0))
            EE.tensor_mul(sBv, rs(aexv(AEX[1])), rsm(wg, 1))
            EE.tensor_add(sAv, sAv, sBv)
            EE.tensor_mul(sBv, rs(aexv(AEX[2])), rsm(wg, 2))
            EE.tensor_add(outv, sAv, sBv)

        apply_R(w1g, rs(rs1a[:, :, 1:, :]))
        apply_R(w2g, rs(c2awv))
        for mo in range(3):
            m1, m2 = (mo + 1) % 3, (mo + 2) % 3
            EE.tensor_mul(sAv[:, 0], rs1a[:, m1, 1 + lo:1 + hi, :],
                          c2awv[:, m2, lo:hi, :])
            EE.tensor_mul(sBv[:, 0], rs1a[:, m2, 1 + lo:1 + hi, :],
                          c2awv[:, m1, lo:hi, :])
            EE.tensor_sub(c3awv[:, mo, lo:hi, :], sAv[:, 0], sBv[:, 0])
        EE.tensor_mul(sAv, rs(cpb), rs(c2awv))
        EE.tensor_mul(sBv, rs(spb), rs(c3awv))
        EE.tensor_add(rs(rs2a[:, :, 1:, :]), sAv, sBv)
        EE.tensor_mul(sAv, rs(cpb), rs(c3awv))
        EE.tensor_mul(sBv, rs(spb), rs(c2awv))
        EE.tensor_sub(rs(rs3a[:, :, 1:, :]), sAv, sBv)

    for m in range(3):
        rm = rsv(RS_a[m])
        V3.memset(rm[:, :, 0, :], 0.0)
        V3.memset(rm[:, m, 0, :], 1.0)
    x_seed.close()
    x_w.close()

    # ---------------- P7: sidechain inputs + trig ----------------
    x_side2 = ExitStack()
    p_side2 = x_side2.enter_context(tc.tile_pool(name="side2", bufs=1))
    RS_b = [p_side2.tile([P, 3 * N_RES * J], F16, tag=f"RSb{i}",
                         name=f"RSb{i}") for i in range(3)]
    SIDEAOS = p_side2.tile([P, J * NS * 3], F32, tag="SIDEAOS")
    SD = p_side2.tile([P, 5 * N_RES * J], F16, tag="SDb")
    SCA = p_side2.tile([P, 5 * N_RES * J], F16, tag="SCA")
    SSA = p_side2.tile([P, 5 * N_RES * J], F16, tag="SSA")
    SCP = p_side2.tile([P, 5 * N_RES * J], F16, tag="SCP")
    SSP = p_side2.tile([P, 5 * N_RES * J], F16, tag="SSP")
    x_sin = ExitStack()
    p_si = x_sin.enter_context(tc.tile_pool(name="sidein", bufs=1))
    SAs = p_si.tile([P, 5 * N_RES * J], F16, tag="SAs")
    SPHs = p_si.tile([P, 5 * N_RES * J], F16, tag="SPHs")
    nc.sync.dma_start(SD[:], SD_d.ap())
    nc.sync.dma_start(SAs[:], SA_d.ap())
    nc.sync.dma_start(SPHs[:], SPH_d.ap())
    nc.scalar.activation(SCA[:], SAs[:], AF.Sin, bias=cst(-PI / 2))
    nc.scalar.activation(SSA[:], SAs[:], AF.Sin)
    nc.scalar.activation(SSP[:], SPHs[:], AF.Sin)
    nc.scalar.activation(SPHs[:], SPHs[:], AF.Abs)
    nc.scalar.activation(SCP[:], SPHs[:], AF.Sin, scale=-1.0,
                         bias=cst(PI / 2))
    x_sin.close()

    # ---------------- P9: sidechain serial chain ----------------
    sav = SIDEAOS[:].rearrange("p (j r v m) -> p j r v m",
                               j=J, r=N_RES, v=5, m=3)
    p_st = x_side2.enter_context(tc.tile_pool(name="sidetmp", bufs=3))

    def sl(t, v):
        s = t[:].rearrange("p (v r j) -> p v r j", v=5, r=N_RES, j=J)[:, v]
        return _bcast_m(s)

    cur, nxt = RS_a, RS_b
    RD = 40  # residues 0..RD-1 on DVE, RD..49 on GPSIMD (independent chains)
    for v in range(5):
        cpl, spl = sl(SCP, v), sl(SSP, v)
        cal, sal = sl(SCA, v), sl(SSA, v)
        sdl = sl(SD, v)
        r1p, r2p, r3p = rsv(cur[0]), rsv(cur[1]), rsv(cur[2])
        r1c, r2c, r3c = rsv(nxt[0]), rsv(nxt[1]), rsv(nxt[2])
        out = sav[:, :, :, v, :].transpose([0, 3, 2, 1])
        prev = (t0v if v == 0
                else sav[:, :, :, v - 1, :].transpose([0, 3, 2, 1]))

        for EE, lo, hi, sfx in ((V3, 0, RD, "d"), (GP, RD, N_RES, "g")):
            n = hi - lo

            def stp(tag):
                return p_st.tile([P, 3 * n * J], F16, tag=tag + sfx,
                                 name=tag + sfx)[:].rearrange(
                    "p (m r j) -> p m r j", m=3, r=n, j=J)

            def rs(ap):
                return ap[:, :, lo:hi, :]

            t1, t2, c2x = stp("s1"), stp("s2"), stp("sc2x")
            EE.tensor_mul(t1, rs(cpl), rs(r2p))
            EE.tensor_mul(t2, rs(spl), rs(r3p))
            EE.tensor_add(c2x, t1, t2)
            if v < 4:
                t1, t2 = stp("s1"), stp("s2")
                EE.tensor_mul(t1, rs(cpl), rs(r3p))
                EE.tensor_mul(t2, rs(spl), rs(r2p))
                EE.tensor_sub(rs(r3c), t1, t2)
            t1, t2 = stp("s1"), stp("s2")
            EE.tensor_mul(t1, rs(cal), rs(r1p))
            EE.tensor_mul(t2, rs(sal), c2x)
            EE.tensor_add(rs(r1c), t1, t2)
            if v < 4:
                t1, t2 = stp("s1"), stp("s2")
                EE.tensor_mul(t1, rs(cal), c2x)
                EE.tensor_mul(t2, rs(sal), rs(r1p))
                EE.tensor_sub(rs(r2c), t1, t2)
            t9 = stp("st9")
            EE.tensor_mul(t9, rs(sdl), rs(r1c))
            EE.tensor_add(rs(out), rs(prev), t9)
        cur, nxt = nxt, cur

    # ---------------- P10: sidechain DMA out ----------------
    sflat = SIDEAOS[:].rearrange("p (j x) -> p j x", j=J, x=NS * 3)
    dflat = OUT_d.ap().rearrange("p (j a m) -> p j a m", j=J, a=400, m=3)[
        :, :, NB:400, :].rearrange("p j a m -> p j (a m)")
    nc.sync.dma_start(dflat, sflat)

    x_side2.close()
    es.close()


def _build():
    if "nc" in _cache:
        return _cache["nc"]
    nc = bacc.Bacc("TRN2", target_bir_lowering=False, debug=False,
                   num_devices=NCORES)
    io = {
        "TH": nc.dram_tensor("TH", [P, KP * J], F16, kind="ExternalInput"),
        "PH": nc.dram_tensor("PH", [P, KP * J], F16, kind="ExternalInput"),
        "DD": nc.dram_tensor("DD", [P, KP * J], F16, kind="ExternalInput"),
        "SEED": nc.dram_tensor("SEED", [P, 48], F32, kind="ExternalInput"),
        "SD": nc.dram_tensor("SD", [P, 5 * N_RES * J], F16,
                             kind="ExternalInput"),
        "SA": nc.dram_tensor("SA", [P, 5 * N_RES * J], F16,
                             kind="ExternalInput"),
        "SPH": nc.dram_tensor("SPH", [P, 5 * N_RES * J], F16,
                              kind="ExternalInput"),
        "OUT": nc.dram_tensor("OUT", [P, J * 400 * 3], F32,
                              kind="ExternalOutput"),
    }
    with tile.TileContext(nc) as tc:
        _emit(tc, io)
    nc.compile()
    _cache["nc"] = nc
    return nc


def _marshal_core(cd, ca, ct, sd, sa, st):
    """Per-core (S samples) numpy repack into [128, free] layouts."""
    n = cd.shape[0]

    def tp(arr, ncol):  # (n, ncol) -> [128, ncol*16] k-major
        return np.ascontiguousarray(
            arr.reshape(P, J, ncol).transpose(0, 2, 1).reshape(P, ncol * J))

    th = np.full((n, KP), PI, np.float32)
    th[:, :K] = ca[:, 1:148]
    ph = np.zeros((n, KP), np.float32)
    ph[:, :K] = ct[:, :K]
    dd = np.zeros((n, KP), np.float32)
    dd[:, :K] = cd[:, 2:149]
    seed = np.stack([cd[:, 0], cd[:, 1], ca[:, 0]], axis=1)

    def tps(arr):  # (n, 50, 5) -> [128, 4000] (v, r, j)
        return np.ascontiguousarray(
            arr.reshape(P, J, N_RES, 5).transpose(0, 3, 2, 1).reshape(P, 4000))

    sph = np.empty((n, N_RES, 5), np.float32)
    sph[:, :, 0] = CHI0
    sph[:, :, 1:] = st.reshape(n, N_RES, V)
    return {
        "TH": tp(th, KP).astype(np.float16),
        "PH": tp(ph, KP).astype(np.float16),
        "DD": tp(dd, KP).astype(np.float16),
        "SEED": tp(seed, 3),
        "SD": tps(sd.reshape(n, N_RES, 5)).astype(np.float16),
        "SA": tps(sa.reshape(n, N_RES, 5)).astype(np.float16),
        "SPH": tps(sph).astype(np.float16),
    }


def kernel(central_distances, central_angles, central_dihedrals,
           side_distances, side_angles, side_dihedrals, **kw):
    nc = _build()
    in_maps = []
    for i in range(NCORES):
        sl_ = slice(i * S, (i + 1) * S)
        in_maps.append(_marshal_core(
            central_distances[sl_], central_angles[sl_],
            central_dihedrals[sl_], side_distances[sl_],
            side_angles[sl_], side_dihedrals[sl_]))
    res = run_bass_kernel_spmd(nc, in_maps, core_ids=list(range(NCORES)),
                               trace=TRACE, stitch_traces=False)
    _cache["last_results"] = res
    outs = []
    for i in range(NCORES):
        o = res.results[i]["OUT"].reshape(P, J, 400, 3).reshape(S, 400, 3)
        outs.append(o)
    return np.ascontiguousarray(np.concatenate(outs, axis=0))


# revision 37
# speedup vs baseline: 1.1285x; 1.0292x over previous
"""Trainium2 Bass kernel for nn_BackMapLayerWithSidechains.

Algorithm: the sequential NeRF atom-placement chain is reformulated as a
product of affine transforms (R, t): each step composes M = Rx(phi) @
Rz(pi - theta) and t += d * R[:, 0].  Composition is associative, so the
147-step backbone chain is evaluated with a blocked scan:
  - within-chunk prefixes (C=13 chunks x L=12 steps) computed serially but
    vectorized over (chunk, sample),
  - chunk prefixes composed serially (13 small composes),
  - one wide "apply" produces every global atom position.
Sidechain frames branch off backbone frames exactly:
  F_side(r) = F_global(3r-1) . Rx(phi_{3r-1})   (r >= 1; r=0 is identity at
  atom 1), then 5 more affine steps per residue, vectorized over
  (residue, sample).  Rotation state, trig tables, and inputs run in fp16
  (bounded values, 10-bit mantissa); global positions accumulate in fp32.
Serial phases split independent work (disjoint residues / sample lanes)
across DVE and GPSIMD; trig uses ACT Sin with the free affine; t-chain
updates ride GPSIMD (one-way dependency off the DVE critical path).

Data parallel over 8 NeuronCores: 2048 samples/core.  Host-side numpy only
reshapes/pads inputs into the on-chip layouts (no compute).
"""

import math
from contextlib import ExitStack

import numpy as np

import concourse.bacc as bacc
import concourse.bass as bass
import concourse.mybir as mybir
import concourse.tile as tile
from concourse.bass_utils import run_bass_kernel_spmd

F32 = mybir.dt.float32
BF16 = mybir.dt.bfloat16
F16 = mybir.dt.float16
AF = mybir.ActivationFunctionType

P = 128          # partitions
J = 16           # samples per partition (per core: 2048 = 128*16)
S = P * J        # samples per core
NCORES = 8
B = S * NCORES   # 16384

N_RES = 50
V = 4
NB = 150
NS = 250
K = 147          # real backbone steps
C = 13           # chunks
L = 12           # steps/chunk
KP = C * L       # 156 padded steps
NA = 159         # padded backbone atom count (3 + KP)
CHI0 = 2.0943951023931953
PI = math.pi

_cache = {}
TRACE = False


def _bcast_m(ap3):
    """[p, X, 16] -> [p, 3, X, 16] via 0-stride comp dim."""
    return ap3.unsqueeze(1).broadcast_to([P, 3] + list(ap3.shape[1:]))


def _emit(tc, io):
    nc = tc.nc
    V3 = nc.vector
    GP = nc.gpsimd

    TH_d, PH_d, DD_d, SEED_d, SD_d, SA_d, SPH_d, OUT_d = (
        io["TH"], io["PH"], io["DD"], io["SEED"], io["SD"], io["SA"],
        io["SPH"], io["OUT"])

    # Pools: strict LIFO release order.  Open order (outer->inner):
    # perm, side1, w, wt, trig1, dd, c3, thph
    es = ExitStack()
    p_perm = es.enter_context(tc.tile_pool(name="perm", bufs=1))
    p_side1 = es.enter_context(tc.tile_pool(name="side1", bufs=1))
    x_w = ExitStack()
    p_w = x_w.enter_context(tc.tile_pool(name="w", bufs=1))
    x_wt = ExitStack()
    p_wt = x_wt.enter_context(tc.tile_pool(name="wt", bufs=1))
    x_trig1 = ExitStack()
    p_trig1 = x_trig1.enter_context(tc.tile_pool(name="trig1", bufs=1))
    x_dd = ExitStack()
    p_dd = x_dd.enter_context(tc.tile_pool(name="dd", bufs=1))
    x_c3 = ExitStack()
    p_c3 = x_c3.enter_context(tc.tile_pool(name="c3", bufs=1))
    p_tmp = x_c3.enter_context(tc.tile_pool(name="wtmp", bufs=4))
    x_thph = ExitStack()
    p_thph = x_thph.enter_context(tc.tile_pool(name="thph", bufs=1))

    consts = {}

    def cst(val):
        if val not in consts:
            t = p_perm.tile([P, 1], F32, tag=f"cst{len(consts)}",
                            name=f"cst{len(consts)}")
            V3.memset(t[:], val)
            consts[val] = t[:]
        return consts[val]

    SEED = p_perm.tile([P, 48], F32, tag="SEED")
    CP = p_perm.tile([P, KP * J], F16, tag="CP")
    SP = p_perm.tile([P, KP * J], F16, tag="SP")
    A_all = p_perm.tile([P, C * 192], F32, tag="A_all")

    RS_a = [p_side1.tile([P, 3 * N_RES * J], F16, tag=f"RSa{i}",
                         name=f"RSa{i}") for i in range(3)]
    T0 = p_side1.tile([P, 3 * N_RES * J], F32, tag="T0")
    SD = p_side1.tile([P, 5 * N_RES * J], F16, tag="SDb")
    SCA = p_side1.tile([P, 5 * N_RES * J], F16, tag="SCA")
    SSA = p_side1.tile([P, 5 * N_RES * J], F16, tag="SSA")
    SCP = p_side1.tile([P, 5 * N_RES * J], F16, tag="SCP")
    SSP = p_side1.tile([P, 5 * N_RES * J], F16, tag="SSP")

    WC1 = p_w.tile([P, 3 * KP * J], F16, tag="WC1")
    WC2 = p_w.tile([P, 3 * KP * J], F16, tag="WC2")
    WT = p_wt.tile([P, 3 * KP * J], F16, tag="WT")
    CA_ = p_trig1.tile([P, KP * J], F16, tag="CA_")
    SA = p_trig1.tile([P, KP * J], F16, tag="SA")
    DD = p_dd.tile([P, KP * J], F16, tag="DD")
    SAs = p_dd.tile([P, 5 * N_RES * J], F16, tag="SAs")
    SPHs = p_dd.tile([P, 5 * N_RES * J], F16, tag="SPHs")
    c3t = [p_c3.tile([P, 3 * C * J], F16, tag=f"c3{i}", name=f"c3{i}")
           for i in range(2)]
    TH = p_thph.tile([P, KP * J], F16, tag="TH")
    PH = p_thph.tile([P, KP * J], F16, tag="PH")

    # ---------------- P0: input DMAs (backbone) ----------------
    nc.sync.dma_start(TH[:], TH_d.ap())
    nc.sync.dma_start(PH[:], PH_d.ap())
    nc.sync.dma_start(DD[:], DD_d.ap())
    nc.sync.dma_start(SEED[:], SEED_d.ap())
    nc.sync.dma_start(SD[:], SD_d.ap())
    nc.sync.dma_start(SAs[:], SA_d.ap())
    nc.sync.dma_start(SPHs[:], SPH_d.ap())

    # ---------------- P1: backbone trig (ACT), fp16 outputs ----------
    nc.scalar.activation(CA_[:], TH[:], AF.Sin, bias=cst(-PI / 2))
    nc.scalar.activation(SA[:], TH[:], AF.Sin)
    nc.scalar.activation(SP[:], PH[:], AF.Sin)
    nc.scalar.activation(PH[:], PH[:], AF.Abs)
    nc.scalar.activation(CP[:], PH[:], AF.Sin, scale=-1.0, bias=cst(PI / 2))
    x_thph.close()
    # sidechain trig early: overlaps the whole backbone on ACT
    nc.scalar.activation(SCA[:], SAs[:], AF.Sin, bias=cst(-PI / 2))
    nc.scalar.activation(SSA[:], SAs[:], AF.Sin)
    nc.scalar.activation(SSP[:], SPHs[:], AF.Sin)
    nc.scalar.activation(SPHs[:], SPHs[:], AF.Abs)
    nc.scalar.activation(SCP[:], SPHs[:], AF.Sin, scale=-1.0,
                         bias=cst(PI / 2))

    def wv(t):
        return t[:].rearrange("p (m k j) -> p m k j", m=3, k=KP, j=J)

    def wl(t, l):
        return t[:].rearrange("p (m c l j) -> p m c l j",
                              m=3, c=C, l=L, j=J)[:, :, :, l, :]

    def trig_l(t, l, bcast=True):
        s = t[:].rearrange("p (c l j) -> p c l j", c=C, l=L, j=J)[:, :, l, :]
        return _bcast_m(s) if bcast else s

    def c3v(i):
        return c3t[i][:].rearrange("p (m c j) -> p m c j", m=3, c=C, j=J)

    def anv(t):
        return t[:].rearrange("p (c n m j) -> p c n m j", c=C, n=4, m=3, j=J)

    Av = anv(A_all)

    # ---------------- P2a: within-chunk init (l = 0) ----------------
    w1_0, w2_0 = wl(WC1, 0), wl(WC2, 0)
    ca0, sa0 = trig_l(CA_, 0, False), trig_l(SA, 0, False)
    cp0, sp0 = trig_l(CP, 0, False), trig_l(SP, 0, False)
    nc.scalar.copy(w1_0[:, 0], ca0)
    V3.tensor_mul(w1_0[:, 1], sa0, cp0)
    V3.tensor_mul(w1_0[:, 2], sa0, sp0)
    V3.tensor_scalar_mul(w2_0[:, 0], sa0, -1.0)
    V3.tensor_mul(w2_0[:, 1], ca0, cp0)
    V3.tensor_mul(w2_0[:, 2], ca0, sp0)
    c3i = c3v(0)
    V3.memset(c3i[:, 0], 0.0)
    V3.tensor_scalar_mul(c3i[:, 1], sp0, -1.0)
    nc.scalar.copy(c3i[:, 2], cp0)
    GP.tensor_mul(wl(WT, 0), trig_l(DD, 0), wl(WC1, 0))

    # ---------------- P2b: within-chunk serial scan ----------------
    for l in range(1, L):
        cpl, spl = trig_l(CP, l), trig_l(SP, l)
        cal, sal = trig_l(CA_, l), trig_l(SA, l)
        p1, p2 = wl(WC1, l - 1), wl(WC2, l - 1)
        p3 = c3v((l - 1) % 2)
        c3c = c3v(l % 2)
        sh = [P, 3 * C * J]

        def tt(tag):
            return p_tmp.tile(sh, F16, tag=tag, name=tag)[:].rearrange(
                "p (m c j) -> p m c j", m=3, c=C, j=J)

        def ttf(tag):
            return p_tmp.tile(sh, F32, tag=tag, name=tag)[:].rearrange(
                "p (m c j) -> p m c j", m=3, c=C, j=J)

        t1, t2, c2x = tt("t1"), tt("t2"), tt("c2x")
        V3.tensor_mul(t1, cpl, p2)
        V3.tensor_mul(t2, spl, p3)
        V3.tensor_add(c2x, t1, t2)
        t1, t2 = tt("t1"), tt("t2")
        V3.tensor_mul(t1, cpl, p3)
        V3.tensor_mul(t2, spl, p2)
        V3.tensor_sub(c3c, t1, t2)
        t1, t2 = tt("t1"), tt("t2")
        V3.tensor_mul(t1, cal, p1)
        V3.tensor_mul(t2, sal, c2x)
        V3.tensor_add(wl(WC1, l), t1, t2)
        t1, t2 = tt("t1"), tt("t2")
        V3.tensor_mul(t1, cal, c2x)
        V3.tensor_mul(t2, sal, p1)
        V3.tensor_sub(wl(WC2, l), t1, t2)
        t9 = tt("t9")
        GP.tensor_mul(t9, trig_l(DD, l), wl(WC1, l))
        GP.tensor_add(wl(WT, l), wl(WT, l - 1), t9)

    # ---------------- P3a: chunk totals T ----------------
    # T lives in its own pool opened after closing c3/dd/trig1? T build
    # reads c3 -> keep c3 open until the copies below are emitted.
    x_T = ExitStack()
    p_T = x_T.enter_context(tc.tile_pool(name="Tp", bufs=1))
    p_u = x_T.enter_context(tc.tile_pool(name="up", bufs=2))
    T_all = p_T.tile([P, C * 192], F32, tag="T_all")
    Tv = anv(T_all)
    nc.scalar.copy(Tv[:, :, 0], wl(WC1, L - 1).transpose([0, 2, 1, 3]))
    nc.scalar.copy(Tv[:, :, 1], wl(WC2, L - 1).transpose([0, 2, 1, 3]))
    nc.scalar.copy(Tv[:, :, 2], c3v((L - 1) % 2).transpose([0, 2, 1, 3]))
    nc.scalar.copy(Tv[:, :, 3], wl(WT, L - 1).transpose([0, 2, 1, 3]))

    # ---------------- P3b: seed A_0 = F0 ----------------
    sdv = SEED[:].rearrange("p (q j) -> p q j", q=3, j=J)
    cd0, cd1, ca0s = sdv[:, 0], sdv[:, 1], sdv[:, 2]
    sca0 = p_T.tile([P, J], F32, tag="sca0", name="sca0")[:]
    cca0 = p_T.tile([P, J], F32, tag="cca0", name="cca0")[:]
    nc.scalar.activation(sca0, ca0s, AF.Sin)
    nc.scalar.activation(cca0, ca0s, AF.Sin, bias=cst(-PI / 2))
    V3.memset(Av[:, 0], 0.0)
    nc.scalar.copy(Av[:, 0, 0, 0], cca0)
    nc.scalar.copy(Av[:, 0, 0, 1], sca0)
    V3.tensor_scalar_mul(Av[:, 0, 1, 0], sca0, -1.0)
    nc.scalar.copy(Av[:, 0, 1, 1], cca0)
    V3.memset(Av[:, 0, 2, 2], 1.0)
    tm0 = p_T.tile([P, J], F32, tag="tm0", name="tm0")[:]
    V3.tensor_mul(tm0, cd1, cca0)
    V3.tensor_add(Av[:, 0, 3, 0], cd0, tm0)
    V3.tensor_mul(Av[:, 0, 3, 1], cd1, sca0)

    # ---------------- P3c: chunk-prefix chain ----------------
    JD = 12  # sample-lane split: j 0..11 DVE, 12..15 GPSIMD (independent)
    for c in range(1, C):
        for EE, jl, jh, sfx in ((V3, 0, JD, "d"), (GP, JD, J, "g")):
            nj = jh - jl
            acm = [Av[:, c - 1, m, :, jl:jh].unsqueeze(1).broadcast_to(
                [P, 4, 3, nj]) for m in range(3)]
            trm = [Tv[:, c - 1, :, m, jl:jh].unsqueeze(2).broadcast_to(
                [P, 4, 3, nj]) for m in range(3)]

            def uu(tag):
                return p_u.tile([P, 12 * nj], F32, tag=tag + sfx,
                                name=tag + sfx)[:].rearrange(
                    "p (n m j) -> p n m j", n=4, m=3, j=nj)

            u1, u2, u3 = uu("u1"), uu("u2"), uu("u3")
            EE.tensor_mul(u1, acm[0], trm[0])
            EE.tensor_mul(u2, acm[1], trm[1])
            EE.tensor_add(u3, u1, u2)
            u1 = uu("u1")
            EE.tensor_mul(u1, acm[2], trm[2])
            EE.tensor_add(Av[:, c, :, :, jl:jh], u3, u1)
            EE.tensor_add(Av[:, c, 3, :, jl:jh], Av[:, c, 3, :, jl:jh],
                          Av[:, c - 1, 3, :, jl:jh])

    # close inner pools (LIFO): Tp/up, then c3, dd, trig1
    x_T.close()
    x_c3.close()
    x_dd.close()
    x_trig1.close()

    # ---------------- P4: apply -> backbone AoS ----------------
    x_bbaos = ExitStack()
    p_bb = x_bbaos.enter_context(tc.tile_pool(name="bbaos", bufs=1))
    BBAOS = p_bb.tile([P, J * NA * 3], F32, tag="BBAOS")

    def bbv():
        return BBAOS[:].rearrange("p (j a m) -> p j a m", j=J, a=NA, m=3)

    x_apply = ExitStack()
    p_ap = x_apply.enter_context(tc.tile_pool(name="applyp", bufs=2))
    wtv = WT[:].rearrange("p (m c l j) -> p m c l j", m=3, c=C, l=L, j=J)
    for c in range(C):
        # [p, comp, L, J] per chunk (3 free dims: walrus TENSOR3D limit)
        aw = [Av[:, c, m].unsqueeze(2).broadcast_to([P, 3, L, J])
              for m in range(3)]
        at = Av[:, c, 3].unsqueeze(2).broadcast_to([P, 3, L, J])
        wtm = [wtv[:, m, c].unsqueeze(1).broadcast_to([P, 3, L, J])
               for m in range(3)]

        EE = GP if c >= 10 else V3

        def qq(tag):
            return p_ap.tile([P, 3 * L * J], F16, tag=tag,
                             name=f"{tag}_{c}")[:].rearrange(
                "p (m l j) -> p m l j", m=3, l=L, j=J)

        q1, q2 = qq("q1"), qq("q2")
        EE.tensor_mul(q1, aw[0], wtm[0])
        EE.tensor_mul(q2, aw[1], wtm[1])
        EE.tensor_add(q1, q1, q2)
        q2 = qq("q2")
        EE.tensor_mul(q2, aw[2], wtm[2])
        EE.tensor_add(q1, q1, q2)
        outv = bbv()[:, :, 3 + c * L:3 + (c + 1) * L, :].transpose(
            [0, 3, 2, 1])
        EE.tensor_add(outv, q1, at)
    x_apply.close()

    # ---------------- P5: atoms 0,1,2 ----------------
    bb = bbv()
    V3.memset(bb[:, :, 0:2, :], 0.0)
    nc.scalar.copy(bb[:, :, 1, 0], cd0)
    V3.memset(bb[:, :, 2, 2], 0.0)
    nc.scalar.copy(bb[:, :, 2, 0], Av[:, 0, 3, 0])
    nc.scalar.copy(bb[:, :, 2, 1], Av[:, 0, 3, 1])

    # ---------------- P6: backbone DMA out + T0 gather ----------------
    outdv = OUT_d.ap().rearrange("p (j a m) -> p j a m", j=J, a=400, m=3)
    nc.sync.dma_start(outdv[:, :, 0:NB, :], bb[:, :, 0:NB, :])
    t0v = T0[:].rearrange("p (m r j) -> p m r j", m=3, r=N_RES, j=J)
    nc.scalar.copy(t0v, bb[:, :, 1:NB:3, :].transpose([0, 3, 2, 1]))
    x_bbaos.close()
    x_wt.close()

    # ---------------- P8: sidechain seeds ----------------
    x_seed = ExitStack()
    p_sd = x_seed.enter_context(tc.tile_pool(name="seedp", bufs=1))

    def rsv(t):
        return t[:].rearrange("p (m r j) -> p m r j", m=3, r=N_RES, j=J)

    AEX = [p_sd.tile([P, 3 * 49 * J], F16, tag=f"AEX{m}", name=f"AEX{m}")
           for m in range(3)]

    def aexv(t):
        return t[:].rearrange("p (m r j) -> p m r j", m=3, r=49, j=J)

    for m in range(3):
        dst = aexv(AEX[m])
        for cq in range(12):
            nc.scalar.copy(
                dst[:, :, 4 * cq:4 * cq + 4, :],
                Av[:, cq, m].unsqueeze(2).broadcast_to([P, 3, 4, J]))
        nc.scalar.copy(dst[:, :, 48, :], Av[:, 12, m])

    w1g = wv(WC1)[:, :, 1:146:3, :]
    w2g = wv(WC2)[:, :, 1:146:3, :]
    C2AW = p_sd.tile([P, 3 * 49 * J], F16, tag="C2AW")
    C3AW = p_sd.tile([P, 3 * 49 * J], F16, tag="C3AW")
    c2awv, c3awv = aexv(C2AW), aexv(C3AW)
    kj = lambda t: t[:].rearrange("p (k j) -> p k j", k=KP, j=J)
    cpb = _bcast_m(kj(CP)[:, 2:147:3, :])
    spb = _bcast_m(kj(SP)[:, 2:147:3, :])
    rs1a, rs2a, rs3a = rsv(RS_a[0]), rsv(RS_a[1]), rsv(RS_a[2])
    RSD = 37  # gather-index split: [0:RSD] DVE, [RSD:49] GPSIMD

    for EE, lo, hi, sfx in ((V3, 0, RSD, "d"), (GP, RSD, 49, "g")):
        n = hi - lo

        def sdt(tag):
            return p_sd.tile([P, 3 * n * J], F16, tag=tag + sfx,
                             name=tag + sfx)[:].rearrange(
                "p (m r j) -> p m r j", m=3, r=n, j=J)

        def rs(ap):
            return ap[:, :, lo:hi, :]

        def rsm(ap, m):
            return ap[:, m, lo:hi, :].unsqueeze(1).broadcast_to(
                [P, 3, n, J])

        sAv, sBv = sdt("sA"), sdt("sB")

        def apply_R(wg, outv):
            EE.tensor_mul(sAv, rs(aexv(AEX[0])), rsm(wg, 0))
            EE.tensor_mul(sBv, rs(aexv(AEX[1])), rsm(wg, 1))
            EE.tensor_add(sAv, sAv, sBv)
            EE.tensor_mul(sBv, rs(aexv(AEX[2])), rsm(wg, 2))
            EE.tensor_add(outv, sAv, sBv)

        apply_R(w1g, rs(rs1a[:, :, 1:, :]))
        apply_R(w2g, rs(c2awv))
        for mo in range(3):
            m1, m2 = (mo + 1) % 3, (mo + 2) % 3
            EE.tensor_mul(sAv[:, 0], rs1a[:, m1, 1 + lo:1 + hi, :],
                          c2awv[:, m2, lo:hi, :])
            EE.tensor_mul(sBv[:, 0], rs1a[:, m2, 1 + lo:1 + hi, :],
                          c2awv[:, m1, lo:hi, :])
            EE.tensor_sub(c3awv[:, mo, lo:hi, :], sAv[:, 0], sBv[:, 0])
        EE.tensor_mul(sAv, rs(cpb), rs(c2awv))
        EE.tensor_mul(sBv, rs(spb), rs(c3awv))
        EE.tensor_add(rs(rs2a[:, :, 1:, :]), sAv, sBv)
        EE.tensor_mul(sAv, rs(cpb), rs(c3awv))
        EE.tensor_mul(sBv, rs(spb), rs(c2awv))
        EE.tensor_sub(rs(rs3a[:, :, 1:, :]), sAv, sBv)

    for m in range(3):
        rm = rsv(RS_a[m])
        V3.memset(rm[:, :, 0, :], 0.0)
        V3.memset(rm[:, m, 0, :], 1.0)
    x_seed.close()
    x_w.close()

    # ---------------- P7: sidechain inputs + trig ----------------
    x_side2 = ExitStack()
    p_side2 = x_side2.enter_context(tc.tile_pool(name="side2", bufs=1))
    RS_b = [p_side2.tile([P, 3 * N_RES * J], F16, tag=f"RSb{i}",
                         name=f"RSb{i}") for i in range(3)]
    SIDEAOS = p_side2.tile([P, J * NS * 3], F32, tag="SIDEAOS")

    # ---------------- P9: sidechain serial chain ----------------
    sav = SIDEAOS[:].rearrange("p (j r v m) -> p j r v m",
                               j=J, r=N_RES, v=5, m=3)
    p_st = x_side2.enter_context(tc.tile_pool(name="sidetmp", bufs=3))

    def sl(t, v):
        s = t[:].rearrange("p (v r j) -> p v r j", v=5, r=N_RES, j=J)[:, v]
        return _bcast_m(s)

    cur, nxt = RS_a, RS_b
    RD = 40  # residues 0..RD-1 on DVE, RD..49 on GPSIMD (independent chains)
    for v in range(5):
        cpl, spl = sl(SCP, v), sl(SSP, v)
        cal, sal = sl(SCA, v), sl(SSA, v)
        sdl = sl(SD, v)
        r1p, r2p, r3p = rsv(cur[0]), rsv(cur[1]), rsv(cur[2])
        r1c, r2c, r3c = rsv(nxt[0]), rsv(nxt[1]), rsv(nxt[2])
        out = sav[:, :, :, v, :].transpose([0, 3, 2, 1])
        prev = (t0v if v == 0
                else sav[:, :, :, v - 1, :].transpose([0, 3, 2, 1]))

        for EE, lo, hi, sfx in ((V3, 0, RD, "d"), (GP, RD, N_RES, "g")):
            n = hi - lo

            def stp(tag):
                return p_st.tile([P, 3 * n * J], F16, tag=tag + sfx,
                                 name=tag + sfx)[:].rearrange(
                    "p (m r j) -> p m r j", m=3, r=n, j=J)

            def rs(ap):
                return ap[:, :, lo:hi, :]

            t1, t2, c2x = stp("s1"), stp("s2"), stp("sc2x")
            EE.tensor_mul(t1, rs(cpl), rs(r2p))
            EE.tensor_mul(t2, rs(spl), rs(r3p))
            EE.tensor_add(c2x, t1, t2)
            if v < 4:
                t1, t2 = stp("s1"), stp("s2")
                EE.tensor_mul(t1, rs(cpl), rs(r3p))
                EE.tensor_mul(t2, rs(spl), rs(r2p))
                EE.tensor_sub(rs(r3c), t1, t2)
            t1, t2 = stp("s1"), stp("s2")
            EE.tensor_mul(t1, rs(cal), rs(r1p))
            EE.tensor_mul(t2, rs(sal), c2x)
            EE.tensor_add(rs(r1c), t1, t2)
            if v < 4:
                t1, t2 = stp("s1"), stp("s2")
                EE.tensor_mul(t1, rs(cal), c2x)
                EE.tensor_mul(t2, rs(sal), rs(r1p))
                EE.tensor_sub(rs(r2c), t1, t2)
            t9 = stp("st9")
            EE.tensor_mul(t9, rs(sdl), rs(r1c))
            EE.tensor_add(rs(out), rs(prev), t9)
        cur, nxt = nxt, cur

    # ---------------- P10: sidechain DMA out ----------------
    sflat = SIDEAOS[:].rearrange("p (j x) -> p j x", j=J, x=NS * 3)
    dflat = OUT_d.ap().rearrange("p (j a m) -> p j a m", j=J, a=400, m=3)[
        :, :, NB:400, :].rearrange("p j a m -> p j (a m)")
    nc.sync.dma_start(dflat, sflat)

    x_side2.close()
    es.close()


def _build():
    if "nc" in _cache:
        return _cache["nc"]
    nc = bacc.Bacc("TRN2", target_bir_lowering=False, debug=False,
                   num_devices=NCORES)
    io = {
        "TH": nc.dram_tensor("TH", [P, KP * J], F16, kind="ExternalInput"),
        "PH": nc.dram_tensor("PH", [P, KP * J], F16, kind="ExternalInput"),
        "DD": nc.dram_tensor("DD", [P, KP * J], F16, kind="ExternalInput"),
        "SEED": nc.dram_tensor("SEED", [P, 48], F32, kind="ExternalInput"),
        "SD": nc.dram_tensor("SD", [P, 5 * N_RES * J], F16,
                             kind="ExternalInput"),
        "SA": nc.dram_tensor("SA", [P, 5 * N_RES * J], F16,
                             kind="ExternalInput"),
        "SPH": nc.dram_tensor("SPH", [P, 5 * N_RES * J], F16,
                              kind="ExternalInput"),
        "OUT": nc.dram_tensor("OUT", [P, J * 400 * 3], F32,
                              kind="ExternalOutput"),
    }
    with tile.TileContext(nc) as tc:
        _emit(tc, io)
    nc.compile()
    _cache["nc"] = nc
    return nc


def _marshal_core(cd, ca, ct, sd, sa, st):
    """Per-core (S samples) numpy repack into [128, free] layouts."""
    n = cd.shape[0]

    def tp(arr, ncol):  # (n, ncol) -> [128, ncol*16] k-major
        return np.ascontiguousarray(
            arr.reshape(P, J, ncol).transpose(0, 2, 1).reshape(P, ncol * J))

    th = np.full((n, KP), PI, np.float32)
    th[:, :K] = ca[:, 1:148]
    ph = np.zeros((n, KP), np.float32)
    ph[:, :K] = ct[:, :K]
    dd = np.zeros((n, KP), np.float32)
    dd[:, :K] = cd[:, 2:149]
    seed = np.stack([cd[:, 0], cd[:, 1], ca[:, 0]], axis=1)

    def tps(arr):  # (n, 50, 5) -> [128, 4000] (v, r, j)
        return np.ascontiguousarray(
            arr.reshape(P, J, N_RES, 5).transpose(0, 3, 2, 1).reshape(P, 4000))

    sph = np.empty((n, N_RES, 5), np.float32)
    sph[:, :, 0] = CHI0
    sph[:, :, 1:] = st.reshape(n, N_RES, V)
    return {
        "TH": tp(th, KP).astype(np.float16),
        "PH": tp(ph, KP).astype(np.float16),
        "DD": tp(dd, KP).astype(np.float16),
        "SEED": tp(seed, 3),
        "SD": tps(sd.reshape(n, N_RES, 5)).astype(np.float16),
        "SA": tps(sa.reshape(n, N_RES, 5)).astype(np.float16),
        "SPH": tps(sph).astype(np.float16),
    }


def kernel(central_distances, central_angles, central_dihedrals,
           side_distances, side_angles, side_dihedrals, **kw):
    nc = _build()
    in_maps = []
    for i in range(NCORES):
        sl_ = slice(i * S, (i + 1) * S)
        in_maps.append(_marshal_core(
            central_distances[sl_], central_angles[sl_],
            central_dihedrals[sl_], side_distances[sl_],
            side_angles[sl_], side_dihedrals[sl_]))
    res = run_bass_kernel_spmd(nc, in_maps, core_ids=list(range(NCORES)),
                               trace=TRACE, stitch_traces=False)
    _cache["last_results"] = res
    outs = []
    for i in range(NCORES):
        o = res.results[i]["OUT"].reshape(P, J, 400, 3).reshape(S, 400, 3)
        outs.append(o)
    return np.ascontiguousarray(np.concatenate(outs, axis=0))


# revision 38
# speedup vs baseline: 1.1719x; 1.0384x over previous
"""Trainium2 Bass kernel for nn_BackMapLayerWithSidechains.

Algorithm: the sequential NeRF atom-placement chain is reformulated as a
product of affine transforms (R, t): each step composes M = Rx(phi) @
Rz(pi - theta) and t += d * R[:, 0].  Composition is associative, so the
147-step backbone chain is evaluated with a blocked scan:
  - within-chunk prefixes (C=13 chunks x L=12 steps) computed serially but
    vectorized over (chunk, sample),
  - chunk prefixes composed serially (13 small composes),
  - one wide "apply" produces every global atom position.
Sidechain frames branch off backbone frames exactly:
  F_side(r) = F_global(3r-1) . Rx(phi_{3r-1})   (r >= 1; r=0 is identity at
  atom 1), then 5 more affine steps per residue, vectorized over
  (residue, sample).  Rotation state, trig tables, and inputs run in fp16
  (bounded values, 10-bit mantissa); global positions accumulate in fp32.
Serial phases split independent work (disjoint residues / sample lanes)
across DVE and GPSIMD; trig uses ACT Sin with the free affine; t-chain
updates ride GPSIMD (one-way dependency off the DVE critical path).

Data parallel over 8 NeuronCores: 2048 samples/core.  Host-side numpy only
reshapes/pads inputs into the on-chip layouts (no compute).
"""

import math
from contextlib import ExitStack

import numpy as np

import concourse.bacc as bacc
import concourse.bass as bass
import concourse.mybir as mybir
import concourse.tile as tile
from concourse.bass_utils import run_bass_kernel_spmd

F32 = mybir.dt.float32
BF16 = mybir.dt.bfloat16
F16 = mybir.dt.float16
AF = mybir.ActivationFunctionType

P = 128          # partitions
J = 16           # samples per partition (per core: 2048 = 128*16)
S = P * J        # samples per core
NCORES = 8
B = S * NCORES   # 16384

N_RES = 50
V = 4
NB = 150
NS = 250
K = 147          # real backbone steps
C = 13           # chunks
L = 12           # steps/chunk
KP = C * L       # 156 padded steps
NA = 159         # padded backbone atom count (3 + KP)
CHI0 = 2.0943951023931953
PI = math.pi

_cache = {}
TRACE = False


def _bcast_m(ap3):
    """[p, X, 16] -> [p, 3, X, 16] via 0-stride comp dim."""
    return ap3.unsqueeze(1).broadcast_to([P, 3] + list(ap3.shape[1:]))


def _emit(tc, io):
    nc = tc.nc
    V3 = nc.vector
    GP = nc.gpsimd

    TH_d, PH_d, DD_d, SEED_d, SD_d, SA_d, SPH_d, OUT_d = (
        io["TH"], io["PH"], io["DD"], io["SEED"], io["SD"], io["SA"],
        io["SPH"], io["OUT"])

    # Pools: strict LIFO release order.  Open order (outer->inner):
    # perm, side1, w, wt, trig1, dd, c3, thph
    es = ExitStack()
    p_perm = es.enter_context(tc.tile_pool(name="perm", bufs=1))
    p_side1 = es.enter_context(tc.tile_pool(name="side1", bufs=1))
    x_w = ExitStack()
    p_w = x_w.enter_context(tc.tile_pool(name="w", bufs=1))
    x_wt = ExitStack()
    p_wt = x_wt.enter_context(tc.tile_pool(name="wt", bufs=1))
    x_trig1 = ExitStack()
    p_trig1 = x_trig1.enter_context(tc.tile_pool(name="trig1", bufs=1))
    x_dd = ExitStack()
    p_dd = x_dd.enter_context(tc.tile_pool(name="dd", bufs=1))
    x_c3 = ExitStack()
    p_c3 = x_c3.enter_context(tc.tile_pool(name="c3", bufs=1))
    p_tmp = x_c3.enter_context(tc.tile_pool(name="wtmp", bufs=4))
    x_thph = ExitStack()
    p_thph = x_thph.enter_context(tc.tile_pool(name="thph", bufs=1))

    consts = {}

    def cst(val):
        if val not in consts:
            t = p_perm.tile([P, 1], F32, tag=f"cst{len(consts)}",
                            name=f"cst{len(consts)}")
            V3.memset(t[:], val)
            consts[val] = t[:]
        return consts[val]

    SEED = p_perm.tile([P, 48], F32, tag="SEED")
    CP = p_perm.tile([P, KP * J], F16, tag="CP")
    SP = p_perm.tile([P, KP * J], F16, tag="SP")
    A_all = p_perm.tile([P, C * 192], F32, tag="A_all")

    RS_a = [p_side1.tile([P, 3 * N_RES * J], F16, tag=f"RSa{i}",
                         name=f"RSa{i}") for i in range(3)]
    T0 = p_side1.tile([P, 3 * N_RES * J], F32, tag="T0")
    SD = p_side1.tile([P, 5 * N_RES * J], F16, tag="SDb")
    SCA = p_side1.tile([P, 5 * N_RES * J], F16, tag="SCA")
    SSA = p_side1.tile([P, 5 * N_RES * J], F16, tag="SSA")
    SCP = p_side1.tile([P, 5 * N_RES * J], F16, tag="SCP")
    SSP = p_side1.tile([P, 5 * N_RES * J], F16, tag="SSP")

    WC1 = p_w.tile([P, 3 * KP * J], F16, tag="WC1")
    WC2 = p_w.tile([P, 3 * KP * J], F16, tag="WC2")
    WT = p_wt.tile([P, 3 * KP * J], F16, tag="WT")
    CA_ = p_trig1.tile([P, KP * J], F16, tag="CA_")
    SA = p_trig1.tile([P, KP * J], F16, tag="SA")
    DD = p_dd.tile([P, KP * J], F16, tag="DD")
    SAs = p_dd.tile([P, 5 * N_RES * J], F16, tag="SAs")
    SPHs = p_dd.tile([P, 5 * N_RES * J], F16, tag="SPHs")
    c3t = [p_c3.tile([P, 3 * C * J], F16, tag=f"c3{i}", name=f"c3{i}")
           for i in range(2)]
    TH = p_thph.tile([P, KP * J], F16, tag="TH")
    PH = p_thph.tile([P, KP * J], F16, tag="PH")

    # ---------------- P0: input DMAs (backbone) ----------------
    nc.sync.dma_start(TH[:], TH_d.ap())
    nc.sync.dma_start(PH[:], PH_d.ap())
    nc.sync.dma_start(DD[:], DD_d.ap())
    nc.sync.dma_start(SEED[:], SEED_d.ap())
    nc.sync.dma_start(SD[:], SD_d.ap())
    nc.sync.dma_start(SAs[:], SA_d.ap())
    nc.sync.dma_start(SPHs[:], SPH_d.ap())

    # ---------------- P1: backbone trig (ACT), fp16 outputs ----------
    nc.scalar.activation(CA_[:], TH[:], AF.Sin, bias=cst(-PI / 2))
    nc.scalar.activation(SA[:], TH[:], AF.Sin)
    nc.scalar.activation(SP[:], PH[:], AF.Sin)
    nc.scalar.activation(PH[:], PH[:], AF.Abs)
    nc.scalar.activation(CP[:], PH[:], AF.Sin, scale=-1.0, bias=cst(PI / 2))
    x_thph.close()
    # sidechain trig early: overlaps the whole backbone on ACT
    nc.scalar.activation(SCA[:], SAs[:], AF.Sin, bias=cst(-PI / 2))
    nc.scalar.activation(SSA[:], SAs[:], AF.Sin)
    nc.scalar.activation(SSP[:], SPHs[:], AF.Sin)
    nc.scalar.activation(SPHs[:], SPHs[:], AF.Abs)
    nc.scalar.activation(SCP[:], SPHs[:], AF.Sin, scale=-1.0,
                         bias=cst(PI / 2))

    def wv(t):
        return t[:].rearrange("p (m k j) -> p m k j", m=3, k=KP, j=J)

    def wl(t, l):
        return t[:].rearrange("p (m c l j) -> p m c l j",
                              m=3, c=C, l=L, j=J)[:, :, :, l, :]

    def trig_l(t, l, bcast=True):
        s = t[:].rearrange("p (c l j) -> p c l j", c=C, l=L, j=J)[:, :, l, :]
        return _bcast_m(s) if bcast else s

    def c3v(i):
        return c3t[i][:].rearrange("p (m c j) -> p m c j", m=3, c=C, j=J)

    def anv(t):
        return t[:].rearrange("p (c n m j) -> p c n m j", c=C, n=4, m=3, j=J)

    Av = anv(A_all)

    # ---------------- P2a: within-chunk init (l = 0) ----------------
    w1_0, w2_0 = wl(WC1, 0), wl(WC2, 0)
    ca0, sa0 = trig_l(CA_, 0, False), trig_l(SA, 0, False)
    cp0, sp0 = trig_l(CP, 0, False), trig_l(SP, 0, False)
    nc.scalar.copy(w1_0[:, 0], ca0)
    V3.tensor_mul(w1_0[:, 1], sa0, cp0)
    V3.tensor_mul(w1_0[:, 2], sa0, sp0)
    V3.tensor_scalar_mul(w2_0[:, 0], sa0, -1.0)
    V3.tensor_mul(w2_0[:, 1], ca0, cp0)
    V3.tensor_mul(w2_0[:, 2], ca0, sp0)
    c3i = c3v(0)
    V3.memset(c3i[:, 0], 0.0)
    V3.tensor_scalar_mul(c3i[:, 1], sp0, -1.0)
    nc.scalar.copy(c3i[:, 2], cp0)
    GP.tensor_mul(wl(WT, 0), trig_l(DD, 0), wl(WC1, 0))

    # ---------------- P2b: within-chunk serial scan ----------------
    for l in range(1, L):
        cpl, spl = trig_l(CP, l), trig_l(SP, l)
        cal, sal = trig_l(CA_, l), trig_l(SA, l)
        p1, p2 = wl(WC1, l - 1), wl(WC2, l - 1)
        p3 = c3v((l - 1) % 2)
        c3c = c3v(l % 2)
        sh = [P, 3 * C * J]

        def tt(tag):
            return p_tmp.tile(sh, F16, tag=tag, name=tag)[:].rearrange(
                "p (m c j) -> p m c j", m=3, c=C, j=J)

        def ttf(tag):
            return p_tmp.tile(sh, F32, tag=tag, name=tag)[:].rearrange(
                "p (m c j) -> p m c j", m=3, c=C, j=J)

        t1, t2, c2x = tt("t1"), tt("t2"), tt("c2x")
        V3.tensor_mul(t1, cpl, p2)
        V3.tensor_mul(t2, spl, p3)
        V3.tensor_add(c2x, t1, t2)
        t1, t2 = tt("t1"), tt("t2")
        V3.tensor_mul(t1, cpl, p3)
        V3.tensor_mul(t2, spl, p2)
        V3.tensor_sub(c3c, t1, t2)
        t1, t2 = tt("t1"), tt("t2")
        V3.tensor_mul(t1, cal, p1)
        V3.tensor_mul(t2, sal, c2x)
        V3.tensor_add(wl(WC1, l), t1, t2)
        t1, t2 = tt("t1"), tt("t2")
        V3.tensor_mul(t1, cal, c2x)
        V3.tensor_mul(t2, sal, p1)
        V3.tensor_sub(wl(WC2, l), t1, t2)
        t9 = tt("t9")
        GP.tensor_mul(t9, trig_l(DD, l), wl(WC1, l))
        GP.tensor_add(wl(WT, l), wl(WT, l - 1), t9)

    # ---------------- P3a: chunk totals T ----------------
    # T lives in its own pool opened after closing c3/dd/trig1? T build
    # reads c3 -> keep c3 open until the copies below are emitted.
    x_T = ExitStack()
    p_T = x_T.enter_context(tc.tile_pool(name="Tp", bufs=1))
    p_u = x_T.enter_context(tc.tile_pool(name="up", bufs=2))
    T_all = p_T.tile([P, C * 192], F32, tag="T_all")
    Tv = anv(T_all)
    w1f, w2f = wl(WC1, L - 1), wl(WC2, L - 1)
    c3f, wtf = c3v((L - 1) % 2), wl(WT, L - 1)
    for c_ in range(C):
        nc.scalar.copy(Tv[:, c_, 0], w1f[:, :, c_, :])
        nc.scalar.copy(Tv[:, c_, 1], w2f[:, :, c_, :])
        nc.scalar.copy(Tv[:, c_, 2], c3f[:, :, c_, :])
        nc.scalar.copy(Tv[:, c_, 3], wtf[:, :, c_, :])

    # ---------------- P3b: seed A_0 = F0 ----------------
    sdv = SEED[:].rearrange("p (q j) -> p q j", q=3, j=J)
    cd0, cd1, ca0s = sdv[:, 0], sdv[:, 1], sdv[:, 2]
    sca0 = p_T.tile([P, J], F32, tag="sca0", name="sca0")[:]
    cca0 = p_T.tile([P, J], F32, tag="cca0", name="cca0")[:]
    nc.scalar.activation(sca0, ca0s, AF.Sin)
    nc.scalar.activation(cca0, ca0s, AF.Sin, bias=cst(-PI / 2))
    V3.memset(Av[:, 0], 0.0)
    nc.scalar.copy(Av[:, 0, 0, 0], cca0)
    nc.scalar.copy(Av[:, 0, 0, 1], sca0)
    V3.tensor_scalar_mul(Av[:, 0, 1, 0], sca0, -1.0)
    nc.scalar.copy(Av[:, 0, 1, 1], cca0)
    V3.memset(Av[:, 0, 2, 2], 1.0)
    tm0 = p_T.tile([P, J], F32, tag="tm0", name="tm0")[:]
    V3.tensor_mul(tm0, cd1, cca0)
    V3.tensor_add(Av[:, 0, 3, 0], cd0, tm0)
    V3.tensor_mul(Av[:, 0, 3, 1], cd1, sca0)

    # ---------------- P3c: chunk-prefix chain ----------------
    JD = 12  # sample-lane split: j 0..11 DVE, 12..15 GPSIMD (independent)
    for c in range(1, C):
        for EE, jl, jh, sfx in ((V3, 0, JD, "d"), (GP, JD, J, "g")):
            nj = jh - jl
            acm = [Av[:, c - 1, m, :, jl:jh].unsqueeze(1).broadcast_to(
                [P, 4, 3, nj]) for m in range(3)]
            trm = [Tv[:, c - 1, :, m, jl:jh].unsqueeze(2).broadcast_to(
                [P, 4, 3, nj]) for m in range(3)]

            def uu(tag):
                return p_u.tile([P, 12 * nj], F32, tag=tag + sfx,
                                name=tag + sfx)[:].rearrange(
                    "p (n m j) -> p n m j", n=4, m=3, j=nj)

            u1, u2, u3 = uu("u1"), uu("u2"), uu("u3")
            EE.tensor_mul(u1, acm[0], trm[0])
            EE.tensor_mul(u2, acm[1], trm[1])
            EE.tensor_add(u3, u1, u2)
            u1 = uu("u1")
            EE.tensor_mul(u1, acm[2], trm[2])
            EE.tensor_add(Av[:, c, :, :, jl:jh], u3, u1)
            EE.tensor_add(Av[:, c, 3, :, jl:jh], Av[:, c, 3, :, jl:jh],
                          Av[:, c - 1, 3, :, jl:jh])

    # close inner pools (LIFO): Tp/up, then c3, dd, trig1
    x_T.close()
    x_c3.close()
    x_dd.close()
    x_trig1.close()

    # ---------------- P4: apply -> backbone AoS ----------------
    x_bbaos = ExitStack()
    p_bb = x_bbaos.enter_context(tc.tile_pool(name="bbaos", bufs=1))
    BBAOS = p_bb.tile([P, J * NA * 3], F32, tag="BBAOS")

    def bbv():
        return BBAOS[:].rearrange("p (j a m) -> p j a m", j=J, a=NA, m=3)

    x_apply = ExitStack()
    p_ap = x_apply.enter_context(tc.tile_pool(name="applyp", bufs=2))
    wtv = WT[:].rearrange("p (m c l j) -> p m c l j", m=3, c=C, l=L, j=J)
    for c in range(C):
        # [p, comp, L, J] per chunk (3 free dims: walrus TENSOR3D limit)
        aw = [Av[:, c, m].unsqueeze(2).broadcast_to([P, 3, L, J])
              for m in range(3)]
        at = Av[:, c, 3].unsqueeze(2).broadcast_to([P, 3, L, J])
        wtm = [wtv[:, m, c].unsqueeze(1).broadcast_to([P, 3, L, J])
               for m in range(3)]

        EE = GP if c >= 10 else V3

        def qq(tag):
            return p_ap.tile([P, 3 * L * J], F16, tag=tag,
                             name=f"{tag}_{c}")[:].rearrange(
                "p (m l j) -> p m l j", m=3, l=L, j=J)

        q1, q2 = qq("q1"), qq("q2")
        EE.tensor_mul(q1, aw[0], wtm[0])
        EE.tensor_mul(q2, aw[1], wtm[1])
        EE.tensor_add(q1, q1, q2)
        q2 = qq("q2")
        EE.tensor_mul(q2, aw[2], wtm[2])
        EE.tensor_add(q1, q1, q2)
        outv = bbv()[:, :, 3 + c * L:3 + (c + 1) * L, :].transpose(
            [0, 3, 2, 1])
        EE.tensor_add(outv, q1, at)
    x_apply.close()

    # ---------------- P5: atoms 0,1,2 ----------------
    bb = bbv()
    V3.memset(bb[:, :, 0:2, :], 0.0)
    nc.scalar.copy(bb[:, :, 1, 0], cd0)
    V3.memset(bb[:, :, 2, 2], 0.0)
    nc.scalar.copy(bb[:, :, 2, 0], Av[:, 0, 3, 0])
    nc.scalar.copy(bb[:, :, 2, 1], Av[:, 0, 3, 1])

    # ---------------- P6: backbone DMA out + T0 gather ----------------
    outdv = OUT_d.ap().rearrange("p (j a m) -> p j a m", j=J, a=400, m=3)
    nc.sync.dma_start(outdv[:, :, 0:87, :], bb[:, :, 0:87, :])
    nc.sync.dma_start(outdv[:, :, 87:NB, :], bb[:, :, 87:NB, :])
    t0v = T0[:].rearrange("p (m r j) -> p m r j", m=3, r=N_RES, j=J)
    nc.scalar.copy(t0v, bb[:, :, 1:NB:3, :].transpose([0, 3, 2, 1]))
    x_bbaos.close()
    x_wt.close()

    # ---------------- P8: sidechain seeds ----------------
    x_seed = ExitStack()
    p_sd = x_seed.enter_context(tc.tile_pool(name="seedp", bufs=1))

    def rsv(t):
        return t[:].rearrange("p (m r j) -> p m r j", m=3, r=N_RES, j=J)

    AEX = [p_sd.tile([P, 3 * 49 * J], F16, tag=f"AEX{m}", name=f"AEX{m}")
           for m in range(3)]

    def aexv(t):
        return t[:].rearrange("p (m r j) -> p m r j", m=3, r=49, j=J)

    for m in range(3):
        dst = aexv(AEX[m])
        for cq in range(12):
            nc.scalar.copy(
                dst[:, :, 4 * cq:4 * cq + 4, :],
                Av[:, cq, m].unsqueeze(2).broadcast_to([P, 3, 4, J]))
        nc.scalar.copy(dst[:, :, 48, :], Av[:, 12, m])

    w1g = wv(WC1)[:, :, 1:146:3, :]
    w2g = wv(WC2)[:, :, 1:146:3, :]
    C2AW = p_sd.tile([P, 3 * 49 * J], F16, tag="C2AW")
    C3AW = p_sd.tile([P, 3 * 49 * J], F16, tag="C3AW")
    c2awv, c3awv = aexv(C2AW), aexv(C3AW)
    kj = lambda t: t[:].rearrange("p (k j) -> p k j", k=KP, j=J)
    cpb = _bcast_m(kj(CP)[:, 2:147:3, :])
    spb = _bcast_m(kj(SP)[:, 2:147:3, :])
    rs1a, rs2a, rs3a = rsv(RS_a[0]), rsv(RS_a[1]), rsv(RS_a[2])
    RSD = 39  # gather-index split: [0:RSD] DVE, [RSD:49] GPSIMD

    for EE, lo, hi, sfx in ((V3, 0, RSD, "d"), (GP, RSD, 49, "g")):
        n = hi - lo

        def sdt(tag):
            return p_sd.tile([P, 3 * n * J], F16, tag=tag + sfx,
                             name=tag + sfx)[:].rearrange(
                "p (m r j) -> p m r j", m=3, r=n, j=J)

        def rs(ap):
            return ap[:, :, lo:hi, :]

        def rsm(ap, m):
            return ap[:, m, lo:hi, :].unsqueeze(1).broadcast_to(
                [P, 3, n, J])

        sAv, sBv = sdt("sA"), sdt("sB")

        def apply_R(wg, outv):
            EE.tensor_mul(sAv, rs(aexv(AEX[0])), rsm(wg, 0))
            EE.tensor_mul(sBv, rs(aexv(AEX[1])), rsm(wg, 1))
            EE.tensor_add(sAv, sAv, sBv)
            EE.tensor_mul(sBv, rs(aexv(AEX[2])), rsm(wg, 2))
            EE.tensor_add(outv, sAv, sBv)

        apply_R(w1g, rs(rs1a[:, :, 1:, :]))
        apply_R(w2g, rs(c2awv))
        for mo in range(3):
            m1, m2 = (mo + 1) % 3, (mo + 2) % 3
            EE.tensor_mul(sAv[:, 0], rs1a[:, m1, 1 + lo:1 + hi, :],
                          c2awv[:, m2, lo:hi, :])
            EE.tensor_mul(sBv[:, 0], rs1a[:, m2, 1 + lo:1 + hi, :],
                          c2awv[:, m1, lo:hi, :])
            EE.tensor_sub(c3awv[:, mo, lo:hi, :], sAv[:, 0], sBv[:, 0])
        EE.tensor_mul(sAv, rs(cpb), rs(c2awv))
        EE.tensor_mul(sBv, rs(spb), rs(c3awv))
        EE.tensor_add(rs(rs2a[:, :, 1:, :]), sAv, sBv)
        EE.tensor_mul(sAv, rs(cpb), rs(c3awv))
        EE.tensor_mul(sBv, rs(spb), rs(c2awv))
        EE.tensor_sub(rs(rs3a[:, :, 1:, :]), sAv, sBv)

    for m in range(3):
        rm = rsv(RS_a[m])
        V3.memset(rm[:, :, 0, :], 0.0)
        V3.memset(rm[:, m, 0, :], 1.0)
    x_seed.close()
    x_w.close()

    # ---------------- P7: sidechain inputs + trig ----------------
    x_side2 = ExitStack()
    p_side2 = x_side2.enter_context(tc.tile_pool(name="side2", bufs=1))
    RS_b = [p_side2.tile([P, 3 * N_RES * J], F16, tag=f"RSb{i}",
                         name=f"RSb{i}") for i in range(3)]
    SIDEAOS = p_side2.tile([P, J * NS * 3], F32, tag="SIDEAOS")

    # ---------------- P9: sidechain serial chain ----------------
    sav = SIDEAOS[:].rearrange("p (j r v m) -> p j r v m",
                               j=J, r=N_RES, v=5, m=3)
    p_st = x_side2.enter_context(tc.tile_pool(name="sidetmp", bufs=3))

    def sl(t, v):
        s = t[:].rearrange("p (v r j) -> p v r j", v=5, r=N_RES, j=J)[:, v]
        return _bcast_m(s)

    cur, nxt = RS_a, RS_b
    RD = 40  # residues 0..RD-1 on DVE, RD..49 on GPSIMD (independent chains)
    for v in range(5):
        cpl, spl = sl(SCP, v), sl(SSP, v)
        cal, sal = sl(SCA, v), sl(SSA, v)
        sdl = sl(SD, v)
        r1p, r2p, r3p = rsv(cur[0]), rsv(cur[1]), rsv(cur[2])
        r1c, r2c, r3c = rsv(nxt[0]), rsv(nxt[1]), rsv(nxt[2])
        out = sav[:, :, :, v, :].transpose([0, 3, 2, 1])
        prev = (t0v if v == 0
                else sav[:, :, :, v - 1, :].transpose([0, 3, 2, 1]))

        for EE, lo, hi, sfx in ((V3, 0, RD, "d"), (GP, RD, N_RES, "g")):
            n = hi - lo

            def stp(tag):
                return p_st.tile([P, 3 * n * J], F16, tag=tag + sfx,
                                 name=tag + sfx)[:].rearrange(
                    "p (m r j) -> p m r j", m=3, r=n, j=J)

            def rs(ap):
                return ap[:, :, lo:hi, :]

            t1, t2, c2x = stp("s1"), stp("s2"), stp("sc2x")
            EE.tensor_mul(t1, rs(cpl), rs(r2p))
            EE.tensor_mul(t2, rs(spl), rs(r3p))
            EE.tensor_add(c2x, t1, t2)
            if v < 4:
                t1, t2 = stp("s1"), stp("s2")
                EE.tensor_mul(t1, rs(cpl), rs(r3p))
                EE.tensor_mul(t2, rs(spl), rs(r2p))
                EE.tensor_sub(rs(r3c), t1, t2)
            t1, t2 = stp("s1"), stp("s2")
            EE.tensor_mul(t1, rs(cal), rs(r1p))
            EE.tensor_mul(t2, rs(sal), c2x)
            EE.tensor_add(rs(r1c), t1, t2)
            if v < 4:
                t1, t2 = stp("s1"), stp("s2")
                EE.tensor_mul(t1, rs(cal), c2x)
                EE.tensor_mul(t2, rs(sal), rs(r1p))
                EE.tensor_sub(rs(r2c), t1, t2)
            t9 = stp("st9")
            EE.tensor_mul(t9, rs(sdl), rs(r1c))
            EE.tensor_add(rs(out), rs(prev), t9)
        cur, nxt = nxt, cur

    # ---------------- P10: sidechain DMA out ----------------
    sflat = SIDEAOS[:].rearrange("p (j x) -> p j x", j=J, x=NS * 3)
    dflat = OUT_d.ap().rearrange("p (j a m) -> p j a m", j=J, a=400, m=3)[
        :, :, NB:400, :].rearrange("p j a m -> p j (a m)")
    nc.sync.dma_start(dflat[:, :, 600:750], sflat[:, :, 600:750])
    nc.sync.dma_start(dflat[:, :, 0:600], sflat[:, :, 0:600])

    x_side2.close()
    es.close()


def _build():
    if "nc" in _cache:
        return _cache["nc"]
    nc = bacc.Bacc("TRN2", target_bir_lowering=False, debug=False,
                   num_devices=NCORES)
    io = {
        "TH": nc.dram_tensor("TH", [P, KP * J], F16, kind="ExternalInput"),
        "PH": nc.dram_tensor("PH", [P, KP * J], F16, kind="ExternalInput"),
        "DD": nc.dram_tensor("DD", [P, KP * J], F16, kind="ExternalInput"),
        "SEED": nc.dram_tensor("SEED", [P, 48], F32, kind="ExternalInput"),
        "SD": nc.dram_tensor("SD", [P, 5 * N_RES * J], F16,
                             kind="ExternalInput"),
        "SA": nc.dram_tensor("SA", [P, 5 * N_RES * J], F16,
                             kind="ExternalInput"),
        "SPH": nc.dram_tensor("SPH", [P, 5 * N_RES * J], F16,
                              kind="ExternalInput"),
        "OUT": nc.dram_tensor("OUT", [P, J * 400 * 3], F32,
                              kind="ExternalOutput"),
    }
    with tile.TileContext(nc) as tc:
        _emit(tc, io)
    nc.compile()
    _cache["nc"] = nc
    return nc


def _marshal_core(cd, ca, ct, sd, sa, st):
    """Per-core (S samples) numpy repack into [128, free] layouts."""
    n = cd.shape[0]

    def tp(arr, ncol):  # (n, ncol) -> [128, ncol*16] k-major
        return np.ascontiguousarray(
            arr.reshape(P, J, ncol).transpose(0, 2, 1).reshape(P, ncol * J))

    th = np.full((n, KP), PI, np.float32)
    th[:, :K] = ca[:, 1:148]
    ph = np.zeros((n, KP), np.float32)
    ph[:, :K] = ct[:, :K]
    dd = np.zeros((n, KP), np.float32)
    dd[:, :K] = cd[:, 2:149]
    seed = np.stack([cd[:, 0], cd[:, 1], ca[:, 0]], axis=1)

    def tps(arr):  # (n, 50, 5) -> [128, 4000] (v, r, j)
        return np.ascontiguousarray(
            arr.reshape(P, J, N_RES, 5).transpose(0, 3, 2, 1).reshape(P, 4000))

    sph = np.empty((n, N_RES, 5), np.float32)
    sph[:, :, 0] = CHI0
    sph[:, :, 1:] = st.reshape(n, N_RES, V)
    return {
        "TH": tp(th, KP).astype(np.float16),
        "PH": tp(ph, KP).astype(np.float16),
        "DD": tp(dd, KP).astype(np.float16),
        "SEED": tp(seed, 3),
        "SD": tps(sd.reshape(n, N_RES, 5)).astype(np.float16),
        "SA": tps(sa.reshape(n, N_RES, 5)).astype(np.float16),
        "SPH": tps(sph).astype(np.float16),
    }


def kernel(central_distances, central_angles, central_dihedrals,
           side_distances, side_angles, side_dihedrals, **kw):
    nc = _build()
    in_maps = []
    for i in range(NCORES):
        sl_ = slice(i * S, (i + 1) * S)
        in_maps.append(_marshal_core(
            central_distances[sl_], central_angles[sl_],
            central_dihedrals[sl_], side_distances[sl_],
            side_angles[sl_], side_dihedrals[sl_]))
    res = run_bass_kernel_spmd(nc, in_maps, core_ids=list(range(NCORES)),
                               trace=TRACE, stitch_traces=False)
    _cache["last_results"] = res
    outs = []
    for i in range(NCORES):
        o = res.results[i]["OUT"].reshape(P, J, 400, 3).reshape(S, 400, 3)
        outs.append(o)
    return np.ascontiguousarray(np.concatenate(outs, axis=0))


# revision 41
# speedup vs baseline: 1.1776x; 1.0049x over previous
"""Trainium2 Bass kernel for nn_BackMapLayerWithSidechains.

Algorithm: the sequential NeRF atom-placement chain is reformulated as a
product of affine transforms (R, t): each step composes M = Rx(phi) @
Rz(pi - theta) and t += d * R[:, 0].  Composition is associative, so the
147-step backbone chain is evaluated with a blocked scan:
  - within-chunk prefixes (C=13 chunks x L=12 steps) computed serially but
    vectorized over (chunk, sample),
  - chunk prefixes composed serially (13 small composes),
  - one wide "apply" produces every global atom position.
Sidechain frames branch off backbone frames exactly:
  F_side(r) = F_global(3r-1) . Rx(phi_{3r-1})   (r >= 1; r=0 is identity at
  atom 1), then 5 more affine steps per residue, vectorized over
  (residue, sample).  Rotation state, trig tables, and inputs run in fp16
  (bounded values, 10-bit mantissa); global positions accumulate in fp32.
Serial phases split independent work (disjoint residues / sample lanes)
across DVE and GPSIMD; trig uses ACT Sin with the free affine; t-chain
updates ride GPSIMD (one-way dependency off the DVE critical path).

Data parallel over 8 NeuronCores: 2048 samples/core.  Host-side numpy only
reshapes/pads inputs into the on-chip layouts (no compute).
"""

import math
from contextlib import ExitStack

import numpy as np

import concourse.bacc as bacc
import concourse.bass as bass
import concourse.mybir as mybir
import concourse.tile as tile
from concourse.bass_utils import run_bass_kernel_spmd

F32 = mybir.dt.float32
BF16 = mybir.dt.bfloat16
F16 = mybir.dt.float16
AF = mybir.ActivationFunctionType

P = 128          # partitions
J = 16           # samples per partition (per core: 2048 = 128*16)
S = P * J        # samples per core
NCORES = 8
B = S * NCORES   # 16384

N_RES = 50
V = 4
NB = 150
NS = 250
K = 147          # real backbone steps
C = 13           # chunks
L = 12           # steps/chunk
KP = C * L       # 156 padded steps
NA = 159         # padded backbone atom count (3 + KP)
CHI0 = 2.0943951023931953
PI = math.pi

_cache = {}
TRACE = False


def _bcast_m(ap3):
    """[p, X, 16] -> [p, 3, X, 16] via 0-stride comp dim."""
    return ap3.unsqueeze(1).broadcast_to([P, 3] + list(ap3.shape[1:]))


def _emit(tc, io):
    nc = tc.nc
    V3 = nc.vector
    GP = nc.gpsimd

    TH_d, PH_d, DD_d, SEED_d, SD_d, SA_d, SPH_d, OUT_d = (
        io["TH"], io["PH"], io["DD"], io["SEED"], io["SD"], io["SA"],
        io["SPH"], io["OUT"])

    # Pools: strict LIFO release order.  Open order (outer->inner):
    # perm, side1, w, wt, trig1, dd, c3, thph
    es = ExitStack()
    p_perm = es.enter_context(tc.tile_pool(name="perm", bufs=1))
    p_side1 = es.enter_context(tc.tile_pool(name="side1", bufs=1))
    x_w = ExitStack()
    p_w = x_w.enter_context(tc.tile_pool(name="w", bufs=1))
    x_wt = ExitStack()
    p_wt = x_wt.enter_context(tc.tile_pool(name="wt", bufs=1))
    x_trig1 = ExitStack()
    p_trig1 = x_trig1.enter_context(tc.tile_pool(name="trig1", bufs=1))
    x_dd = ExitStack()
    p_dd = x_dd.enter_context(tc.tile_pool(name="dd", bufs=1))
    x_c3 = ExitStack()
    p_c3 = x_c3.enter_context(tc.tile_pool(name="c3", bufs=1))
    p_tmp = x_c3.enter_context(tc.tile_pool(name="wtmp", bufs=4))
    x_thph = ExitStack()
    p_thph = x_thph.enter_context(tc.tile_pool(name="thph", bufs=1))

    consts = {}

    def cst(val):
        if val not in consts:
            t = p_perm.tile([P, 1], F32, tag=f"cst{len(consts)}",
                            name=f"cst{len(consts)}")
            V3.memset(t[:], val)
            consts[val] = t[:]
        return consts[val]

    SEED = p_perm.tile([P, 48], F32, tag="SEED")
    CP = p_perm.tile([P, KP * J], F16, tag="CP")
    SP = p_perm.tile([P, KP * J], F16, tag="SP")
    A_all = p_perm.tile([P, C * 192], F32, tag="A_all")

    RS_a = [p_side1.tile([P, 3 * N_RES * J], F16, tag=f"RSa{i}",
                         name=f"RSa{i}") for i in range(3)]
    T0 = p_side1.tile([P, 3 * N_RES * J], F32, tag="T0")
    SD = p_side1.tile([P, 5 * N_RES * J], F16, tag="SDb")
    SCA = p_side1.tile([P, 5 * N_RES * J], F16, tag="SCA")
    SSA = p_side1.tile([P, 5 * N_RES * J], F16, tag="SSA")
    SCP = p_side1.tile([P, 5 * N_RES * J], F16, tag="SCP")
    SSP = p_side1.tile([P, 5 * N_RES * J], F16, tag="SSP")

    WC1 = p_w.tile([P, 3 * KP * J], F16, tag="WC1")
    WC2 = p_w.tile([P, 3 * KP * J], F16, tag="WC2")
    WT = p_wt.tile([P, 3 * KP * J], F16, tag="WT")
    CA_ = p_trig1.tile([P, KP * J], F16, tag="CA_")
    SA = p_trig1.tile([P, KP * J], F16, tag="SA")
    DD = p_dd.tile([P, KP * J], F16, tag="DD")
    SAs = p_dd.tile([P, 5 * N_RES * J], F16, tag="SAs")
    SPHs = p_dd.tile([P, 5 * N_RES * J], F16, tag="SPHs")
    c3t = [p_c3.tile([P, 3 * C * J], F16, tag=f"c3{i}", name=f"c3{i}")
           for i in range(2)]
    TH = p_thph.tile([P, KP * J], F16, tag="TH")
    PH = p_thph.tile([P, KP * J], F16, tag="PH")

    # ---------------- P0: input DMAs (backbone) ----------------
    nc.sync.dma_start(TH[:], TH_d.ap())
    nc.sync.dma_start(PH[:], PH_d.ap())
    nc.sync.dma_start(DD[:], DD_d.ap())
    nc.sync.dma_start(SEED[:], SEED_d.ap())
    nc.sync.dma_start(SD[:], SD_d.ap())
    nc.sync.dma_start(SAs[:], SA_d.ap())
    nc.sync.dma_start(SPHs[:], SPH_d.ap())

    # ---------------- P1: backbone trig (ACT), fp16 outputs ----------
    nc.scalar.activation(CA_[:], TH[:], AF.Sin, bias=cst(-PI / 2))
    nc.scalar.activation(SA[:], TH[:], AF.Sin)
    nc.scalar.activation(SP[:], PH[:], AF.Sin)
    nc.scalar.activation(PH[:], PH[:], AF.Abs)
    nc.scalar.activation(CP[:], PH[:], AF.Sin, scale=-1.0, bias=cst(PI / 2))
    x_thph.close()
    # sidechain trig early: overlaps the whole backbone on ACT
    nc.scalar.activation(SCA[:], SAs[:], AF.Sin, bias=cst(-PI / 2))
    nc.scalar.activation(SSA[:], SAs[:], AF.Sin)
    nc.scalar.activation(SSP[:], SPHs[:], AF.Sin)
    nc.scalar.activation(SPHs[:], SPHs[:], AF.Abs)
    nc.scalar.activation(SCP[:], SPHs[:], AF.Sin, scale=-1.0,
                         bias=cst(PI / 2))

    def wv(t):
        return t[:].rearrange("p (m k j) -> p m k j", m=3, k=KP, j=J)

    def wl(t, l):
        return t[:].rearrange("p (m c l j) -> p m c l j",
                              m=3, c=C, l=L, j=J)[:, :, :, l, :]

    def trig_l(t, l, bcast=True):
        s = t[:].rearrange("p (c l j) -> p c l j", c=C, l=L, j=J)[:, :, l, :]
        return _bcast_m(s) if bcast else s

    def c3v(i):
        return c3t[i][:].rearrange("p (m c j) -> p m c j", m=3, c=C, j=J)

    def anv(t):
        return t[:].rearrange("p (c n m j) -> p c n m j", c=C, n=4, m=3, j=J)

    Av = anv(A_all)

    # ---------------- P2a: within-chunk init (l = 0) ----------------
    w1_0, w2_0 = wl(WC1, 0), wl(WC2, 0)
    ca0, sa0 = trig_l(CA_, 0, False), trig_l(SA, 0, False)
    cp0, sp0 = trig_l(CP, 0, False), trig_l(SP, 0, False)
    nc.scalar.copy(w1_0[:, 0], ca0)
    V3.tensor_mul(w1_0[:, 1], sa0, cp0)
    V3.tensor_mul(w1_0[:, 2], sa0, sp0)
    V3.tensor_scalar_mul(w2_0[:, 0], sa0, -1.0)
    V3.tensor_mul(w2_0[:, 1], ca0, cp0)
    V3.tensor_mul(w2_0[:, 2], ca0, sp0)
    c3i = c3v(0)
    V3.memset(c3i[:, 0], 0.0)
    V3.tensor_scalar_mul(c3i[:, 1], sp0, -1.0)
    nc.scalar.copy(c3i[:, 2], cp0)
    GP.tensor_mul(wl(WT, 0), trig_l(DD, 0), wl(WC1, 0))

    # ---------------- P2b: within-chunk serial scan ----------------
    for l in range(1, L):
        cpl, spl = trig_l(CP, l), trig_l(SP, l)
        cal, sal = trig_l(CA_, l), trig_l(SA, l)
        p1, p2 = wl(WC1, l - 1), wl(WC2, l - 1)
        p3 = c3v((l - 1) % 2)
        c3c = c3v(l % 2)
        sh = [P, 3 * C * J]

        def tt(tag):
            return p_tmp.tile(sh, F16, tag=tag, name=tag)[:].rearrange(
                "p (m c j) -> p m c j", m=3, c=C, j=J)

        def ttf(tag):
            return p_tmp.tile(sh, F32, tag=tag, name=tag)[:].rearrange(
                "p (m c j) -> p m c j", m=3, c=C, j=J)

        t1, t2, c2x = tt("t1"), tt("t2"), tt("c2x")
        V3.tensor_mul(t1, cpl, p2)
        V3.tensor_mul(t2, spl, p3)
        V3.tensor_add(c2x, t1, t2)
        t1, t2 = tt("t1"), tt("t2")
        V3.tensor_mul(t1, cpl, p3)
        V3.tensor_mul(t2, spl, p2)
        V3.tensor_sub(c3c, t1, t2)
        t1, t2 = tt("t1"), tt("t2")
        V3.tensor_mul(t1, cal, p1)
        V3.tensor_mul(t2, sal, c2x)
        V3.tensor_add(wl(WC1, l), t1, t2)
        t1, t2 = tt("t1"), tt("t2")
        V3.tensor_mul(t1, cal, c2x)
        V3.tensor_mul(t2, sal, p1)
        V3.tensor_sub(wl(WC2, l), t1, t2)
        t9 = tt("t9")
        GP.tensor_mul(t9, trig_l(DD, l), wl(WC1, l))
        GP.tensor_add(wl(WT, l), wl(WT, l - 1), t9)

    # ---------------- P3a: chunk totals T ----------------
    # T lives in its own pool opened after closing c3/dd/trig1? T build
    # reads c3 -> keep c3 open until the copies below are emitted.
    x_T = ExitStack()
    p_T = x_T.enter_context(tc.tile_pool(name="Tp", bufs=1))
    p_u = x_T.enter_context(tc.tile_pool(name="up", bufs=4))
    T_all = p_T.tile([P, C * 192], F32, tag="T_all")
    Tv = anv(T_all)
    w1f, w2f = wl(WC1, L - 1), wl(WC2, L - 1)
    c3f, wtf = c3v((L - 1) % 2), wl(WT, L - 1)
    for c_ in range(C):
        nc.scalar.copy(Tv[:, c_, 0], w1f[:, :, c_, :])
        nc.scalar.copy(Tv[:, c_, 1], w2f[:, :, c_, :])
        nc.scalar.copy(Tv[:, c_, 2], c3f[:, :, c_, :])
        nc.scalar.copy(Tv[:, c_, 3], wtf[:, :, c_, :])

    # ---------------- P3b: seed A_0 = F0 ----------------
    sdv = SEED[:].rearrange("p (q j) -> p q j", q=3, j=J)
    cd0, cd1, ca0s = sdv[:, 0], sdv[:, 1], sdv[:, 2]
    sca0 = p_T.tile([P, J], F32, tag="sca0", name="sca0")[:]
    cca0 = p_T.tile([P, J], F32, tag="cca0", name="cca0")[:]
    nc.scalar.activation(sca0, ca0s, AF.Sin)
    nc.scalar.activation(cca0, ca0s, AF.Sin, bias=cst(-PI / 2))
    V3.memset(Av[:, 0], 0.0)
    nc.scalar.copy(Av[:, 0, 0, 0], cca0)
    nc.scalar.copy(Av[:, 0, 0, 1], sca0)
    V3.tensor_scalar_mul(Av[:, 0, 1, 0], sca0, -1.0)
    nc.scalar.copy(Av[:, 0, 1, 1], cca0)
    V3.memset(Av[:, 0, 2, 2], 1.0)
    tm0 = p_T.tile([P, J], F32, tag="tm0", name="tm0")[:]
    V3.tensor_mul(tm0, cd1, cca0)
    V3.tensor_add(Av[:, 0, 3, 0], cd0, tm0)
    V3.tensor_mul(Av[:, 0, 3, 1], cd1, sca0)

    # ---------------- P3c: chunk-prefix chain ----------------
    JD = 12  # sample-lane split: j 0..11 DVE, 12..15 GPSIMD (independent)
    for c in range(1, C):
        for EE, jl, jh, sfx in ((V3, 0, JD, "d"), (GP, JD, J, "g")):
            nj = jh - jl
            acm = [Av[:, c - 1, m, :, jl:jh].unsqueeze(1).broadcast_to(
                [P, 4, 3, nj]) for m in range(3)]
            trm = [Tv[:, c - 1, :, m, jl:jh].unsqueeze(2).broadcast_to(
                [P, 4, 3, nj]) for m in range(3)]

            def uu(tag):
                return p_u.tile([P, 12 * nj], F32, tag=tag + sfx,
                                name=tag + sfx)[:].rearrange(
                    "p (n m j) -> p n m j", n=4, m=3, j=nj)

            u1, u2, u3 = uu("u1"), uu("u2"), uu("u3")
            EE.tensor_mul(u1, acm[0], trm[0])
            EE.tensor_mul(u2, acm[1], trm[1])
            EE.tensor_add(u3, u1, u2)
            u1 = uu("u1")
            EE.tensor_mul(u1, acm[2], trm[2])
            EE.tensor_add(Av[:, c, :, :, jl:jh], u3, u1)
            EE.tensor_add(Av[:, c, 3, :, jl:jh], Av[:, c, 3, :, jl:jh],
                          Av[:, c - 1, 3, :, jl:jh])

    # close inner pools (LIFO): Tp/up, then c3, dd, trig1
    x_T.close()
    x_c3.close()
    x_dd.close()
    x_trig1.close()

    # ---------------- P4: apply -> backbone AoS ----------------
    x_bbaos = ExitStack()
    p_bb = x_bbaos.enter_context(tc.tile_pool(name="bbaos", bufs=1))
    BBAOS = p_bb.tile([P, J * NA * 3], F32, tag="BBAOS")

    def bbv():
        return BBAOS[:].rearrange("p (j a m) -> p j a m", j=J, a=NA, m=3)

    x_apply = ExitStack()
    p_ap = x_apply.enter_context(tc.tile_pool(name="applyp", bufs=3))
    wtv = WT[:].rearrange("p (m c l j) -> p m c l j", m=3, c=C, l=L, j=J)
    for c in range(C):
        # [p, comp, L, J] per chunk (3 free dims: walrus TENSOR3D limit)
        aw = [Av[:, c, m].unsqueeze(2).broadcast_to([P, 3, L, J])
              for m in range(3)]
        at = Av[:, c, 3].unsqueeze(2).broadcast_to([P, 3, L, J])
        wtm = [wtv[:, m, c].unsqueeze(1).broadcast_to([P, 3, L, J])
               for m in range(3)]

        EE = GP if c >= 10 else V3

        def qq(tag):
            return p_ap.tile([P, 3 * L * J], F16, tag=tag,
                             name=f"{tag}_{c}")[:].rearrange(
                "p (m l j) -> p m l j", m=3, l=L, j=J)

        q1, q2 = qq("q1"), qq("q2")
        EE.tensor_mul(q1, aw[0], wtm[0])
        EE.tensor_mul(q2, aw[1], wtm[1])
        EE.tensor_add(q1, q1, q2)
        q2 = qq("q2")
        EE.tensor_mul(q2, aw[2], wtm[2])
        EE.tensor_add(q1, q1, q2)
        outv = bbv()[:, :, 3 + c * L:3 + (c + 1) * L, :].transpose(
            [0, 3, 2, 1])
        EE.tensor_add(outv, q1, at)
    x_apply.close()

    # ---------------- P5: atoms 0,1,2 ----------------
    bb = bbv()
    V3.memset(bb[:, :, 0:2, :], 0.0)
    nc.scalar.copy(bb[:, :, 1, 0], cd0)
    V3.memset(bb[:, :, 2, 2], 0.0)
    nc.scalar.copy(bb[:, :, 2, 0], Av[:, 0, 3, 0])
    nc.scalar.copy(bb[:, :, 2, 1], Av[:, 0, 3, 1])

    # ---------------- P6: backbone DMA out + T0 gather ----------------
    outdv = OUT_d.ap().rearrange("p (j a m) -> p j a m", j=J, a=400, m=3)
    nc.sync.dma_start(outdv[:, :, 0:87, :], bb[:, :, 0:87, :])
    nc.sync.dma_start(outdv[:, :, 87:NB, :], bb[:, :, 87:NB, :])
    t0v = T0[:].rearrange("p (m r j) -> p m r j", m=3, r=N_RES, j=J)
    nc.scalar.copy(t0v, bb[:, :, 1:NB:3, :].transpose([0, 3, 2, 1]))
    x_bbaos.close()
    x_wt.close()

    # ---------------- P8: sidechain seeds ----------------
    x_seed = ExitStack()
    p_sd = x_seed.enter_context(tc.tile_pool(name="seedp", bufs=1))

    def rsv(t):
        return t[:].rearrange("p (m r j) -> p m r j", m=3, r=N_RES, j=J)

    AEX = [p_sd.tile([P, 3 * 49 * J], F16, tag=f"AEX{m}", name=f"AEX{m}")
           for m in range(3)]

    def aexv(t):
        return t[:].rearrange("p (m r j) -> p m r j", m=3, r=49, j=J)

    for m in range(3):
        dst = aexv(AEX[m])
        for cq in range(12):
            nc.scalar.copy(
                dst[:, :, 4 * cq:4 * cq + 4, :],
                Av[:, cq, m].unsqueeze(2).broadcast_to([P, 3, 4, J]))
        nc.scalar.copy(dst[:, :, 48, :], Av[:, 12, m])

    w1g = wv(WC1)[:, :, 1:146:3, :]
    w2g = wv(WC2)[:, :, 1:146:3, :]
    C2AW = p_sd.tile([P, 3 * 49 * J], F16, tag="C2AW")
    C3AW = p_sd.tile([P, 3 * 49 * J], F16, tag="C3AW")
    c2awv, c3awv = aexv(C2AW), aexv(C3AW)
    kj = lambda t: t[:].rearrange("p (k j) -> p k j", k=KP, j=J)
    cpb = _bcast_m(kj(CP)[:, 2:147:3, :])
    spb = _bcast_m(kj(SP)[:, 2:147:3, :])
    rs1a, rs2a, rs3a = rsv(RS_a[0]), rsv(RS_a[1]), rsv(RS_a[2])
    RSD = 39  # gather-index split: [0:RSD] DVE, [RSD:49] GPSIMD

    for EE, lo, hi, sfx in ((V3, 0, RSD, "d"), (GP, RSD, 49, "g")):
        n = hi - lo

        def sdt(tag):
            return p_sd.tile([P, 3 * n * J], F16, tag=tag + sfx,
                             name=tag + sfx)[:].rearrange(
                "p (m r j) -> p m r j", m=3, r=n, j=J)

        def rs(ap):
            return ap[:, :, lo:hi, :]

        def rsm(ap, m):
            return ap[:, m, lo:hi, :].unsqueeze(1).broadcast_to(
                [P, 3, n, J])

        sAv, sBv = sdt("sA"), sdt("sB")

        def apply_R(wg, outv):
            EE.tensor_mul(sAv, rs(aexv(AEX[0])), rsm(wg, 0))
            EE.tensor_mul(sBv, rs(aexv(AEX[1])), rsm(wg, 1))
            EE.tensor_add(sAv, sAv, sBv)
            EE.tensor_mul(sBv, rs(aexv(AEX[2])), rsm(wg, 2))
            EE.tensor_add(outv, sAv, sBv)

        apply_R(w1g, rs(rs1a[:, :, 1:, :]))
        apply_R(w2g, rs(c2awv))
        for mo in range(3):
            m1, m2 = (mo + 1) % 3, (mo + 2) % 3
            EE.tensor_mul(sAv[:, 0], rs1a[:, m1, 1 + lo:1 + hi, :],
                          c2awv[:, m2, lo:hi, :])
            EE.tensor_mul(sBv[:, 0], rs1a[:, m2, 1 + lo:1 + hi, :],
                          c2awv[:, m1, lo:hi, :])
            EE.tensor_sub(c3awv[:, mo, lo:hi, :], sAv[:, 0], sBv[:, 0])
        EE.tensor_mul(sAv, rs(cpb), rs(c2awv))
        EE.tensor_mul(sBv, rs(spb), rs(c3awv))
        EE.tensor_add(rs(rs2a[:, :, 1:, :]), sAv, sBv)
        EE.tensor_mul(sAv, rs(cpb), rs(c3awv))
        EE.tensor_mul(sBv, rs(spb), rs(c2awv))
        EE.tensor_sub(rs(rs3a[:, :, 1:, :]), sAv, sBv)

    for m in range(3):
        rm = rsv(RS_a[m])
        V3.memset(rm[:, :, 0, :], 0.0)
        V3.memset(rm[:, m, 0, :], 1.0)
    x_seed.close()
    x_w.close()

    # ---------------- P7: sidechain inputs + trig ----------------
    x_side2 = ExitStack()
    p_side2 = x_side2.enter_context(tc.tile_pool(name="side2", bufs=1))
    RS_b = [p_side2.tile([P, 3 * N_RES * J], F16, tag=f"RSb{i}",
                         name=f"RSb{i}") for i in range(3)]
    SIDEAOS = p_side2.tile([P, J * NS * 3], F32, tag="SIDEAOS")

    # ---------------- P9: sidechain serial chain ----------------
    sav = SIDEAOS[:].rearrange("p (j r v m) -> p j r v m",
                               j=J, r=N_RES, v=5, m=3)
    p_st = x_side2.enter_context(tc.tile_pool(name="sidetmp", bufs=3))

    def sl(t, v):
        s = t[:].rearrange("p (v r j) -> p v r j", v=5, r=N_RES, j=J)[:, v]
        return _bcast_m(s)

    cur, nxt = RS_a, RS_b
    RD = 40  # residues 0..RD-1 on DVE, RD..49 on GPSIMD (independent chains)
    for v in range(5):
        cpl, spl = sl(SCP, v), sl(SSP, v)
        cal, sal = sl(SCA, v), sl(SSA, v)
        sdl = sl(SD, v)
        r1p, r2p, r3p = rsv(cur[0]), rsv(cur[1]), rsv(cur[2])
        r1c, r2c, r3c = rsv(nxt[0]), rsv(nxt[1]), rsv(nxt[2])
        out = sav[:, :, :, v, :].transpose([0, 3, 2, 1])
        prev = (t0v if v == 0
                else sav[:, :, :, v - 1, :].transpose([0, 3, 2, 1]))

        for EE, lo, hi, sfx in ((V3, 0, RD, "d"), (GP, RD, N_RES, "g")):
            n = hi - lo

            def stp(tag):
                return p_st.tile([P, 3 * n * J], F16, tag=tag + sfx,
                                 name=tag + sfx)[:].rearrange(
                    "p (m r j) -> p m r j", m=3, r=n, j=J)

            def rs(ap):
                return ap[:, :, lo:hi, :]

            t1, t2, c2x = stp("s1"), stp("s2"), stp("sc2x")
            EE.tensor_mul(t1, rs(cpl), rs(r2p))
            EE.tensor_mul(t2, rs(spl), rs(r3p))
            EE.tensor_add(c2x, t1, t2)
            if v < 4:
                t1, t2 = stp("s1"), stp("s2")
                EE.tensor_mul(t1, rs(cpl), rs(r3p))
                EE.tensor_mul(t2, rs(spl), rs(r2p))
                EE.tensor_sub(rs(r3c), t1, t2)
            t1, t2 = stp("s1"), stp("s2")
            EE.tensor_mul(t1, rs(cal), rs(r1p))
            EE.tensor_mul(t2, rs(sal), c2x)
            EE.tensor_add(rs(r1c), t1, t2)
            if v < 4:
                t1, t2 = stp("s1"), stp("s2")
                EE.tensor_mul(t1, rs(cal), c2x)
                EE.tensor_mul(t2, rs(sal), rs(r1p))
                EE.tensor_sub(rs(r2c), t1, t2)
            t9 = stp("st9")
            EE.tensor_mul(t9, rs(sdl), rs(r1c))
            EE.tensor_add(rs(out), rs(prev), t9)
        cur, nxt = nxt, cur

    # ---------------- P10: sidechain DMA out ----------------
    sflat = SIDEAOS[:].rearrange("p (j x) -> p j x", j=J, x=NS * 3)
    dflat = OUT_d.ap().rearrange("p (j a m) -> p j a m", j=J, a=400, m=3)[
        :, :, NB:400, :].rearrange("p j a m -> p j (a m)")
    nc.sync.dma_start(dflat[:, :, 600:750], sflat[:, :, 600:750])
    nc.sync.dma_start(dflat[:, :, 0:600], sflat[:, :, 0:600])

    x_side2.close()
    es.close()


def _build():
    if "nc" in _cache:
        return _cache["nc"]
    nc = bacc.Bacc("TRN2", target_bir_lowering=False, debug=False,
                   num_devices=NCORES)
    io = {
        "TH": nc.dram_tensor("TH", [P, KP * J], F16, kind="ExternalInput"),
        "PH": nc.dram_tensor("PH", [P, KP * J], F16, kind="ExternalInput"),
        "DD": nc.dram_tensor("DD", [P, KP * J], F16, kind="ExternalInput"),
        "SEED": nc.dram_tensor("SEED", [P, 48], F32, kind="ExternalInput"),
        "SD": nc.dram_tensor("SD", [P, 5 * N_RES * J], F16,
                             kind="ExternalInput"),
        "SA": nc.dram_tensor("SA", [P, 5 * N_RES * J], F16,
                             kind="ExternalInput"),
        "SPH": nc.dram_tensor("SPH", [P, 5 * N_RES * J], F16,
                              kind="ExternalInput"),
        "OUT": nc.dram_tensor("OUT", [P, J * 400 * 3], F32,
                              kind="ExternalOutput"),
    }
    with tile.TileContext(nc) as tc:
        _emit(tc, io)
    nc.compile()
    _cache["nc"] = nc
    return nc


def _marshal_core(cd, ca, ct, sd, sa, st):
    """Per-core (S samples) numpy repack into [128, free] layouts."""
    n = cd.shape[0]

    def tp(arr, ncol):  # (n, ncol) -> [128, ncol*16] k-major
        return np.ascontiguousarray(
            arr.reshape(P, J, ncol).transpose(0, 2, 1).reshape(P, ncol * J))

    th = np.full((n, KP), PI, np.float32)
    th[:, :K] = ca[:, 1:148]
    ph = np.zeros((n, KP), np.float32)
    ph[:, :K] = ct[:, :K]
    dd = np.zeros((n, KP), np.float32)
    dd[:, :K] = cd[:, 2:149]
    seed = np.stack([cd[:, 0], cd[:, 1], ca[:, 0]], axis=1)

    def tps(arr):  # (n, 50, 5) -> [128, 4000] (v, r, j)
        return np.ascontiguousarray(
            arr.reshape(P, J, N_RES, 5).transpose(0, 3, 2, 1).reshape(P, 4000))

    sph = np.empty((n, N_RES, 5), np.float32)
    sph[:, :, 0] = CHI0
    sph[:, :, 1:] = st.reshape(n, N_RES, V)
    return {
        "TH": tp(th, KP).astype(np.float16),
        "PH": tp(ph, KP).astype(np.float16),
        "DD": tp(dd, KP).astype(np.float16),
        "SEED": tp(seed, 3),
        "SD": tps(sd.reshape(n, N_RES, 5)).astype(np.float16),
        "SA": tps(sa.reshape(n, N_RES, 5)).astype(np.float16),
        "SPH": tps(sph).astype(np.float16),
    }


def kernel(central_distances, central_angles, central_dihedrals,
           side_distances, side_angles, side_dihedrals, **kw):
    nc = _build()
    in_maps = []
    for i in range(NCORES):
        sl_ = slice(i * S, (i + 1) * S)
        in_maps.append(_marshal_core(
            central_distances[sl_], central_angles[sl_],
            central_dihedrals[sl_], side_distances[sl_],
            side_angles[sl_], side_dihedrals[sl_]))
    res = run_bass_kernel_spmd(nc, in_maps, core_ids=list(range(NCORES)),
                               trace=TRACE, stitch_traces=False)
    _cache["last_results"] = res
    outs = []
    for i in range(NCORES):
        o = res.results[i]["OUT"].reshape(P, J, 400, 3).reshape(S, 400, 3)
        outs.append(o)
    return np.ascontiguousarray(np.concatenate(outs, axis=0))


# revision 42
# speedup vs baseline: 1.2036x; 1.0221x over previous
"""Trainium2 Bass kernel for nn_BackMapLayerWithSidechains.

Algorithm: the sequential NeRF atom-placement chain is reformulated as a
product of affine transforms (R, t): each step composes M = Rx(phi) @
Rz(pi - theta) and t += d * R[:, 0].  Composition is associative, so the
147-step backbone chain is evaluated with a blocked scan:
  - within-chunk prefixes (C=13 chunks x L=12 steps) computed serially but
    vectorized over (chunk, sample),
  - chunk prefixes composed serially (13 small composes),
  - one wide "apply" produces every global atom position.
Sidechain frames branch off backbone frames exactly:
  F_side(r) = F_global(3r-1) . Rx(phi_{3r-1})   (r >= 1; r=0 is identity at
  atom 1), then 5 more affine steps per residue, vectorized over
  (residue, sample).  Rotation state, trig tables, and inputs run in fp16
  (bounded values, 10-bit mantissa); global positions accumulate in fp32.
Serial phases split independent work (disjoint residues / sample lanes)
across DVE and GPSIMD; trig uses ACT Sin with the free affine; t-chain
updates ride GPSIMD (one-way dependency off the DVE critical path).

Data parallel over 8 NeuronCores: 2048 samples/core.  Host-side numpy only
reshapes/pads inputs into the on-chip layouts (no compute).
"""

import math
from contextlib import ExitStack

import numpy as np

import concourse.bacc as bacc
import concourse.bass as bass
import concourse.mybir as mybir
import concourse.tile as tile
from concourse.bass_utils import run_bass_kernel_spmd

F32 = mybir.dt.float32
BF16 = mybir.dt.bfloat16
F16 = mybir.dt.float16
AF = mybir.ActivationFunctionType

P = 128          # partitions
J = 16           # samples per partition (per core: 2048 = 128*16)
S = P * J        # samples per core
NCORES = 8
B = S * NCORES   # 16384

N_RES = 50
V = 4
NB = 150
NS = 250
K = 147          # real backbone steps
C = 13           # chunks
L = 12           # steps/chunk
KP = C * L       # 156 padded steps
NA = 159         # padded backbone atom count (3 + KP)
CHI0 = 2.0943951023931953
PI = math.pi

_cache = {}
TRACE = False


def _bcast_m(ap3):
    """[p, X, 16] -> [p, 3, X, 16] via 0-stride comp dim."""
    return ap3.unsqueeze(1).broadcast_to([P, 3] + list(ap3.shape[1:]))


def _emit(tc, io):
    nc = tc.nc
    V3 = nc.vector
    GP = nc.gpsimd

    TH_d, PH_d, DD_d, SEED_d, SD_d, SA_d, SPH_d, OUT_d = (
        io["TH"], io["PH"], io["DD"], io["SEED"], io["SD"], io["SA"],
        io["SPH"], io["OUT"])

    # Pools: strict LIFO release order.  Open order (outer->inner):
    # perm, side1, w, wt, trig1, dd, c3, thph
    es = ExitStack()
    p_perm = es.enter_context(tc.tile_pool(name="perm", bufs=1))
    p_side1 = es.enter_context(tc.tile_pool(name="side1", bufs=1))
    x_w = ExitStack()
    p_w = x_w.enter_context(tc.tile_pool(name="w", bufs=1))
    x_wt = ExitStack()
    p_wt = x_wt.enter_context(tc.tile_pool(name="wt", bufs=1))
    x_trig1 = ExitStack()
    p_trig1 = x_trig1.enter_context(tc.tile_pool(name="trig1", bufs=1))
    x_dd = ExitStack()
    p_dd = x_dd.enter_context(tc.tile_pool(name="dd", bufs=1))
    x_c3 = ExitStack()
    p_c3 = x_c3.enter_context(tc.tile_pool(name="c3", bufs=1))
    p_tmp = x_c3.enter_context(tc.tile_pool(name="wtmp", bufs=4))
    x_thph = ExitStack()
    p_thph = x_thph.enter_context(tc.tile_pool(name="thph", bufs=1))

    consts = {}

    def cst(val):
        if val not in consts:
            t = p_perm.tile([P, 1], F32, tag=f"cst{len(consts)}",
                            name=f"cst{len(consts)}")
            V3.memset(t[:], val)
            consts[val] = t[:]
        return consts[val]

    SEED = p_perm.tile([P, 48], F32, tag="SEED")
    CP = p_perm.tile([P, KP * J], F16, tag="CP")
    SP = p_perm.tile([P, KP * J], F16, tag="SP")
    A_all = p_perm.tile([P, C * 192], F32, tag="A_all")

    RS_a = [p_side1.tile([P, 3 * N_RES * J], F16, tag=f"RSa{i}",
                         name=f"RSa{i}") for i in range(3)]
    T0 = p_side1.tile([P, 3 * N_RES * J], F32, tag="T0")
    SD = p_side1.tile([P, 5 * N_RES * J], F16, tag="SDb")
    SCA = p_side1.tile([P, 5 * N_RES * J], F16, tag="SCA")
    SSA = p_side1.tile([P, 5 * N_RES * J], F16, tag="SSA")
    SCP = p_side1.tile([P, 5 * N_RES * J], F16, tag="SCP")
    SSP = p_side1.tile([P, 5 * N_RES * J], F16, tag="SSP")

    WC1 = p_w.tile([P, 3 * KP * J], F16, tag="WC1")
    WC2 = p_w.tile([P, 3 * KP * J], F16, tag="WC2")
    WT = p_wt.tile([P, 3 * KP * J], F16, tag="WT")
    CA_ = p_trig1.tile([P, KP * J], F16, tag="CA_")
    SA = p_trig1.tile([P, KP * J], F16, tag="SA")
    DD = p_dd.tile([P, KP * J], F16, tag="DD")
    SAs = p_dd.tile([P, 5 * N_RES * J], F16, tag="SAs")
    SPHs = p_dd.tile([P, 5 * N_RES * J], F16, tag="SPHs")
    c3t = [p_c3.tile([P, 3 * C * J], F16, tag=f"c3{i}", name=f"c3{i}")
           for i in range(2)]
    TH = p_thph.tile([P, KP * J], F16, tag="TH")
    PH = p_thph.tile([P, KP * J], F16, tag="PH")

    # ---------------- P0: input DMAs (backbone) ----------------
    nc.sync.dma_start(TH[:], TH_d.ap())
    nc.sync.dma_start(PH[:], PH_d.ap())
    nc.sync.dma_start(DD[:], DD_d.ap())
    nc.sync.dma_start(SEED[:], SEED_d.ap())
    nc.sync.dma_start(SD[:], SD_d.ap())
    nc.sync.dma_start(SAs[:], SA_d.ap())
    nc.sync.dma_start(SPHs[:], SPH_d.ap())

    # ---------------- P1: backbone trig (ACT), fp16 outputs ----------
    nc.scalar.activation(CA_[:], TH[:], AF.Sin, bias=cst(-PI / 2))
    nc.scalar.activation(SA[:], TH[:], AF.Sin)
    nc.scalar.activation(SP[:], PH[:], AF.Sin)
    nc.scalar.activation(PH[:], PH[:], AF.Abs)
    nc.scalar.activation(CP[:], PH[:], AF.Sin, scale=-1.0, bias=cst(PI / 2))
    x_thph.close()
    # sidechain trig early: overlaps the whole backbone on ACT
    nc.scalar.activation(SCA[:], SAs[:], AF.Sin, bias=cst(-PI / 2))
    nc.scalar.activation(SSA[:], SAs[:], AF.Sin)
    nc.scalar.activation(SSP[:], SPHs[:], AF.Sin)
    nc.scalar.activation(SPHs[:], SPHs[:], AF.Abs)
    nc.scalar.activation(SCP[:], SPHs[:], AF.Sin, scale=-1.0,
                         bias=cst(PI / 2))

    def wv(t):
        return t[:].rearrange("p (m k j) -> p m k j", m=3, k=KP, j=J)

    def wl(t, l):
        return t[:].rearrange("p (m c l j) -> p m c l j",
                              m=3, c=C, l=L, j=J)[:, :, :, l, :]

    def trig_l(t, l, bcast=True):
        s = t[:].rearrange("p (c l j) -> p c l j", c=C, l=L, j=J)[:, :, l, :]
        return _bcast_m(s) if bcast else s

    def c3v(i):
        return c3t[i][:].rearrange("p (m c j) -> p m c j", m=3, c=C, j=J)

    def anv(t):
        return t[:].rearrange("p (c n m j) -> p c n m j", c=C, n=4, m=3, j=J)

    Av = anv(A_all)

    # ---------------- P2a: within-chunk init (l = 0) ----------------
    w1_0, w2_0 = wl(WC1, 0), wl(WC2, 0)
    ca0, sa0 = trig_l(CA_, 0, False), trig_l(SA, 0, False)
    cp0, sp0 = trig_l(CP, 0, False), trig_l(SP, 0, False)
    nc.scalar.copy(w1_0[:, 0], ca0)
    V3.tensor_mul(w1_0[:, 1], sa0, cp0)
    V3.tensor_mul(w1_0[:, 2], sa0, sp0)
    V3.tensor_scalar_mul(w2_0[:, 0], sa0, -1.0)
    V3.tensor_mul(w2_0[:, 1], ca0, cp0)
    V3.tensor_mul(w2_0[:, 2], ca0, sp0)
    c3i = c3v(0)
    V3.memset(c3i[:, 0], 0.0)
    V3.tensor_scalar_mul(c3i[:, 1], sp0, -1.0)
    nc.scalar.copy(c3i[:, 2], cp0)
    GP.tensor_mul(wl(WT, 0), trig_l(DD, 0), wl(WC1, 0))

    # ---------------- P2b: within-chunk serial scan ----------------
    for l in range(1, L):
        cpl, spl = trig_l(CP, l), trig_l(SP, l)
        cal, sal = trig_l(CA_, l), trig_l(SA, l)
        p1, p2 = wl(WC1, l - 1), wl(WC2, l - 1)
        p3 = c3v((l - 1) % 2)
        c3c = c3v(l % 2)
        sh = [P, 3 * C * J]

        def tt(tag):
            return p_tmp.tile(sh, F16, tag=tag, name=tag)[:].rearrange(
                "p (m c j) -> p m c j", m=3, c=C, j=J)

        def ttf(tag):
            return p_tmp.tile(sh, F32, tag=tag, name=tag)[:].rearrange(
                "p (m c j) -> p m c j", m=3, c=C, j=J)

        t1, t2, c2x = tt("t1"), tt("t2"), tt("c2x")
        V3.tensor_mul(t1, cpl, p2)
        V3.tensor_mul(t2, spl, p3)
        V3.tensor_add(c2x, t1, t2)
        t1, t2 = tt("t1"), tt("t2")
        V3.tensor_mul(t1, cpl, p3)
        V3.tensor_mul(t2, spl, p2)
        V3.tensor_sub(c3c, t1, t2)
        t1, t2 = tt("t1"), tt("t2")
        V3.tensor_mul(t1, cal, p1)
        V3.tensor_mul(t2, sal, c2x)
        V3.tensor_add(wl(WC1, l), t1, t2)
        t1, t2 = tt("t1"), tt("t2")
        V3.tensor_mul(t1, cal, c2x)
        V3.tensor_mul(t2, sal, p1)
        V3.tensor_sub(wl(WC2, l), t1, t2)
        t9 = tt("t9")
        GP.tensor_mul(t9, trig_l(DD, l), wl(WC1, l))
        GP.tensor_add(wl(WT, l), wl(WT, l - 1), t9)

    # ---------------- P3a: chunk totals T ----------------
    # T lives in its own pool opened after closing c3/dd/trig1? T build
    # reads c3 -> keep c3 open until the copies below are emitted.
    x_T = ExitStack()
    p_T = x_T.enter_context(tc.tile_pool(name="Tp", bufs=1))
    p_u = x_T.enter_context(tc.tile_pool(name="up", bufs=4))
    T_all = p_T.tile([P, C * 192], F32, tag="T_all")
    Tv = anv(T_all)
    w1f, w2f = wl(WC1, L - 1), wl(WC2, L - 1)
    c3f, wtf = c3v((L - 1) % 2), wl(WT, L - 1)
    for c_ in range(C):
        nc.scalar.copy(Tv[:, c_, 0], w1f[:, :, c_, :])
        nc.scalar.copy(Tv[:, c_, 1], w2f[:, :, c_, :])
        nc.scalar.copy(Tv[:, c_, 2], c3f[:, :, c_, :])
        nc.scalar.copy(Tv[:, c_, 3], wtf[:, :, c_, :])

    # ---------------- P3b: seed A_0 = F0 ----------------
    sdv = SEED[:].rearrange("p (q j) -> p q j", q=3, j=J)
    cd0, cd1, ca0s = sdv[:, 0], sdv[:, 1], sdv[:, 2]
    sca0 = p_T.tile([P, J], F32, tag="sca0", name="sca0")[:]
    cca0 = p_T.tile([P, J], F32, tag="cca0", name="cca0")[:]
    nc.scalar.activation(sca0, ca0s, AF.Sin)
    nc.scalar.activation(cca0, ca0s, AF.Sin, bias=cst(-PI / 2))
    V3.memset(Av[:, 0], 0.0)
    nc.scalar.copy(Av[:, 0, 0, 0], cca0)
    nc.scalar.copy(Av[:, 0, 0, 1], sca0)
    V3.tensor_scalar_mul(Av[:, 0, 1, 0], sca0, -1.0)
    nc.scalar.copy(Av[:, 0, 1, 1], cca0)
    V3.memset(Av[:, 0, 2, 2], 1.0)
    tm0 = p_T.tile([P, J], F32, tag="tm0", name="tm0")[:]
    V3.tensor_mul(tm0, cd1, cca0)
    V3.tensor_add(Av[:, 0, 3, 0], cd0, tm0)
    V3.tensor_mul(Av[:, 0, 3, 1], cd1, sca0)

    # ---------------- P3c: chunk-prefix chain ----------------
    JD = 12  # sample-lane split: j 0..11 DVE, 12..15 GPSIMD (independent)
    for c in range(1, C):
        for EE, jl, jh, sfx in ((V3, 0, JD, "d"), (GP, JD, J, "g")):
            nj = jh - jl
            acm = [Av[:, c - 1, m, :, jl:jh].unsqueeze(1).broadcast_to(
                [P, 4, 3, nj]) for m in range(3)]
            trm = [Tv[:, c - 1, :, m, jl:jh].unsqueeze(2).broadcast_to(
                [P, 4, 3, nj]) for m in range(3)]

            def uu(tag):
                return p_u.tile([P, 12 * nj], F32, tag=tag + sfx,
                                name=tag + sfx)[:].rearrange(
                    "p (n m j) -> p n m j", n=4, m=3, j=nj)

            u1, u2, u3 = uu("u1"), uu("u2"), uu("u3")
            EE.tensor_mul(u1, acm[0], trm[0])
            EE.tensor_mul(u2, acm[1], trm[1])
            EE.tensor_add(u3, u1, u2)
            u1 = uu("u1")
            EE.tensor_mul(u1, acm[2], trm[2])
            EE.tensor_add(Av[:, c, :, :, jl:jh], u3, u1)
            EE.tensor_add(Av[:, c, 3, :, jl:jh], Av[:, c, 3, :, jl:jh],
                          Av[:, c - 1, 3, :, jl:jh])

    # close inner pools (LIFO): Tp/up, then c3, dd, trig1
    x_T.close()
    x_c3.close()
    x_dd.close()
    x_trig1.close()

    # ---------------- P8: sidechain seeds ----------------
    x_seed = ExitStack()
    p_sd = x_seed.enter_context(tc.tile_pool(name="seedp", bufs=1))

    def rsv(t):
        return t[:].rearrange("p (m r j) -> p m r j", m=3, r=N_RES, j=J)

    AEX = [p_sd.tile([P, 3 * 49 * J], F16, tag=f"AEX{m}", name=f"AEX{m}")
           for m in range(3)]

    def aexv(t):
        return t[:].rearrange("p (m r j) -> p m r j", m=3, r=49, j=J)

    for m in range(3):
        dst = aexv(AEX[m])
        for cq in range(12):
            nc.scalar.copy(
                dst[:, :, 4 * cq:4 * cq + 4, :],
                Av[:, cq, m].unsqueeze(2).broadcast_to([P, 3, 4, J]))
        nc.scalar.copy(dst[:, :, 48, :], Av[:, 12, m])

    w1g = wv(WC1)[:, :, 1:146:3, :]
    w2g = wv(WC2)[:, :, 1:146:3, :]
    C2AW = p_sd.tile([P, 3 * 49 * J], F16, tag="C2AW")
    C3AW = p_sd.tile([P, 3 * 49 * J], F16, tag="C3AW")
    c2awv, c3awv = aexv(C2AW), aexv(C3AW)
    kj = lambda t: t[:].rearrange("p (k j) -> p k j", k=KP, j=J)
    cpb = _bcast_m(kj(CP)[:, 2:147:3, :])
    spb = _bcast_m(kj(SP)[:, 2:147:3, :])
    rs1a, rs2a, rs3a = rsv(RS_a[0]), rsv(RS_a[1]), rsv(RS_a[2])
    RSD = 39  # gather-index split: [0:RSD] DVE, [RSD:49] GPSIMD

    for EE, lo, hi, sfx in ((V3, 0, RSD, "d"), (GP, RSD, 49, "g")):
        n = hi - lo

        def sdt(tag):
            return p_sd.tile([P, 3 * n * J], F16, tag=tag + sfx,
                             name=tag + sfx)[:].rearrange(
                "p (m r j) -> p m r j", m=3, r=n, j=J)

        def rs(ap):
            return ap[:, :, lo:hi, :]

        def rsm(ap, m):
            return ap[:, m, lo:hi, :].unsqueeze(1).broadcast_to(
                [P, 3, n, J])

        sAv, sBv = sdt("sA"), sdt("sB")

        def apply_R(wg, outv):
            EE.tensor_mul(sAv, rs(aexv(AEX[0])), rsm(wg, 0))
            EE.tensor_mul(sBv, rs(aexv(AEX[1])), rsm(wg, 1))
            EE.tensor_add(sAv, sAv, sBv)
            EE.tensor_mul(sBv, rs(aexv(AEX[2])), rsm(wg, 2))
            EE.tensor_add(outv, sAv, sBv)

        apply_R(w1g, rs(rs1a[:, :, 1:, :]))
        apply_R(w2g, rs(c2awv))
        for mo in range(3):
            m1, m2 = (mo + 1) % 3, (mo + 2) % 3
            EE.tensor_mul(sAv[:, 0], rs1a[:, m1, 1 + lo:1 + hi, :],
                          c2awv[:, m2, lo:hi, :])
            EE.tensor_mul(sBv[:, 0], rs1a[:, m2, 1 + lo:1 + hi, :],
                          c2awv[:, m1, lo:hi, :])
            EE.tensor_sub(c3awv[:, mo, lo:hi, :], sAv[:, 0], sBv[:, 0])
        EE.tensor_mul(sAv, rs(cpb), rs(c2awv))
        EE.tensor_mul(sBv, rs(spb), rs(c3awv))
        EE.tensor_add(rs(rs2a[:, :, 1:, :]), sAv, sBv)
        EE.tensor_mul(sAv, rs(cpb), rs(c3awv))
        EE.tensor_mul(sBv, rs(spb), rs(c2awv))
        EE.tensor_sub(rs(rs3a[:, :, 1:, :]), sAv, sBv)

    for m in range(3):
        rm = rsv(RS_a[m])
        V3.memset(rm[:, :, 0, :], 0.0)
        V3.memset(rm[:, m, 0, :], 1.0)
    x_seed.close()

    # ---------------- P4: apply -> backbone AoS ----------------
    x_bbaos = ExitStack()
    p_bb = x_bbaos.enter_context(tc.tile_pool(name="bbaos", bufs=1))
    BBAOS = p_bb.tile([P, J * NA * 3], F32, tag="BBAOS")

    def bbv():
        return BBAOS[:].rearrange("p (j a m) -> p j a m", j=J, a=NA, m=3)

    x_apply = ExitStack()
    p_ap = x_apply.enter_context(tc.tile_pool(name="applyp", bufs=3))
    wtv = WT[:].rearrange("p (m c l j) -> p m c l j", m=3, c=C, l=L, j=J)
    for c in range(C):
        # [p, comp, L, J] per chunk (3 free dims: walrus TENSOR3D limit)
        aw = [Av[:, c, m].unsqueeze(2).broadcast_to([P, 3, L, J])
              for m in range(3)]
        at = Av[:, c, 3].unsqueeze(2).broadcast_to([P, 3, L, J])
        wtm = [wtv[:, m, c].unsqueeze(1).broadcast_to([P, 3, L, J])
               for m in range(3)]

        EE = GP if c >= 10 else V3

        def qq(tag):
            return p_ap.tile([P, 3 * L * J], F16, tag=tag,
                             name=f"{tag}_{c}")[:].rearrange(
                "p (m l j) -> p m l j", m=3, l=L, j=J)

        q1, q2 = qq("q1"), qq("q2")
        EE.tensor_mul(q1, aw[0], wtm[0])
        EE.tensor_mul(q2, aw[1], wtm[1])
        EE.tensor_add(q1, q1, q2)
        q2 = qq("q2")
        EE.tensor_mul(q2, aw[2], wtm[2])
        EE.tensor_add(q1, q1, q2)
        outv = bbv()[:, :, 3 + c * L:3 + (c + 1) * L, :].transpose(
            [0, 3, 2, 1])
        EE.tensor_add(outv, q1, at)
    x_apply.close()

    # ---------------- P5: atoms 0,1,2 ----------------
    bb = bbv()
    V3.memset(bb[:, :, 0:2, :], 0.0)
    nc.scalar.copy(bb[:, :, 1, 0], cd0)
    V3.memset(bb[:, :, 2, 2], 0.0)
    nc.scalar.copy(bb[:, :, 2, 0], Av[:, 0, 3, 0])
    nc.scalar.copy(bb[:, :, 2, 1], Av[:, 0, 3, 1])

    # ---------------- P6: backbone DMA out + T0 gather ----------------
    outdv = OUT_d.ap().rearrange("p (j a m) -> p j a m", j=J, a=400, m=3)
    nc.sync.dma_start(outdv[:, :, 0:87, :], bb[:, :, 0:87, :])
    nc.sync.dma_start(outdv[:, :, 87:NB, :], bb[:, :, 87:NB, :])
    t0v = T0[:].rearrange("p (m r j) -> p m r j", m=3, r=N_RES, j=J)
    nc.scalar.copy(t0v, bb[:, :, 1:NB:3, :].transpose([0, 3, 2, 1]))
    x_bbaos.close()
    x_wt.close()
    x_w.close()


    # ---------------- P7: sidechain inputs + trig ----------------
    x_side2 = ExitStack()
    p_side2 = x_side2.enter_context(tc.tile_pool(name="side2", bufs=1))
    RS_b = [p_side2.tile([P, 3 * N_RES * J], F16, tag=f"RSb{i}",
                         name=f"RSb{i}") for i in range(3)]
    SIDEAOS = p_side2.tile([P, J * NS * 3], F32, tag="SIDEAOS")

    # ---------------- P9: sidechain serial chain ----------------
    sav = SIDEAOS[:].rearrange("p (j r v m) -> p j r v m",
                               j=J, r=N_RES, v=5, m=3)
    p_st = x_side2.enter_context(tc.tile_pool(name="sidetmp", bufs=3))

    def sl(t, v):
        s = t[:].rearrange("p (v r j) -> p v r j", v=5, r=N_RES, j=J)[:, v]
        return _bcast_m(s)

    cur, nxt = RS_a, RS_b
    RD = 40  # residues 0..RD-1 on DVE, RD..49 on GPSIMD (independent chains)
    for v in range(5):
        cpl, spl = sl(SCP, v), sl(SSP, v)
        cal, sal = sl(SCA, v), sl(SSA, v)
        sdl = sl(SD, v)
        r1p, r2p, r3p = rsv(cur[0]), rsv(cur[1]), rsv(cur[2])
        r1c, r2c, r3c = rsv(nxt[0]), rsv(nxt[1]), rsv(nxt[2])
        out = sav[:, :, :, v, :].transpose([0, 3, 2, 1])
        prev = (t0v if v == 0
                else sav[:, :, :, v - 1, :].transpose([0, 3, 2, 1]))

        for EE, lo, hi, sfx in ((V3, 0, RD, "d"), (GP, RD, N_RES, "g")):
            n = hi - lo

            def stp(tag):
                return p_st.tile([P, 3 * n * J], F16, tag=tag + sfx,
                                 name=tag + sfx)[:].rearrange(
                    "p (m r j) -> p m r j", m=3, r=n, j=J)

            def rs(ap):
                return ap[:, :, lo:hi, :]

            t1, t2, c2x = stp("s1"), stp("s2"), stp("sc2x")
            EE.tensor_mul(t1, rs(cpl), rs(r2p))
            EE.tensor_mul(t2, rs(spl), rs(r3p))
            EE.tensor_add(c2x, t1, t2)
            if v < 4:
                t1, t2 = stp("s1"), stp("s2")
                EE.tensor_mul(t1, rs(cpl), rs(r3p))
                EE.tensor_mul(t2, rs(spl), rs(r2p))
                EE.tensor_sub(rs(r3c), t1, t2)
            t1, t2 = stp("s1"), stp("s2")
            EE.tensor_mul(t1, rs(cal), rs(r1p))
            EE.tensor_mul(t2, rs(sal), c2x)
            EE.tensor_add(rs(r1c), t1, t2)
            if v < 4:
                t1, t2 = stp("s1"), stp("s2")
                EE.tensor_mul(t1, rs(cal), c2x)
                EE.tensor_mul(t2, rs(sal), rs(r1p))
                EE.tensor_sub(rs(r2c), t1, t2)
            t9 = stp("st9")
            EE.tensor_mul(t9, rs(sdl), rs(r1c))
            EE.tensor_add(rs(out), rs(prev), t9)
        cur, nxt = nxt, cur

    # ---------------- P10: sidechain DMA out ----------------
    sflat = SIDEAOS[:].rearrange("p (j x) -> p j x", j=J, x=NS * 3)
    dflat = OUT_d.ap().rearrange("p (j a m) -> p j a m", j=J, a=400, m=3)[
        :, :, NB:400, :].rearrange("p j a m -> p j (a m)")
    nc.sync.dma_start(dflat[:, :, 600:750], sflat[:, :, 600:750])
    nc.sync.dma_start(dflat[:, :, 0:600], sflat[:, :, 0:600])

    x_side2.close()
    es.close()


def _build():
    if "nc" in _cache:
        return _cache["nc"]
    nc = bacc.Bacc("TRN2", target_bir_lowering=False, debug=False,
                   num_devices=NCORES)
    io = {
        "TH": nc.dram_tensor("TH", [P, KP * J], F16, kind="ExternalInput"),
        "PH": nc.dram_tensor("PH", [P, KP * J], F16, kind="ExternalInput"),
        "DD": nc.dram_tensor("DD", [P, KP * J], F16, kind="ExternalInput"),
        "SEED": nc.dram_tensor("SEED", [P, 48], F32, kind="ExternalInput"),
        "SD": nc.dram_tensor("SD", [P, 5 * N_RES * J], F16,
                             kind="ExternalInput"),
        "SA": nc.dram_tensor("SA", [P, 5 * N_RES * J], F16,
                             kind="ExternalInput"),
        "SPH": nc.dram_tensor("SPH", [P, 5 * N_RES * J], F16,
                              kind="ExternalInput"),
        "OUT": nc.dram_tensor("OUT", [P, J * 400 * 3], F32,
                              kind="ExternalOutput"),
    }
    with tile.TileContext(nc) as tc:
        _emit(tc, io)
    nc.compile()
    _cache["nc"] = nc
    return nc


def _marshal_core(cd, ca, ct, sd, sa, st):
    """Per-core (S samples) numpy repack into [128, free] layouts."""
    n = cd.shape[0]

    def tp(arr, ncol):  # (n, ncol) -> [128, ncol*16] k-major
        return np.ascontiguousarray(
            arr.reshape(P, J, ncol).transpose(0, 2, 1).reshape(P, ncol * J))

    th = np.full((n, KP), PI, np.float32)
    th[:, :K] = ca[:, 1:148]
    ph = np.zeros((n, KP), np.float32)
    ph[:, :K] = ct[:, :K]
    dd = np.zeros((n, KP), np.float32)
    dd[:, :K] = cd[:, 2:149]
    seed = np.stack([cd[:, 0], cd[:, 1], ca[:, 0]], axis=1)

    def tps(arr):  # (n, 50, 5) -> [128, 4000] (v, r, j)
        return np.ascontiguousarray(
            arr.reshape(P, J, N_RES, 5).transpose(0, 3, 2, 1).reshape(P, 4000))

    sph = np.empty((n, N_RES, 5), np.float32)
    sph[:, :, 0] = CHI0
    sph[:, :, 1:] = st.reshape(n, N_RES, V)
    return {
        "TH": tp(th, KP).astype(np.float16),
        "PH": tp(ph, KP).astype(np.float16),
        "DD": tp(dd, KP).astype(np.float16),
        "SEED": tp(seed, 3),
        "SD": tps(sd.reshape(n, N_RES, 5)).astype(np.float16),
        "SA": tps(sa.reshape(n, N_RES, 5)).astype(np.float16),
        "SPH": tps(sph).astype(np.float16),
    }


def kernel(central_distances, central_angles, central_dihedrals,
           side_distances, side_angles, side_dihedrals, **kw):
    nc = _build()
    in_maps = []
    for i in range(NCORES):
        sl_ = slice(i * S, (i + 1) * S)
        in_maps.append(_marshal_core(
            central_distances[sl_], central_angles[sl_],
            central_dihedrals[sl_], side_distances[sl_],
            side_angles[sl_], side_dihedrals[sl_]))
    res = run_bass_kernel_spmd(nc, in_maps, core_ids=list(range(NCORES)),
                               trace=TRACE, stitch_traces=False)
    _cache["last_results"] = res
    outs = []
    for i in range(NCORES):
        o = res.results[i]["OUT"].reshape(P, J, 400, 3).reshape(S, 400, 3)
        outs.append(o)
    return np.ascontiguousarray(np.concatenate(outs, axis=0))


# revision 46
# speedup vs baseline: 1.2082x; 1.0038x over previous
"""Trainium2 Bass kernel for nn_BackMapLayerWithSidechains.

Algorithm: the sequential NeRF atom-placement chain is reformulated as a
product of affine transforms (R, t): each step composes M = Rx(phi) @
Rz(pi - theta) and t += d * R[:, 0].  Composition is associative, so the
147-step backbone chain is evaluated with a blocked scan:
  - within-chunk prefixes (C=13 chunks x L=12 steps) computed serially but
    vectorized over (chunk, sample),
  - chunk prefixes composed serially (13 small composes),
  - one wide "apply" produces every global atom position.
Sidechain frames branch off backbone frames exactly:
  F_side(r) = F_global(3r-1) . Rx(phi_{3r-1})   (r >= 1; r=0 is identity at
  atom 1), then 5 more affine steps per residue, vectorized over
  (residue, sample).  Rotation state, trig tables, and inputs run in fp16
  (bounded values, 10-bit mantissa); global positions accumulate in fp32.
Serial phases split independent work (disjoint residues / sample lanes)
across DVE and GPSIMD; trig uses ACT Sin with the free affine; t-chain
updates ride GPSIMD (one-way dependency off the DVE critical path).

Data parallel over 8 NeuronCores: 2048 samples/core.  Host-side numpy only
reshapes/pads inputs into the on-chip layouts (no compute).
"""

import math
from contextlib import ExitStack

import numpy as np

import concourse.bacc as bacc
import concourse.bass as bass
import concourse.mybir as mybir
import concourse.tile as tile
from concourse.bass_utils import run_bass_kernel_spmd

F32 = mybir.dt.float32
BF16 = mybir.dt.bfloat16
F16 = mybir.dt.float16
AF = mybir.ActivationFunctionType

P = 128          # partitions
J = 16           # samples per partition (per core: 2048 = 128*16)
S = P * J        # samples per core
NCORES = 8
B = S * NCORES   # 16384

N_RES = 50
V = 4
NB = 150
NS = 250
K = 147          # real backbone steps
C = 13           # chunks
L = 12           # steps/chunk
KP = C * L       # 156 padded steps
NA = 159         # padded backbone atom count (3 + KP)
CHI0 = 2.0943951023931953
PI = math.pi

_cache = {}
TRACE = False


def _bcast_m(ap3):
    """[p, X, 16] -> [p, 3, X, 16] via 0-stride comp dim."""
    return ap3.unsqueeze(1).broadcast_to([P, 3] + list(ap3.shape[1:]))


def _emit(tc, io):
    nc = tc.nc
    V3 = nc.vector
    GP = nc.gpsimd

    TH_d, PH_d, DD_d, SEED_d, SD_d, SA_d, SPH_d, OUT_d = (
        io["TH"], io["PH"], io["DD"], io["SEED"], io["SD"], io["SA"],
        io["SPH"], io["OUT"])

    # Pools: strict LIFO release order.  Open order (outer->inner):
    # perm, side1, w, wt, trig1, dd, c3, thph
    es = ExitStack()
    p_perm = es.enter_context(tc.tile_pool(name="perm", bufs=1))
    p_side1 = es.enter_context(tc.tile_pool(name="side1", bufs=1))
    x_w = ExitStack()
    p_w = x_w.enter_context(tc.tile_pool(name="w", bufs=1))
    x_wt = ExitStack()
    p_wt = x_wt.enter_context(tc.tile_pool(name="wt", bufs=1))
    x_trig1 = ExitStack()
    p_trig1 = x_trig1.enter_context(tc.tile_pool(name="trig1", bufs=1))
    x_dd = ExitStack()
    p_dd = x_dd.enter_context(tc.tile_pool(name="dd", bufs=1))
    x_c3 = ExitStack()
    p_c3 = x_c3.enter_context(tc.tile_pool(name="c3", bufs=1))
    p_tmp = x_c3.enter_context(tc.tile_pool(name="wtmp", bufs=4))
    x_thph = ExitStack()
    p_thph = x_thph.enter_context(tc.tile_pool(name="thph", bufs=1))

    consts = {}

    def cst(val):
        if val not in consts:
            t = p_perm.tile([P, 1], F32, tag=f"cst{len(consts)}",
                            name=f"cst{len(consts)}")
            V3.memset(t[:], val)
            consts[val] = t[:]
        return consts[val]

    SEED = p_perm.tile([P, 48], F32, tag="SEED")
    CP = p_perm.tile([P, KP * J], F16, tag="CP")
    SP = p_perm.tile([P, KP * J], F16, tag="SP")
    A_all = p_perm.tile([P, C * 192], F32, tag="A_all")

    RS_a = [p_side1.tile([P, 3 * N_RES * J], F16, tag=f"RSa{i}",
                         name=f"RSa{i}") for i in range(3)]
    T0 = p_side1.tile([P, 3 * N_RES * J], F32, tag="T0")
    SD = p_side1.tile([P, 5 * N_RES * J], F16, tag="SDb")
    SCA = p_side1.tile([P, 5 * N_RES * J], F16, tag="SCA")
    SSA = p_side1.tile([P, 5 * N_RES * J], F16, tag="SSA")
    SCP = p_side1.tile([P, 5 * N_RES * J], F16, tag="SCP")
    SSP = p_side1.tile([P, 5 * N_RES * J], F16, tag="SSP")

    WC1 = p_w.tile([P, 3 * KP * J], F16, tag="WC1")
    WC2 = p_w.tile([P, 3 * KP * J], F16, tag="WC2")
    WT = p_wt.tile([P, 3 * KP * J], F16, tag="WT")
    CA_ = p_trig1.tile([P, KP * J], F16, tag="CA_")
    SA = p_trig1.tile([P, KP * J], F16, tag="SA")
    DD = p_dd.tile([P, KP * J], F16, tag="DD")
    SAs = p_dd.tile([P, 5 * N_RES * J], F16, tag="SAs")
    SPHs = p_dd.tile([P, 5 * N_RES * J], F16, tag="SPHs")
    c3t = [p_c3.tile([P, 3 * C * J], F16, tag=f"c3{i}", name=f"c3{i}")
           for i in range(2)]
    TH = p_thph.tile([P, KP * J], F16, tag="TH")
    PH = p_thph.tile([P, KP * J], F16, tag="PH")

    # ---------------- P0: input DMAs (backbone) ----------------
    nc.sync.dma_start(TH[:], TH_d.ap())
    nc.sync.dma_start(PH[:], PH_d.ap())
    nc.sync.dma_start(DD[:], DD_d.ap())
    nc.sync.dma_start(SEED[:], SEED_d.ap())
    nc.sync.dma_start(SD[:], SD_d.ap())
    nc.sync.dma_start(SAs[:], SA_d.ap())
    nc.sync.dma_start(SPHs[:], SPH_d.ap())

    # ---------------- P1: backbone trig (ACT), fp16 outputs ----------
    nc.scalar.activation(CA_[:], TH[:], AF.Sin, bias=cst(-PI / 2))
    nc.scalar.activation(SA[:], TH[:], AF.Sin)
    nc.scalar.activation(SP[:], PH[:], AF.Sin)
    nc.scalar.activation(PH[:], PH[:], AF.Abs)
    nc.scalar.activation(CP[:], PH[:], AF.Sin, scale=-1.0, bias=cst(PI / 2))
    x_thph.close()
    # sidechain trig early: overlaps the whole backbone on ACT
    nc.scalar.activation(SCA[:], SAs[:], AF.Sin, bias=cst(-PI / 2))
    nc.scalar.activation(SSA[:], SAs[:], AF.Sin)
    nc.scalar.activation(SSP[:], SPHs[:], AF.Sin)
    nc.scalar.activation(SPHs[:], SPHs[:], AF.Abs)
    nc.scalar.activation(SCP[:], SPHs[:], AF.Sin, scale=-1.0,
                         bias=cst(PI / 2))

    def wv(t):
        return t[:].rearrange("p (m k j) -> p m k j", m=3, k=KP, j=J)

    def wl(t, l):
        return t[:].rearrange("p (m c l j) -> p m c l j",
                              m=3, c=C, l=L, j=J)[:, :, :, l, :]

    def trig_l(t, l, bcast=True):
        s = t[:].rearrange("p (c l j) -> p c l j", c=C, l=L, j=J)[:, :, l, :]
        return _bcast_m(s) if bcast else s

    def c3v(i):
        return c3t[i][:].rearrange("p (m c j) -> p m c j", m=3, c=C, j=J)

    def anv(t):
        return t[:].rearrange("p (c n m j) -> p c n m j", c=C, n=4, m=3, j=J)

    Av = anv(A_all)

    # ---------------- P2a: within-chunk init (l = 0) ----------------
    w1_0, w2_0 = wl(WC1, 0), wl(WC2, 0)
    ca0, sa0 = trig_l(CA_, 0, False), trig_l(SA, 0, False)
    cp0, sp0 = trig_l(CP, 0, False), trig_l(SP, 0, False)
    nc.scalar.copy(w1_0[:, 0], ca0)
    V3.tensor_mul(w1_0[:, 1], sa0, cp0)
    V3.tensor_mul(w1_0[:, 2], sa0, sp0)
    V3.tensor_scalar_mul(w2_0[:, 0], sa0, -1.0)
    V3.tensor_mul(w2_0[:, 1], ca0, cp0)
    V3.tensor_mul(w2_0[:, 2], ca0, sp0)
    c3i = c3v(0)
    V3.memset(c3i[:, 0], 0.0)
    V3.tensor_scalar_mul(c3i[:, 1], sp0, -1.0)
    nc.scalar.copy(c3i[:, 2], cp0)
    GP.tensor_mul(wl(WT, 0), trig_l(DD, 0), wl(WC1, 0))

    # ---------------- P2b: within-chunk serial scan ----------------
    for l in range(1, L):
        cpl, spl = trig_l(CP, l), trig_l(SP, l)
        cal, sal = trig_l(CA_, l), trig_l(SA, l)
        p1, p2 = wl(WC1, l - 1), wl(WC2, l - 1)
        p3 = c3v((l - 1) % 2)
        c3c = c3v(l % 2)
        sh = [P, 3 * C * J]

        def tt(tag):
            return p_tmp.tile(sh, F16, tag=tag, name=tag)[:].rearrange(
                "p (m c j) -> p m c j", m=3, c=C, j=J)

        def ttf(tag):
            return p_tmp.tile(sh, F32, tag=tag, name=tag)[:].rearrange(
                "p (m c j) -> p m c j", m=3, c=C, j=J)

        t1, t2, c2x = tt("t1"), tt("t2"), tt("c2x")
        V3.tensor_mul(t1, cpl, p2)
        V3.tensor_mul(t2, spl, p3)
        V3.tensor_add(c2x, t1, t2)
        t1, t2 = tt("t1"), tt("t2")
        V3.tensor_mul(t1, cpl, p3)
        V3.tensor_mul(t2, spl, p2)
        V3.tensor_sub(c3c, t1, t2)
        t1, t2 = tt("t1"), tt("t2")
        V3.tensor_mul(t1, cal, p1)
        V3.tensor_mul(t2, sal, c2x)
        V3.tensor_add(wl(WC1, l), t1, t2)
        t1, t2 = tt("t1"), tt("t2")
        V3.tensor_mul(t1, cal, c2x)
        V3.tensor_mul(t2, sal, p1)
        V3.tensor_sub(wl(WC2, l), t1, t2)
        t9 = tt("t9")
        GP.tensor_mul(t9, trig_l(DD, l), wl(WC1, l))
        GP.tensor_add(wl(WT, l), wl(WT, l - 1), t9)

    # ---------------- P3a: chunk totals T ----------------
    # T lives in its own pool opened after closing c3/dd/trig1? T build
    # reads c3 -> keep c3 open until the copies below are emitted.
    x_T = ExitStack()
    p_T = x_T.enter_context(tc.tile_pool(name="Tp", bufs=1))
    p_u = x_T.enter_context(tc.tile_pool(name="up", bufs=4))
    T_all = p_T.tile([P, C * 192], F32, tag="T_all")
    Tv = anv(T_all)
    w1f, w2f = wl(WC1, L - 1), wl(WC2, L - 1)
    c3f, wtf = c3v((L - 1) % 2), wl(WT, L - 1)
    for c_ in range(C):
        nc.scalar.copy(Tv[:, c_, 0], w1f[:, :, c_, :])
        nc.scalar.copy(Tv[:, c_, 1], w2f[:, :, c_, :])
        nc.scalar.copy(Tv[:, c_, 2], c3f[:, :, c_, :])
        nc.scalar.copy(Tv[:, c_, 3], wtf[:, :, c_, :])

    # ---------------- P3b: seed A_0 = F0 ----------------
    sdv = SEED[:].rearrange("p (q j) -> p q j", q=3, j=J)
    cd0, cd1, ca0s = sdv[:, 0], sdv[:, 1], sdv[:, 2]
    sca0 = p_T.tile([P, J], F32, tag="sca0", name="sca0")[:]
    cca0 = p_T.tile([P, J], F32, tag="cca0", name="cca0")[:]
    nc.scalar.activation(sca0, ca0s, AF.Sin)
    nc.scalar.activation(cca0, ca0s, AF.Sin, bias=cst(-PI / 2))
    V3.memset(Av[:, 0], 0.0)
    nc.scalar.copy(Av[:, 0, 0, 0], cca0)
    nc.scalar.copy(Av[:, 0, 0, 1], sca0)
    V3.tensor_scalar_mul(Av[:, 0, 1, 0], sca0, -1.0)
    nc.scalar.copy(Av[:, 0, 1, 1], cca0)
    V3.memset(Av[:, 0, 2, 2], 1.0)
    tm0 = p_T.tile([P, J], F32, tag="tm0", name="tm0")[:]
    V3.tensor_mul(tm0, cd1, cca0)
    V3.tensor_add(Av[:, 0, 3, 0], cd0, tm0)
    V3.tensor_mul(Av[:, 0, 3, 1], cd1, sca0)

    # ---------------- P3c: chunk-prefix chain ----------------
    JD = 12  # sample-lane split: j 0..11 DVE, 12..15 GPSIMD (independent)
    for c in range(1, C):
        for EE, jl, jh, sfx in ((V3, 0, JD, "d"), (GP, JD, J, "g")):
            nj = jh - jl
            acm = [Av[:, c - 1, m, :, jl:jh].unsqueeze(1).broadcast_to(
                [P, 4, 3, nj]) for m in range(3)]
            trm = [Tv[:, c - 1, :, m, jl:jh].unsqueeze(2).broadcast_to(
                [P, 4, 3, nj]) for m in range(3)]

            def uu(tag):
                return p_u.tile([P, 12 * nj], F32, tag=tag + sfx,
                                name=tag + sfx)[:].rearrange(
                    "p (n m j) -> p n m j", n=4, m=3, j=nj)

            u1, u2, u3 = uu("u1"), uu("u2"), uu("u3")
            EE.tensor_mul(u1, acm[0], trm[0])
            EE.tensor_mul(u2, acm[1], trm[1])
            EE.tensor_add(u3, u1, u2)
            u1 = uu("u1")
            EE.tensor_mul(u1, acm[2], trm[2])
            EE.tensor_add(Av[:, c, :, :, jl:jh], u3, u1)
            EE.tensor_add(Av[:, c, 3, :, jl:jh], Av[:, c, 3, :, jl:jh],
                          Av[:, c - 1, 3, :, jl:jh])

    # close inner pools (LIFO): Tp/up, then c3, dd, trig1
    x_T.close()
    x_c3.close()
    x_dd.close()
    x_trig1.close()

    # ---------------- P8: sidechain seeds ----------------
    x_seed = ExitStack()
    p_sd = x_seed.enter_context(tc.tile_pool(name="seedp", bufs=1))

    def rsv(t):
        return t[:].rearrange("p (m r j) -> p m r j", m=3, r=N_RES, j=J)

    AEX = [p_sd.tile([P, 3 * 49 * J], F16, tag=f"AEX{m}", name=f"AEX{m}")
           for m in range(3)]

    def aexv(t):
        return t[:].rearrange("p (m r j) -> p m r j", m=3, r=49, j=J)

    for m in range(3):
        dst = aexv(AEX[m])
        for cq in range(12):
            nc.scalar.copy(
                dst[:, :, 4 * cq:4 * cq + 4, :],
                Av[:, cq, m].unsqueeze(2).broadcast_to([P, 3, 4, J]))
        nc.scalar.copy(dst[:, :, 48, :], Av[:, 12, m])

    w1g = wv(WC1)[:, :, 1:146:3, :]
    w2g = wv(WC2)[:, :, 1:146:3, :]
    C2AW = p_sd.tile([P, 3 * 49 * J], F16, tag="C2AW")
    C3AW = p_sd.tile([P, 3 * 49 * J], F16, tag="C3AW")
    c2awv, c3awv = aexv(C2AW), aexv(C3AW)
    kj = lambda t: t[:].rearrange("p (k j) -> p k j", k=KP, j=J)
    cpb = _bcast_m(kj(CP)[:, 2:147:3, :])
    spb = _bcast_m(kj(SP)[:, 2:147:3, :])
    rs1a, rs2a, rs3a = rsv(RS_a[0]), rsv(RS_a[1]), rsv(RS_a[2])
    RSD = 39  # gather-index split: [0:RSD] DVE, [RSD:49] GPSIMD

    for EE, lo, hi, sfx in ((V3, 0, RSD, "d"), (GP, RSD, 49, "g")):
        n = hi - lo

        def sdt(tag):
            return p_sd.tile([P, 3 * n * J], F16, tag=tag + sfx,
                             name=tag + sfx)[:].rearrange(
                "p (m r j) -> p m r j", m=3, r=n, j=J)

        def rs(ap):
            return ap[:, :, lo:hi, :]

        def rsm(ap, m):
            return ap[:, m, lo:hi, :].unsqueeze(1).broadcast_to(
                [P, 3, n, J])

        sAv, sBv = sdt("sA"), sdt("sB")

        def apply_R(wg, outv):
            EE.tensor_mul(sAv, rs(aexv(AEX[0])), rsm(wg, 0))
            EE.tensor_mul(sBv, rs(aexv(AEX[1])), rsm(wg, 1))
            EE.tensor_add(sAv, sAv, sBv)
            EE.tensor_mul(sBv, rs(aexv(AEX[2])), rsm(wg, 2))
            EE.tensor_add(outv, sAv, sBv)

        apply_R(w1g, rs(rs1a[:, :, 1:, :]))
        apply_R(w2g, rs(c2awv))
        for mo in range(3):
            m1, m2 = (mo + 1) % 3, (mo + 2) % 3
            EE.tensor_mul(sAv[:, 0], rs1a[:, m1, 1 + lo:1 + hi, :],
                          c2awv[:, m2, lo:hi, :])
            EE.tensor_mul(sBv[:, 0], rs1a[:, m2, 1 + lo:1 + hi, :],
                          c2awv[:, m1, lo:hi, :])
            EE.tensor_sub(c3awv[:, mo, lo:hi, :], sAv[:, 0], sBv[:, 0])
        EE.tensor_mul(sAv, rs(cpb), rs(c2awv))
        EE.tensor_mul(sBv, rs(spb), rs(c3awv))
        EE.tensor_add(rs(rs2a[:, :, 1:, :]), sAv, sBv)
        EE.tensor_mul(sAv, rs(cpb), rs(c3awv))
        EE.tensor_mul(sBv, rs(spb), rs(c2awv))
        EE.tensor_sub(rs(rs3a[:, :, 1:, :]), sAv, sBv)

    for m in range(3):
        rm = rsv(RS_a[m])
        V3.memset(rm[:, :, 0, :], 0.0)
        V3.memset(rm[:, m, 0, :], 1.0)
    x_seed.close()

    # ---------------- P4: apply -> backbone AoS ----------------
    x_bbaos = ExitStack()
    p_bb = x_bbaos.enter_context(tc.tile_pool(name="bbaos", bufs=1))
    BBAOS = p_bb.tile([P, J * NA * 3], F32, tag="BBAOS")

    def bbv():
        return BBAOS[:].rearrange("p (j a m) -> p j a m", j=J, a=NA, m=3)

    x_apply = ExitStack()
    p_ap = x_apply.enter_context(tc.tile_pool(name="applyp", bufs=3))
    wtv = WT[:].rearrange("p (m c l j) -> p m c l j", m=3, c=C, l=L, j=J)
    for c in range(C):
        # [p, comp, L, J] per chunk (3 free dims: walrus TENSOR3D limit)
        aw = [Av[:, c, m].unsqueeze(2).broadcast_to([P, 3, L, J])
              for m in range(3)]
        at = Av[:, c, 3].unsqueeze(2).broadcast_to([P, 3, L, J])
        wtm = [wtv[:, m, c].unsqueeze(1).broadcast_to([P, 3, L, J])
               for m in range(3)]

        EE = GP if c >= 10 else V3

        def qq(tag):
            return p_ap.tile([P, 3 * L * J], F16, tag=tag,
                             name=f"{tag}_{c}")[:].rearrange(
                "p (m l j) -> p m l j", m=3, l=L, j=J)

        q1, q2 = qq("q1"), qq("q2")
        EE.tensor_mul(q1, aw[0], wtm[0])
        EE.tensor_mul(q2, aw[1], wtm[1])
        EE.tensor_add(q1, q1, q2)
        q2 = qq("q2")
        EE.tensor_mul(q2, aw[2], wtm[2])
        EE.tensor_add(q1, q1, q2)
        outv = bbv()[:, :, 3 + c * L:3 + (c + 1) * L, :].transpose(
            [0, 3, 2, 1])
        EE.tensor_add(outv, q1, at)
    x_apply.close()

    # ---------------- P5: atoms 0,1,2 ----------------
    bb = bbv()
    V3.memset(bb[:, :, 0:2, :], 0.0)
    nc.scalar.copy(bb[:, :, 1, 0], cd0)
    V3.memset(bb[:, :, 2, 2], 0.0)
    nc.scalar.copy(bb[:, :, 2, 0], Av[:, 0, 3, 0])
    nc.scalar.copy(bb[:, :, 2, 1], Av[:, 0, 3, 1])

    # ---------------- P6: backbone DMA out + T0 gather ----------------
    outdv = OUT_d.ap().rearrange("p (j a m) -> p j a m", j=J, a=400, m=3)
    nc.sync.dma_start(outdv[:, :, 0:87, :], bb[:, :, 0:87, :])
    nc.sync.dma_start(outdv[:, :, 87:NB, :], bb[:, :, 87:NB, :])
    t0v = T0[:].rearrange("p (m r j) -> p m r j", m=3, r=N_RES, j=J)
    nc.scalar.copy(t0v, bb[:, :, 1:NB:3, :].transpose([0, 3, 2, 1]))
    x_bbaos.close()
    x_wt.close()
    x_w.close()


    # ---------------- P7: sidechain inputs + trig ----------------
    x_side2 = ExitStack()
    p_side2 = x_side2.enter_context(tc.tile_pool(name="side2", bufs=1))
    RS_b = [p_side2.tile([P, 3 * N_RES * J], F16, tag=f"RSb{i}",
                         name=f"RSb{i}") for i in range(3)]
    SIDEAOS = p_side2.tile([P, J * NS * 3], F32, tag="SIDEAOS")

    # ---------------- P9: sidechain serial chain ----------------
    sav = SIDEAOS[:].rearrange("p (j r v m) -> p j r v m",
                               j=J, r=N_RES, v=5, m=3)
    p_st = x_side2.enter_context(tc.tile_pool(name="sidetmp", bufs=3))

    def sl(t, v):
        s = t[:].rearrange("p (v r j) -> p v r j", v=5, r=N_RES, j=J)[:, v]
        return _bcast_m(s)

    cur, nxt = RS_a, RS_b
    RD = 40  # residues 0..RD-1 on DVE, RD..49 on GPSIMD (independent chains)
    for v in range(5):
        cpl, spl = sl(SCP, v), sl(SSP, v)
        cal, sal = sl(SCA, v), sl(SSA, v)
        sdl = sl(SD, v)
        r1p, r2p, r3p = rsv(cur[0]), rsv(cur[1]), rsv(cur[2])
        r1c, r2c, r3c = rsv(nxt[0]), rsv(nxt[1]), rsv(nxt[2])
        out = sav[:, :, :, v, :].transpose([0, 3, 2, 1])
        prev = (t0v if v == 0
                else sav[:, :, :, v - 1, :].transpose([0, 3, 2, 1]))

        for EE, lo, hi, sfx in ((V3, 0, RD, "d"), (GP, RD, N_RES, "g")):
            n = hi - lo

            def stp(tag):
                return p_st.tile([P, 3 * n * J], F16, tag=tag + sfx,
                                 name=tag + sfx)[:].rearrange(
                    "p (m r j) -> p m r j", m=3, r=n, j=J)

            def rs(ap):
                return ap[:, :, lo:hi, :]

            t1, t2, c2x = stp("s1"), stp("s2"), stp("sc2x")
            EE.tensor_mul(t1, rs(cpl), rs(r2p))
            EE.tensor_mul(t2, rs(spl), rs(r3p))
            EE.tensor_add(c2x, t1, t2)
            if v < 4:
                t1, t2 = stp("s1"), stp("s2")
                EE.tensor_mul(t1, rs(cpl), rs(r3p))
                EE.tensor_mul(t2, rs(spl), rs(r2p))
                EE.tensor_sub(rs(r3c), t1, t2)
            t1, t2 = stp("s1"), stp("s2")
            EE.tensor_mul(t1, rs(cal), rs(r1p))
            EE.tensor_mul(t2, rs(sal), c2x)
            EE.tensor_add(rs(r1c), t1, t2)
            if v < 4:
                t1, t2 = stp("s1"), stp("s2")
                EE.tensor_mul(t1, rs(cal), c2x)
                EE.tensor_mul(t2, rs(sal), rs(r1p))
                EE.tensor_sub(rs(r2c), t1, t2)
            if v == 4 and sfx == "d":
                h = n // 2
                for a, b in ((0, h), (h, n)):
                    t9h = p_st.tile([P, 3 * (b - a) * J], F16,
                                    tag="st9" + sfx, name=f"st9h{a}")[:]                        .rearrange("p (m r j) -> p m r j",
                                   m=3, r=b - a, j=J)
                    EE.tensor_mul(t9h, sdl[:, :, lo + a:lo + b, :],
                                  r1c[:, :, lo + a:lo + b, :])
                    EE.tensor_add(out[:, :, lo + a:lo + b, :],
                                  prev[:, :, lo + a:lo + b, :], t9h)
            else:
                t9 = stp("st9")
                EE.tensor_mul(t9, rs(sdl), rs(r1c))
                EE.tensor_add(rs(out), rs(prev), t9)
        cur, nxt = nxt, cur

    # ---------------- P10: sidechain DMA out ----------------
    sflat = SIDEAOS[:].rearrange("p (j x) -> p j x", j=J, x=NS * 3)
    dflat = OUT_d.ap().rearrange("p (j a m) -> p j a m", j=J, a=400, m=3)[
        :, :, NB:400, :].rearrange("p j a m -> p j (a m)")
    nc.sync.dma_start(dflat[:, :, 600:750], sflat[:, :, 600:750])
    nc.sync.dma_start(dflat[:, :, 0:300], sflat[:, :, 0:300])
    nc.sync.dma_start(dflat[:, :, 300:600], sflat[:, :, 300:600])

    x_side2.close()
    es.close()


def _build():
    if "nc" in _cache:
        return _cache["nc"]
    nc = bacc.Bacc("TRN2", target_bir_lowering=False, debug=False,
                   num_devices=NCORES)
    io = {
        "TH": nc.dram_tensor("TH", [P, KP * J], F16, kind="ExternalInput"),
        "PH": nc.dram_tensor("PH", [P, KP * J], F16, kind="ExternalInput"),
        "DD": nc.dram_tensor("DD", [P, KP * J], F16, kind="ExternalInput"),
        "SEED": nc.dram_tensor("SEED", [P, 48], F32, kind="ExternalInput"),
        "SD": nc.dram_tensor("SD", [P, 5 * N_RES * J], F16,
                             kind="ExternalInput"),
        "SA": nc.dram_tensor("SA", [P, 5 * N_RES * J], F16,
                             kind="ExternalInput"),
        "SPH": nc.dram_tensor("SPH", [P, 5 * N_RES * J], F16,
                              kind="ExternalInput"),
        "OUT": nc.dram_tensor("OUT", [P, J * 400 * 3], F32,
                              kind="ExternalOutput"),
    }
    with tile.TileContext(nc) as tc:
        _emit(tc, io)
    nc.compile()
    _cache["nc"] = nc
    return nc


def _marshal_core(cd, ca, ct, sd, sa, st):
    """Per-core (S samples) numpy repack into [128, free] layouts."""
    n = cd.shape[0]

    def tp(arr, ncol):  # (n, ncol) -> [128, ncol*16] k-major
        return np.ascontiguousarray(
            arr.reshape(P, J, ncol).transpose(0, 2, 1).reshape(P, ncol * J))

    th = np.full((n, KP), PI, np.float32)
    th[:, :K] = ca[:, 1:148]
    ph = np.zeros((n, KP), np.float32)
    ph[:, :K] = ct[:, :K]
    dd = np.zeros((n, KP), np.float32)
    dd[:, :K] = cd[:, 2:149]
    seed = np.stack([cd[:, 0], cd[:, 1], ca[:, 0]], axis=1)

    def tps(arr):  # (n, 50, 5) -> [128, 4000] (v, r, j)
        return np.ascontiguousarray(
            arr.reshape(P, J, N_RES, 5).transpose(0, 3, 2, 1).reshape(P, 4000))

    sph = np.empty((n, N_RES, 5), np.float32)
    sph[:, :, 0] = CHI0
    sph[:, :, 1:] = st.reshape(n, N_RES, V)
    return {
        "TH": tp(th, KP).astype(np.float16),
        "PH": tp(ph, KP).astype(np.float16),
        "DD": tp(dd, KP).astype(np.float16),
        "SEED": tp(seed, 3),
        "SD": tps(sd.reshape(n, N_RES, 5)).astype(np.float16),
        "SA": tps(sa.reshape(n, N_RES, 5)).astype(np.float16),
        "SPH": tps(sph).astype(np.float16),
    }


def kernel(central_distances, central_angles, central_dihedrals,
           side_distances, side_angles, side_dihedrals, **kw):
    nc = _build()
    in_maps = []
    for i in range(NCORES):
        sl_ = slice(i * S, (i + 1) * S)
        in_maps.append(_marshal_core(
            central_distances[sl_], central_angles[sl_],
            central_dihedrals[sl_], side_distances[sl_],
            side_angles[sl_], side_dihedrals[sl_]))
    res = run_bass_kernel_spmd(nc, in_maps, core_ids=list(range(NCORES)),
                               trace=TRACE, stitch_traces=False)
    _cache["last_results"] = res
    outs = []
    for i in range(NCORES):
        o = res.results[i]["OUT"].reshape(P, J, 400, 3).reshape(S, 400, 3)
        outs.append(o)
    return np.ascontiguousarray(np.concatenate(outs, axis=0))


# revision 47
# speedup vs baseline: 1.2127x; 1.0037x over previous
"""Trainium2 Bass kernel for nn_BackMapLayerWithSidechains.

Algorithm: the sequential NeRF atom-placement chain is reformulated as a
product of affine transforms (R, t): each step composes M = Rx(phi) @
Rz(pi - theta) and t += d * R[:, 0].  Composition is associative, so the
147-step backbone chain is evaluated with a blocked scan:
  - within-chunk prefixes (C=13 chunks x L=12 steps) computed serially but
    vectorized over (chunk, sample),
  - chunk prefixes composed serially (13 small composes),
  - one wide "apply" produces every global atom position.
Sidechain frames branch off backbone frames exactly:
  F_side(r) = F_global(3r-1) . Rx(phi_{3r-1})   (r >= 1; r=0 is identity at
  atom 1), then 5 more affine steps per residue, vectorized over
  (residue, sample).  Rotation state, trig tables, and inputs run in fp16
  (bounded values, 10-bit mantissa); global positions accumulate in fp32.
Serial phases split independent work (disjoint residues / sample lanes)
across DVE and GPSIMD; trig uses ACT Sin with the free affine; t-chain
updates ride GPSIMD (one-way dependency off the DVE critical path).

Data parallel over 8 NeuronCores: 2048 samples/core.  Host-side numpy only
reshapes/pads inputs into the on-chip layouts (no compute).
"""

import math
from contextlib import ExitStack

import numpy as np

import concourse.bacc as bacc
import concourse.bass as bass
import concourse.mybir as mybir
import concourse.tile as tile
from concourse.bass_utils import run_bass_kernel_spmd

F32 = mybir.dt.float32
BF16 = mybir.dt.bfloat16
F16 = mybir.dt.float16
AF = mybir.ActivationFunctionType

P = 128          # partitions
J = 16           # samples per partition (per core: 2048 = 128*16)
S = P * J        # samples per core
NCORES = 8
B = S * NCORES   # 16384

N_RES = 50
V = 4
NB = 150
NS = 250
K = 147          # real backbone steps
C = 13           # chunks
L = 12           # steps/chunk
KP = C * L       # 156 padded steps
NA = 159         # padded backbone atom count (3 + KP)
CHI0 = 2.0943951023931953
PI = math.pi

_cache = {}
TRACE = False


def _bcast_m(ap3):
    """[p, X, 16] -> [p, 3, X, 16] via 0-stride comp dim."""
    return ap3.unsqueeze(1).broadcast_to([P, 3] + list(ap3.shape[1:]))


def _emit(tc, io):
    nc = tc.nc
    V3 = nc.vector
    GP = nc.gpsimd

    TH_d, PH_d, DD_d, SEED_d, SD_d, SA_d, SPH_d, OUT_d = (
        io["TH"], io["PH"], io["DD"], io["SEED"], io["SD"], io["SA"],
        io["SPH"], io["OUT"])

    # Pools: strict LIFO release order.  Open order (outer->inner):
    # perm, side1, w, wt, trig1, dd, c3, thph
    es = ExitStack()
    p_perm = es.enter_context(tc.tile_pool(name="perm", bufs=1))
    p_side1 = es.enter_context(tc.tile_pool(name="side1", bufs=1))
    x_w = ExitStack()
    p_w = x_w.enter_context(tc.tile_pool(name="w", bufs=1))
    x_wt = ExitStack()
    p_wt = x_wt.enter_context(tc.tile_pool(name="wt", bufs=1))
    x_trig1 = ExitStack()
    p_trig1 = x_trig1.enter_context(tc.tile_pool(name="trig1", bufs=1))
    x_dd = ExitStack()
    p_dd = x_dd.enter_context(tc.tile_pool(name="dd", bufs=1))
    x_c3 = ExitStack()
    p_c3 = x_c3.enter_context(tc.tile_pool(name="c3", bufs=1))
    p_tmp = x_c3.enter_context(tc.tile_pool(name="wtmp", bufs=4))
    x_thph = ExitStack()
    p_thph = x_thph.enter_context(tc.tile_pool(name="thph", bufs=1))

    consts = {}

    def cst(val):
        if val not in consts:
            t = p_perm.tile([P, 1], F32, tag=f"cst{len(consts)}",
                            name=f"cst{len(consts)}")
            V3.memset(t[:], val)
            consts[val] = t[:]
        return consts[val]

    SEED = p_perm.tile([P, 48], F32, tag="SEED")
    CP = p_perm.tile([P, KP * J], F16, tag="CP")
    SP = p_perm.tile([P, KP * J], F16, tag="SP")
    A_all = p_perm.tile([P, C * 192], F32, tag="A_all")

    RS_a = [p_side1.tile([P, 3 * N_RES * J], F16, tag=f"RSa{i}",
                         name=f"RSa{i}") for i in range(3)]
    T0 = p_side1.tile([P, 3 * N_RES * J], F32, tag="T0")
    SD = p_side1.tile([P, 5 * N_RES * J], F16, tag="SDb")
    SCA = p_side1.tile([P, 5 * N_RES * J], F16, tag="SCA")
    SSA = p_side1.tile([P, 5 * N_RES * J], F16, tag="SSA")
    SCP = p_side1.tile([P, 5 * N_RES * J], F16, tag="SCP")
    SSP = p_side1.tile([P, 5 * N_RES * J], F16, tag="SSP")

    WC1 = p_w.tile([P, 3 * KP * J], F16, tag="WC1")
    WC2 = p_w.tile([P, 3 * KP * J], F16, tag="WC2")
    WT = p_wt.tile([P, 3 * KP * J], F16, tag="WT")
    CA_ = p_trig1.tile([P, KP * J], F16, tag="CA_")
    SA = p_trig1.tile([P, KP * J], F16, tag="SA")
    DD = p_dd.tile([P, KP * J], F16, tag="DD")
    SAs = p_dd.tile([P, 5 * N_RES * J], F16, tag="SAs")
    SPHs = p_dd.tile([P, 5 * N_RES * J], F16, tag="SPHs")
    c3t = [p_c3.tile([P, 3 * C * J], F16, tag=f"c3{i}", name=f"c3{i}")
           for i in range(2)]
    TH = p_thph.tile([P, KP * J], F16, tag="TH")
    PH = p_thph.tile([P, KP * J], F16, tag="PH")

    # ---------------- P0: input DMAs (backbone) ----------------
    nc.sync.dma_start(TH[:], TH_d.ap())
    nc.sync.dma_start(PH[:], PH_d.ap())
    nc.sync.dma_start(DD[:], DD_d.ap())
    nc.sync.dma_start(SEED[:], SEED_d.ap())
    nc.sync.dma_start(SD[:], SD_d.ap())
    nc.sync.dma_start(SAs[:], SA_d.ap())
    nc.sync.dma_start(SPHs[:], SPH_d.ap())

    # ---------------- P1: backbone trig (ACT), fp16 outputs ----------
    nc.scalar.activation(CA_[:], TH[:], AF.Sin, bias=cst(-PI / 2))
    nc.scalar.activation(SA[:], TH[:], AF.Sin)
    nc.scalar.activation(SP[:], PH[:], AF.Sin)
    nc.scalar.activation(PH[:], PH[:], AF.Abs)
    nc.scalar.activation(CP[:], PH[:], AF.Sin, scale=-1.0, bias=cst(PI / 2))
    x_thph.close()
    # sidechain trig early: overlaps the whole backbone on ACT
    nc.scalar.activation(SCA[:], SAs[:], AF.Sin, bias=cst(-PI / 2))
    nc.scalar.activation(SSA[:], SAs[:], AF.Sin)
    nc.scalar.activation(SSP[:], SPHs[:], AF.Sin)
    nc.scalar.activation(SPHs[:], SPHs[:], AF.Abs)
    nc.scalar.activation(SCP[:], SPHs[:], AF.Sin, scale=-1.0,
                         bias=cst(PI / 2))

    def wv(t):
        return t[:].rearrange("p (m k j) -> p m k j", m=3, k=KP, j=J)

    def wl(t, l):
        return t[:].rearrange("p (m c l j) -> p m c l j",
                              m=3, c=C, l=L, j=J)[:, :, :, l, :]

    def trig_l(t, l, bcast=True):
        s = t[:].rearrange("p (c l j) -> p c l j", c=C, l=L, j=J)[:, :, l, :]
        return _bcast_m(s) if bcast else s

    def c3v(i):
        return c3t[i][:].rearrange("p (m c j) -> p m c j", m=3, c=C, j=J)

    def anv(t):
        return t[:].rearrange("p (c n m j) -> p c n m j", c=C, n=4, m=3, j=J)

    Av = anv(A_all)

    # ---------------- P2a: within-chunk init (l = 0) ----------------
    w1_0, w2_0 = wl(WC1, 0), wl(WC2, 0)
    ca0, sa0 = trig_l(CA_, 0, False), trig_l(SA, 0, False)
    cp0, sp0 = trig_l(CP, 0, False), trig_l(SP, 0, False)
    nc.scalar.copy(w1_0[:, 0], ca0)
    V3.tensor_mul(w1_0[:, 1], sa0, cp0)
    V3.tensor_mul(w1_0[:, 2], sa0, sp0)
    V3.tensor_scalar_mul(w2_0[:, 0], sa0, -1.0)
    V3.tensor_mul(w2_0[:, 1], ca0, cp0)
    V3.tensor_mul(w2_0[:, 2], ca0, sp0)
    c3i = c3v(0)
    V3.memset(c3i[:, 0], 0.0)
    V3.tensor_scalar_mul(c3i[:, 1], sp0, -1.0)
    nc.scalar.copy(c3i[:, 2], cp0)
    GP.tensor_mul(wl(WT, 0), trig_l(DD, 0), wl(WC1, 0))

    # ---------------- P2b: within-chunk serial scan ----------------
    for l in range(1, L):
        cpl, spl = trig_l(CP, l), trig_l(SP, l)
        cal, sal = trig_l(CA_, l), trig_l(SA, l)
        p1, p2 = wl(WC1, l - 1), wl(WC2, l - 1)
        p3 = c3v((l - 1) % 2)
        c3c = c3v(l % 2)
        sh = [P, 3 * C * J]

        def tt(tag):
            return p_tmp.tile(sh, F16, tag=tag, name=tag)[:].rearrange(
                "p (m c j) -> p m c j", m=3, c=C, j=J)

        def ttf(tag):
            return p_tmp.tile(sh, F32, tag=tag, name=tag)[:].rearrange(
                "p (m c j) -> p m c j", m=3, c=C, j=J)

        t1, t2, c2x = tt("t1"), tt("t2"), tt("c2x")
        V3.tensor_mul(t1, cpl, p2)
        V3.tensor_mul(t2, spl, p3)
        V3.tensor_add(c2x, t1, t2)
        t1, t2 = tt("t1"), tt("t2")
        V3.tensor_mul(t1, cpl, p3)
        V3.tensor_mul(t2, spl, p2)
        V3.tensor_sub(c3c, t1, t2)
        t1, t2 = tt("t1"), tt("t2")
        V3.tensor_mul(t1, cal, p1)
        V3.tensor_mul(t2, sal, c2x)
        V3.tensor_add(wl(WC1, l), t1, t2)
        t1, t2 = tt("t1"), tt("t2")
        V3.tensor_mul(t1, cal, c2x)
        V3.tensor_mul(t2, sal, p1)
        V3.tensor_sub(wl(WC2, l), t1, t2)
        t9 = tt("t9")
        GP.tensor_mul(t9, trig_l(DD, l), wl(WC1, l))
        GP.tensor_add(wl(WT, l), wl(WT, l - 1), t9)

    # ---------------- P3a: chunk totals T ----------------
    # T lives in its own pool opened after closing c3/dd/trig1? T build
    # reads c3 -> keep c3 open until the copies below are emitted.
    x_T = ExitStack()
    p_T = x_T.enter_context(tc.tile_pool(name="Tp", bufs=1))
    p_u = x_T.enter_context(tc.tile_pool(name="up", bufs=4))
    T_all = p_T.tile([P, C * 192], F32, tag="T_all")
    Tv = anv(T_all)
    w1f, w2f = wl(WC1, L - 1), wl(WC2, L - 1)
    c3f, wtf = c3v((L - 1) % 2), wl(WT, L - 1)
    for c_ in range(C):
        nc.scalar.copy(Tv[:, c_, 0], w1f[:, :, c_, :])
        nc.scalar.copy(Tv[:, c_, 1], w2f[:, :, c_, :])
        nc.scalar.copy(Tv[:, c_, 2], c3f[:, :, c_, :])
        nc.scalar.copy(Tv[:, c_, 3], wtf[:, :, c_, :])

    # ---------------- P3b: seed A_0 = F0 ----------------
    sdv = SEED[:].rearrange("p (q j) -> p q j", q=3, j=J)
    cd0, cd1, ca0s = sdv[:, 0], sdv[:, 1], sdv[:, 2]
    sca0 = p_T.tile([P, J], F32, tag="sca0", name="sca0")[:]
    cca0 = p_T.tile([P, J], F32, tag="cca0", name="cca0")[:]
    nc.scalar.activation(sca0, ca0s, AF.Sin)
    nc.scalar.activation(cca0, ca0s, AF.Sin, bias=cst(-PI / 2))
    V3.memset(Av[:, 0], 0.0)
    nc.scalar.copy(Av[:, 0, 0, 0], cca0)
    nc.scalar.copy(Av[:, 0, 0, 1], sca0)
    V3.tensor_scalar_mul(Av[:, 0, 1, 0], sca0, -1.0)
    nc.scalar.copy(Av[:, 0, 1, 1], cca0)
    V3.memset(Av[:, 0, 2, 2], 1.0)
    tm0 = p_T.tile([P, J], F32, tag="tm0", name="tm0")[:]
    V3.tensor_mul(tm0, cd1, cca0)
    V3.tensor_add(Av[:, 0, 3, 0], cd0, tm0)
    V3.tensor_mul(Av[:, 0, 3, 1], cd1, sca0)

    # ---------------- P3c: chunk-prefix chain ----------------
    JD = 12  # sample-lane split: j 0..11 DVE, 12..15 GPSIMD (independent)
    for c in range(1, C):
        for EE, jl, jh, sfx in ((V3, 0, JD, "d"), (GP, JD, J, "g")):
            nj = jh - jl
            acm = [Av[:, c - 1, m, :, jl:jh].unsqueeze(1).broadcast_to(
                [P, 4, 3, nj]) for m in range(3)]
            trm = [Tv[:, c - 1, :, m, jl:jh].unsqueeze(2).broadcast_to(
                [P, 4, 3, nj]) for m in range(3)]

            def uu(tag):
                return p_u.tile([P, 12 * nj], F32, tag=tag + sfx,
                                name=tag + sfx)[:].rearrange(
                    "p (n m j) -> p n m j", n=4, m=3, j=nj)

            u1, u2, u3 = uu("u1"), uu("u2"), uu("u3")
            EE.tensor_mul(u1, acm[0], trm[0])
            EE.tensor_mul(u2, acm[1], trm[1])
            EE.tensor_add(u3, u1, u2)
            u1 = uu("u1")
            EE.tensor_mul(u1, acm[2], trm[2])
            EE.tensor_add(Av[:, c, :, :, jl:jh], u3, u1)
            EE.tensor_add(Av[:, c, 3, :, jl:jh], Av[:, c, 3, :, jl:jh],
                          Av[:, c - 1, 3, :, jl:jh])

    # close inner pools (LIFO): Tp/up, then c3, dd, trig1
    x_T.close()
    x_c3.close()
    x_dd.close()
    x_trig1.close()

    # ---------------- P8: sidechain seeds ----------------
    x_seed = ExitStack()
    p_sd = x_seed.enter_context(tc.tile_pool(name="seedp", bufs=1))

    def rsv(t):
        return t[:].rearrange("p (m r j) -> p m r j", m=3, r=N_RES, j=J)

    AEX = [p_sd.tile([P, 3 * 49 * J], F16, tag=f"AEX{m}", name=f"AEX{m}")
           for m in range(3)]

    def aexv(t):
        return t[:].rearrange("p (m r j) -> p m r j", m=3, r=49, j=J)

    for m in range(3):
        dst = aexv(AEX[m])
        for cq in range(12):
            nc.scalar.copy(
                dst[:, :, 4 * cq:4 * cq + 4, :],
                Av[:, cq, m].unsqueeze(2).broadcast_to([P, 3, 4, J]))
        nc.scalar.copy(dst[:, :, 48, :], Av[:, 12, m])

    w1g = wv(WC1)[:, :, 1:146:3, :]
    w2g = wv(WC2)[:, :, 1:146:3, :]
    C2AW = p_sd.tile([P, 3 * 49 * J], F16, tag="C2AW")
    C3AW = p_sd.tile([P, 3 * 49 * J], F16, tag="C3AW")
    c2awv, c3awv = aexv(C2AW), aexv(C3AW)
    kj = lambda t: t[:].rearrange("p (k j) -> p k j", k=KP, j=J)
    cpb = _bcast_m(kj(CP)[:, 2:147:3, :])
    spb = _bcast_m(kj(SP)[:, 2:147:3, :])
    rs1a, rs2a, rs3a = rsv(RS_a[0]), rsv(RS_a[1]), rsv(RS_a[2])
    RSD = 39  # gather-index split: [0:RSD] DVE, [RSD:49] GPSIMD

    for EE, lo, hi, sfx in ((V3, 0, RSD, "d"), (GP, RSD, 49, "g")):
        n = hi - lo

        def sdt(tag):
            return p_sd.tile([P, 3 * n * J], F16, tag=tag + sfx,
                             name=tag + sfx)[:].rearrange(
                "p (m r j) -> p m r j", m=3, r=n, j=J)

        def rs(ap):
            return ap[:, :, lo:hi, :]

        def rsm(ap, m):
            return ap[:, m, lo:hi, :].unsqueeze(1).broadcast_to(
                [P, 3, n, J])

        sAv, sBv = sdt("sA"), sdt("sB")

        def apply_R(wg, outv):
            EE.tensor_mul(sAv, rs(aexv(AEX[0])), rsm(wg, 0))
            EE.tensor_mul(sBv, rs(aexv(AEX[1])), rsm(wg, 1))
            EE.tensor_add(sAv, sAv, sBv)
            EE.tensor_mul(sBv, rs(aexv(AEX[2])), rsm(wg, 2))
            EE.tensor_add(outv, sAv, sBv)

        apply_R(w1g, rs(rs1a[:, :, 1:, :]))
        apply_R(w2g, rs(c2awv))
        for mo in range(3):
            m1, m2 = (mo + 1) % 3, (mo + 2) % 3
            EE.tensor_mul(sAv[:, 0], rs1a[:, m1, 1 + lo:1 + hi, :],
                          c2awv[:, m2, lo:hi, :])
            EE.tensor_mul(sBv[:, 0], rs1a[:, m2, 1 + lo:1 + hi, :],
                          c2awv[:, m1, lo:hi, :])
            EE.tensor_sub(c3awv[:, mo, lo:hi, :], sAv[:, 0], sBv[:, 0])
        EE.tensor_mul(sAv, rs(cpb), rs(c2awv))
        EE.tensor_mul(sBv, rs(spb), rs(c3awv))
        EE.tensor_add(rs(rs2a[:, :, 1:, :]), sAv, sBv)
        EE.tensor_mul(sAv, rs(cpb), rs(c3awv))
        EE.tensor_mul(sBv, rs(spb), rs(c2awv))
        EE.tensor_sub(rs(rs3a[:, :, 1:, :]), sAv, sBv)

    for m in range(3):
        rm = rsv(RS_a[m])
        V3.memset(rm[:, :, 0, :], 0.0)
        V3.memset(rm[:, m, 0, :], 1.0)
    x_seed.close()

    # ---------------- P4: apply -> backbone AoS ----------------
    x_bbaos = ExitStack()
    p_bb = x_bbaos.enter_context(tc.tile_pool(name="bbaos", bufs=1))
    BBAOS = p_bb.tile([P, J * NA * 3], F32, tag="BBAOS")

    def bbv():
        return BBAOS[:].rearrange("p (j a m) -> p j a m", j=J, a=NA, m=3)

    # ---------------- P5: atoms 0,1,2 ----------------
    bb = bbv()
    V3.memset(bb[:, :, 0:2, :], 0.0)
    nc.scalar.copy(bb[:, :, 1, 0], cd0)
    V3.memset(bb[:, :, 2, 2], 0.0)
    nc.scalar.copy(bb[:, :, 2, 0], Av[:, 0, 3, 0])
    nc.scalar.copy(bb[:, :, 2, 1], Av[:, 0, 3, 1])


    x_apply = ExitStack()
    p_ap = x_apply.enter_context(tc.tile_pool(name="applyp", bufs=4))
    wtv = WT[:].rearrange("p (m c l j) -> p m c l j", m=3, c=C, l=L, j=J)
    for c in range(C):
        # [p, comp, L, J] per chunk (3 free dims: walrus TENSOR3D limit)
        aw = [Av[:, c, m].unsqueeze(2).broadcast_to([P, 3, L, J])
              for m in range(3)]
        at = Av[:, c, 3].unsqueeze(2).broadcast_to([P, 3, L, J])
        wtm = [wtv[:, m, c].unsqueeze(1).broadcast_to([P, 3, L, J])
               for m in range(3)]

        EE = GP if c >= 10 else V3

        def qq(tag):
            return p_ap.tile([P, 3 * L * J], F16, tag=tag,
                             name=f"{tag}_{c}")[:].rearrange(
                "p (m l j) -> p m l j", m=3, l=L, j=J)

        q1, q2 = qq("q1"), qq("q2")
        EE.tensor_mul(q1, aw[0], wtm[0])
        EE.tensor_mul(q2, aw[1], wtm[1])
        EE.tensor_add(q1, q1, q2)
        q2 = qq("q2")
        EE.tensor_mul(q2, aw[2], wtm[2])
        EE.tensor_add(q1, q1, q2)
        outv = bbv()[:, :, 3 + c * L:3 + (c + 1) * L, :].transpose(
            [0, 3, 2, 1])
        EE.tensor_add(outv, q1, at)
    x_apply.close()

    # ---------------- P6: backbone DMA out + T0 gather ----------------
    outdv = OUT_d.ap().rearrange("p (j a m) -> p j a m", j=J, a=400, m=3)
    nc.sync.dma_start(outdv[:, :, 0:87, :], bb[:, :, 0:87, :])
    nc.sync.dma_start(outdv[:, :, 87:NB, :], bb[:, :, 87:NB, :])
    t0v = T0[:].rearrange("p (m r j) -> p m r j", m=3, r=N_RES, j=J)
    nc.scalar.copy(t0v, bb[:, :, 1:NB:3, :].transpose([0, 3, 2, 1]))
    x_bbaos.close()
    x_wt.close()
    x_w.close()


    # ---------------- P7: sidechain inputs + trig ----------------
    x_side2 = ExitStack()
    p_side2 = x_side2.enter_context(tc.tile_pool(name="side2", bufs=1))
    RS_b = [p_side2.tile([P, 3 * N_RES * J], F16, tag=f"RSb{i}",
                         name=f"RSb{i}") for i in range(3)]
    SIDEAOS = p_side2.tile([P, J * NS * 3], F32, tag="SIDEAOS")

    # ---------------- P9: sidechain serial chain ----------------
    sav = SIDEAOS[:].rearrange("p (j r v m) -> p j r v m",
                               j=J, r=N_RES, v=5, m=3)
    p_st = x_side2.enter_context(tc.tile_pool(name="sidetmp", bufs=3))

    def sl(t, v):
        s = t[:].rearrange("p (v r j) -> p v r j", v=5, r=N_RES, j=J)[:, v]
        return _bcast_m(s)

    cur, nxt = RS_a, RS_b
    RD = 40  # residues 0..RD-1 on DVE, RD..49 on GPSIMD (independent chains)
    for v in range(5):
        cpl, spl = sl(SCP, v), sl(SSP, v)
        cal, sal = sl(SCA, v), sl(SSA, v)
        sdl = sl(SD, v)
        r1p, r2p, r3p = rsv(cur[0]), rsv(cur[1]), rsv(cur[2])
        r1c, r2c, r3c = rsv(nxt[0]), rsv(nxt[1]), rsv(nxt[2])
        out = sav[:, :, :, v, :].transpose([0, 3, 2, 1])
        prev = (t0v if v == 0
                else sav[:, :, :, v - 1, :].transpose([0, 3, 2, 1]))

        for EE, lo, hi, sfx in ((V3, 0, RD, "d"), (GP, RD, N_RES, "g")):
            n = hi - lo

            def stp(tag):
                return p_st.tile([P, 3 * n * J], F16, tag=tag + sfx,
                                 name=tag + sfx)[:].rearrange(
                    "p (m r j) -> p m r j", m=3, r=n, j=J)

            def rs(ap):
                return ap[:, :, lo:hi, :]

            t1, t2, c2x = stp("s1"), stp("s2"), stp("sc2x")
            EE.tensor_mul(t1, rs(cpl), rs(r2p))
            EE.tensor_mul(t2, rs(spl), rs(r3p))
            EE.tensor_add(c2x, t1, t2)
            if v < 4:
                t1, t2 = stp("s1"), stp("s2")
                EE.tensor_mul(t1, rs(cpl), rs(r3p))
                EE.tensor_mul(t2, rs(spl), rs(r2p))
                EE.tensor_sub(rs(r3c), t1, t2)
            t1, t2 = stp("s1"), stp("s2")
            EE.tensor_mul(t1, rs(cal), rs(r1p))
            EE.tensor_mul(t2, rs(sal), c2x)
            EE.tensor_add(rs(r1c), t1, t2)
            if v < 4:
                t1, t2 = stp("s1"), stp("s2")
                EE.tensor_mul(t1, rs(cal), c2x)
                EE.tensor_mul(t2, rs(sal), rs(r1p))
                EE.tensor_sub(rs(r2c), t1, t2)
            if v == 4 and sfx == "d":
                h = n // 2
                for a, b in ((0, h), (h, n)):
                    t9h = p_st.tile([P, 3 * (b - a) * J], F16,
                                    tag="st9" + sfx, name=f"st9h{a}")[:]                        .rearrange("p (m r j) -> p m r j",
                                   m=3, r=b - a, j=J)
                    EE.tensor_mul(t9h, sdl[:, :, lo + a:lo + b, :],
                                  r1c[:, :, lo + a:lo + b, :])
                    EE.tensor_add(out[:, :, lo + a:lo + b, :],
                                  prev[:, :, lo + a:lo + b, :], t9h)
            else:
                t9 = stp("st9")
                EE.tensor_mul(t9, rs(sdl), rs(r1c))
                EE.tensor_add(rs(out), rs(prev), t9)
        cur, nxt = nxt, cur

    # ---------------- P10: sidechain DMA out ----------------
    sflat = SIDEAOS[:].rearrange("p (j x) -> p j x", j=J, x=NS * 3)
    dflat = OUT_d.ap().rearrange("p (j a m) -> p j a m", j=J, a=400, m=3)[
        :, :, NB:400, :].rearrange("p j a m -> p j (a m)")
    nc.sync.dma_start(dflat[:, :, 600:750], sflat[:, :, 600:750])
    nc.sync.dma_start(dflat[:, :, 0:300], sflat[:, :, 0:300])
    nc.sync.dma_start(dflat[:, :, 300:600], sflat[:, :, 300:600])

    x_side2.close()
    es.close()


def _build():
    if "nc" in _cache:
        return _cache["nc"]
    nc = bacc.Bacc("TRN2", target_bir_lowering=False, debug=False,
                   num_devices=NCORES)
    io = {
        "TH": nc.dram_tensor("TH", [P, KP * J], F16, kind="ExternalInput"),
        "PH": nc.dram_tensor("PH", [P, KP * J], F16, kind="ExternalInput"),
        "DD": nc.dram_tensor("DD", [P, KP * J], F16, kind="ExternalInput"),
        "SEED": nc.dram_tensor("SEED", [P, 48], F32, kind="ExternalInput"),
        "SD": nc.dram_tensor("SD", [P, 5 * N_RES * J], F16,
                             kind="ExternalInput"),
        "SA": nc.dram_tensor("SA", [P, 5 * N_RES * J], F16,
                             kind="ExternalInput"),
        "SPH": nc.dram_tensor("SPH", [P, 5 * N_RES * J], F16,
                              kind="ExternalInput"),
        "OUT": nc.dram_tensor("OUT", [P, J * 400 * 3], F32,
                              kind="ExternalOutput"),
    }
    with tile.TileContext(nc) as tc:
        _emit(tc, io)
    nc.compile()
    _cache["nc"] = nc
    return nc


def _marshal_core(cd, ca, ct, sd, sa, st):
    """Per-core (S samples) numpy repack into [128, free] layouts."""
    n = cd.shape[0]

    def tp(arr, ncol):  # (n, ncol) -> [128, ncol*16] k-major
        return np.ascontiguousarray(
            arr.reshape(P, J, ncol).transpose(0, 2, 1).reshape(P, ncol * J))

    th = np.full((n, KP), PI, np.float32)
    th[:, :K] = ca[:, 1:148]
    ph = np.zeros((n, KP), np.float32)
    ph[:, :K] = ct[:, :K]
    dd = np.zeros((n, KP), np.float32)
    dd[:, :K] = cd[:, 2:149]
    seed = np.stack([cd[:, 0], cd[:, 1], ca[:, 0]], axis=1)

    def tps(arr):  # (n, 50, 5) -> [128, 4000] (v, r, j)
        return np.ascontiguousarray(
            arr.reshape(P, J, N_RES, 5).transpose(0, 3, 2, 1).reshape(P, 4000))

    sph = np.empty((n, N_RES, 5), np.float32)
    sph[:, :, 0] = CHI0
    sph[:, :, 1:] = st.reshape(n, N_RES, V)
    return {
        "TH": tp(th, KP).astype(np.float16),
        "PH": tp(ph, KP).astype(np.float16),
        "DD": tp(dd, KP).astype(np.float16),
        "SEED": tp(seed, 3),
        "SD": tps(sd.reshape(n, N_RES, 5)).astype(np.float16),
        "SA": tps(sa.reshape(n, N_RES, 5)).astype(np.float16),
        "SPH": tps(sph).astype(np.float16),
    }


def kernel(central_distances, central_angles, central_dihedrals,
           side_distances, side_angles, side_dihedrals, **kw):
    nc = _build()
    in_maps = []
    for i in range(NCORES):
        sl_ = slice(i * S, (i + 1) * S)
        in_maps.append(_marshal_core(
            central_distances[sl_], central_angles[sl_],
            central_dihedrals[sl_], side_distances[sl_],
            side_angles[sl_], side_dihedrals[sl_]))
    res = run_bass_kernel_spmd(nc, in_maps, core_ids=list(range(NCORES)),
                               trace=TRACE, stitch_traces=False)
    _cache["last_results"] = res
    outs = []
    for i in range(NCORES):
        o = res.results[i]["OUT"].reshape(P, J, 400, 3).reshape(S, 400, 3)
        outs.append(o)
    return np.ascontiguousarray(np.concatenate(outs, axis=0))


# revision 52
# speedup vs baseline: 1.2177x; 1.0041x over previous
"""Trainium2 Bass kernel for nn_BackMapLayerWithSidechains.

Algorithm: the sequential NeRF atom-placement chain is reformulated as a
product of affine transforms (R, t): each step composes M = Rx(phi) @
Rz(pi - theta) and t += d * R[:, 0].  Composition is associative, so the
147-step backbone chain is evaluated with a blocked scan:
  - within-chunk prefixes (C=13 chunks x L=12 steps) computed serially but
    vectorized over (chunk, sample),
  - chunk prefixes composed serially (13 small composes),
  - one wide "apply" produces every global atom position.
Sidechain frames branch off backbone frames exactly:
  F_side(r) = F_global(3r-1) . Rx(phi_{3r-1})   (r >= 1; r=0 is identity at
  atom 1), then 5 more affine steps per residue, vectorized over
  (residue, sample).  Rotation state, trig tables, and inputs run in fp16
  (bounded values, 10-bit mantissa); global positions accumulate in fp32.
Serial phases split independent work (disjoint residues / sample lanes)
across DVE and GPSIMD; trig uses ACT Sin with the free affine; t-chain
updates ride GPSIMD (one-way dependency off the DVE critical path).

Data parallel over 8 NeuronCores: 2048 samples/core.  Host-side numpy only
reshapes/pads inputs into the on-chip layouts (no compute).
"""

import math
from contextlib import ExitStack

import numpy as np

import concourse.bacc as bacc
import concourse.bass as bass
import concourse.mybir as mybir
import concourse.tile as tile
from concourse.bass_utils import run_bass_kernel_spmd

F32 = mybir.dt.float32
BF16 = mybir.dt.bfloat16
F16 = mybir.dt.float16
AF = mybir.ActivationFunctionType

P = 128          # partitions
J = 16           # samples per partition (per core: 2048 = 128*16)
S = P * J        # samples per core
NCORES = 8
B = S * NCORES   # 16384

N_RES = 50
V = 4
NB = 150
NS = 250
K = 147          # real backbone steps
C = 13           # chunks
L = 12           # steps/chunk
KP = C * L       # 156 padded steps
NA = 159         # padded backbone atom count (3 + KP)
CHI0 = 2.0943951023931953
PI = math.pi

_cache = {}
TRACE = False


def _bcast_m(ap3):
    """[p, X, 16] -> [p, 3, X, 16] via 0-stride comp dim."""
    return ap3.unsqueeze(1).broadcast_to([P, 3] + list(ap3.shape[1:]))


def _emit(tc, io):
    nc = tc.nc
    V3 = nc.vector
    GP = nc.gpsimd

    TH_d, PH_d, DD_d, SEED_d, SD_d, SA_d, SPH_d, OUT_d = (
        io["TH"], io["PH"], io["DD"], io["SEED"], io["SD"], io["SA"],
        io["SPH"], io["OUT"])

    # Pools: strict LIFO release order.  Open order (outer->inner):
    # perm, side1, w, wt, trig1, dd, c3, thph
    es = ExitStack()
    p_perm = es.enter_context(tc.tile_pool(name="perm", bufs=1))
    p_side1 = es.enter_context(tc.tile_pool(name="side1", bufs=1))
    x_w = ExitStack()
    p_w = x_w.enter_context(tc.tile_pool(name="w", bufs=1))
    x_wt = ExitStack()
    p_wt = x_wt.enter_context(tc.tile_pool(name="wt", bufs=1))
    x_trig1 = ExitStack()
    p_trig1 = x_trig1.enter_context(tc.tile_pool(name="trig1", bufs=1))
    x_dd = ExitStack()
    p_dd = x_dd.enter_context(tc.tile_pool(name="dd", bufs=1))
    x_c3 = ExitStack()
    p_c3 = x_c3.enter_context(tc.tile_pool(name="c3", bufs=1))
    p_tmp = x_c3.enter_context(tc.tile_pool(name="wtmp", bufs=4))
    x_thph = ExitStack()
    p_thph = x_thph.enter_context(tc.tile_pool(name="thph", bufs=1))

    consts = {}

    def cst(val):
        if val not in consts:
            t = p_perm.tile([P, 1], F32, tag=f"cst{len(consts)}",
                            name=f"cst{len(consts)}")
            V3.memset(t[:], val)
            consts[val] = t[:]
        return consts[val]

    SEED = p_perm.tile([P, 48], F32, tag="SEED")
    CP = p_perm.tile([P, KP * J], F16, tag="CP")
    SP = p_perm.tile([P, KP * J], F16, tag="SP")
    A_all = p_perm.tile([P, C * 192], F32, tag="A_all")

    RS_a = [p_side1.tile([P, 3 * N_RES * J], F16, tag=f"RSa{i}",
                         name=f"RSa{i}") for i in range(3)]
    T0 = p_side1.tile([P, 3 * N_RES * J], F32, tag="T0")
    SD = p_side1.tile([P, 5 * N_RES * J], F16, tag="SDb")
    SCA = p_side1.tile([P, 5 * N_RES * J], F16, tag="SCA")
    SSA = p_side1.tile([P, 5 * N_RES * J], F16, tag="SSA")
    SCP = p_side1.tile([P, 5 * N_RES * J], F16, tag="SCP")
    SSP = p_side1.tile([P, 5 * N_RES * J], F16, tag="SSP")

    WC1 = p_w.tile([P, 3 * KP * J], F16, tag="WC1")
    WC2 = p_w.tile([P, 3 * KP * J], F16, tag="WC2")
    WT = p_wt.tile([P, 3 * KP * J], F16, tag="WT")
    CA_ = p_trig1.tile([P, KP * J], F16, tag="CA_")
    SA = p_trig1.tile([P, KP * J], F16, tag="SA")
    DD = p_dd.tile([P, KP * J], F16, tag="DD")
    SAs = p_dd.tile([P, 5 * N_RES * J], F16, tag="SAs")
    SPHs = p_dd.tile([P, 5 * N_RES * J], F16, tag="SPHs")
    c3t = [p_c3.tile([P, 3 * C * J], F16, tag=f"c3{i}", name=f"c3{i}")
           for i in range(2)]
    TH = p_thph.tile([P, KP * J], F16, tag="TH")
    PH = p_thph.tile([P, KP * J], F16, tag="PH")

    # ---------------- P0: input DMAs (backbone) ----------------
    nc.sync.dma_start(TH[:], TH_d.ap())
    nc.sync.dma_start(PH[:], PH_d.ap())
    nc.sync.dma_start(DD[:], DD_d.ap())
    nc.sync.dma_start(SEED[:], SEED_d.ap())
    nc.sync.dma_start(SD[:], SD_d.ap())
    nc.sync.dma_start(SAs[:], SA_d.ap())
    nc.sync.dma_start(SPHs[:], SPH_d.ap())

    # ---------------- P1: backbone trig (ACT), fp16 outputs ----------
    nc.scalar.activation(CA_[:], TH[:], AF.Sin, bias=cst(-PI / 2))
    nc.scalar.activation(SA[:], TH[:], AF.Sin)
    nc.scalar.activation(SP[:], PH[:], AF.Sin)
    nc.scalar.activation(PH[:], PH[:], AF.Abs)
    nc.scalar.activation(CP[:], PH[:], AF.Sin, scale=-1.0, bias=cst(PI / 2))
    x_thph.close()
    # sidechain trig early: overlaps the whole backbone on ACT
    nc.scalar.activation(SCA[:], SAs[:], AF.Sin, bias=cst(-PI / 2))
    nc.scalar.activation(SSA[:], SAs[:], AF.Sin)
    nc.scalar.activation(SSP[:], SPHs[:], AF.Sin)
    nc.scalar.activation(SPHs[:], SPHs[:], AF.Abs)
    nc.scalar.activation(SCP[:], SPHs[:], AF.Sin, scale=-1.0,
                         bias=cst(PI / 2))

    def wv(t):
        return t[:].rearrange("p (m k j) -> p m k j", m=3, k=KP, j=J)

    def wl(t, l):
        return t[:].rearrange("p (m c l j) -> p m c l j",
                              m=3, c=C, l=L, j=J)[:, :, :, l, :]

    def trig_l(t, l, bcast=True):
        s = t[:].rearrange("p (c l j) -> p c l j", c=C, l=L, j=J)[:, :, l, :]
        return _bcast_m(s) if bcast else s

    def c3v(i):
        return c3t[i][:].rearrange("p (m c j) -> p m c j", m=3, c=C, j=J)

    def anv(t):
        return t[:].rearrange("p (c n m j) -> p c n m j", c=C, n=4, m=3, j=J)

    Av = anv(A_all)

    # ---------------- P2a: within-chunk init (l = 0) ----------------
    w1_0, w2_0 = wl(WC1, 0), wl(WC2, 0)
    ca0, sa0 = trig_l(CA_, 0, False), trig_l(SA, 0, False)
    cp0, sp0 = trig_l(CP, 0, False), trig_l(SP, 0, False)
    nc.scalar.copy(w1_0[:, 0], ca0)
    V3.tensor_mul(w1_0[:, 1], sa0, cp0)
    V3.tensor_mul(w1_0[:, 2], sa0, sp0)
    V3.tensor_scalar_mul(w2_0[:, 0], sa0, -1.0)
    V3.tensor_mul(w2_0[:, 1], ca0, cp0)
    V3.tensor_mul(w2_0[:, 2], ca0, sp0)
    c3i = c3v(0)
    V3.memset(c3i[:, 0], 0.0)
    V3.tensor_scalar_mul(c3i[:, 1], sp0, -1.0)
    nc.scalar.copy(c3i[:, 2], cp0)
    GP.tensor_mul(wl(WT, 0), trig_l(DD, 0), wl(WC1, 0))

    # ---------------- P2b: within-chunk serial scan ----------------
    for l in range(1, L):
        cpl, spl = trig_l(CP, l), trig_l(SP, l)
        cal, sal = trig_l(CA_, l), trig_l(SA, l)
        p1, p2 = wl(WC1, l - 1), wl(WC2, l - 1)
        p3 = c3v((l - 1) % 2)
        c3c = c3v(l % 2)
        sh = [P, 3 * C * J]

        def tt(tag):
            return p_tmp.tile(sh, F16, tag=tag, name=tag)[:].rearrange(
                "p (m c j) -> p m c j", m=3, c=C, j=J)

        def ttf(tag):
            return p_tmp.tile(sh, F32, tag=tag, name=tag)[:].rearrange(
                "p (m c j) -> p m c j", m=3, c=C, j=J)

        t1, t2, c2x = tt("t1"), tt("t2"), tt("c2x")
        V3.tensor_mul(t1, cpl, p2)
        V3.tensor_mul(t2, spl, p3)
        V3.tensor_add(c2x, t1, t2)
        t1, t2 = tt("t1"), tt("t2")
        V3.tensor_mul(t1, cpl, p3)
        V3.tensor_mul(t2, spl, p2)
        V3.tensor_sub(c3c, t1, t2)
        t1, t2 = tt("t1"), tt("t2")
        V3.tensor_mul(t1, cal, p1)
        V3.tensor_mul(t2, sal, c2x)
        V3.tensor_add(wl(WC1, l), t1, t2)
        t1, t2 = tt("t1"), tt("t2")
        V3.tensor_mul(t1, cal, c2x)
        V3.tensor_mul(t2, sal, p1)
        V3.tensor_sub(wl(WC2, l), t1, t2)
        t9 = tt("t9")
        GP.tensor_mul(t9, trig_l(DD, l), wl(WC1, l))
        GP.tensor_add(wl(WT, l), wl(WT, l - 1), t9)

    # ---------------- P3a: chunk totals T ----------------
    # T lives in its own pool opened after closing c3/dd/trig1? T build
    # reads c3 -> keep c3 open until the copies below are emitted.
    x_T = ExitStack()
    p_T = x_T.enter_context(tc.tile_pool(name="Tp", bufs=1))
    p_u = x_T.enter_context(tc.tile_pool(name="up", bufs=4))
    T_all = p_T.tile([P, C * 192], F32, tag="T_all")
    Tv = anv(T_all)
    w1f, w2f = wl(WC1, L - 1), wl(WC2, L - 1)
    c3f, wtf = c3v((L - 1) % 2), wl(WT, L - 1)
    for c_ in range(C):
        nc.scalar.copy(Tv[:, c_, 0], w1f[:, :, c_, :])
        nc.scalar.copy(Tv[:, c_, 1], w2f[:, :, c_, :])
        nc.scalar.copy(Tv[:, c_, 2], c3f[:, :, c_, :])
        nc.scalar.copy(Tv[:, c_, 3], wtf[:, :, c_, :])

    # ---------------- P3b: seed A_0 = F0 ----------------
    sdv = SEED[:].rearrange("p (q j) -> p q j", q=3, j=J)
    cd0, cd1, ca0s = sdv[:, 0], sdv[:, 1], sdv[:, 2]
    sca0 = p_T.tile([P, J], F32, tag="sca0", name="sca0")[:]
    cca0 = p_T.tile([P, J], F32, tag="cca0", name="cca0")[:]
    nc.scalar.activation(sca0, ca0s, AF.Sin)
    nc.scalar.activation(cca0, ca0s, AF.Sin, bias=cst(-PI / 2))
    V3.memset(Av[:, 0], 0.0)
    nc.scalar.copy(Av[:, 0, 0, 0], cca0)
    nc.scalar.copy(Av[:, 0, 0, 1], sca0)
    V3.tensor_scalar_mul(Av[:, 0, 1, 0], sca0, -1.0)
    nc.scalar.copy(Av[:, 0, 1, 1], cca0)
    V3.memset(Av[:, 0, 2, 2], 1.0)
    tm0 = p_T.tile([P, J], F32, tag="tm0", name="tm0")[:]
    V3.tensor_mul(tm0, cd1, cca0)
    V3.tensor_add(Av[:, 0, 3, 0], cd0, tm0)
    V3.tensor_mul(Av[:, 0, 3, 1], cd1, sca0)

    # ---------------- P3c: chunk-prefix chain ----------------
    JD = 12  # sample-lane split: j 0..11 DVE, 12..15 GPSIMD (independent)
    for c in range(1, C):
        for EE, jl, jh, sfx in ((V3, 0, JD, "d"), (GP, JD, J, "g")):
            nj = jh - jl
            acm = [Av[:, c - 1, m, :, jl:jh].unsqueeze(1).broadcast_to(
                [P, 4, 3, nj]) for m in range(3)]
            trm = [Tv[:, c - 1, :, m, jl:jh].unsqueeze(2).broadcast_to(
                [P, 4, 3, nj]) for m in range(3)]

            def uu(tag):
                return p_u.tile([P, 12 * nj], F32, tag=tag + sfx,
                                name=tag + sfx)[:].rearrange(
                    "p (n m j) -> p n m j", n=4, m=3, j=nj)

            u1, u2, u3 = uu("u1"), uu("u2"), uu("u3")
            EE.tensor_mul(u1, acm[0], trm[0])
            EE.tensor_mul(u2, acm[1], trm[1])
            EE.tensor_add(u3, u1, u2)
            u1 = uu("u1")
            EE.tensor_mul(u1, acm[2], trm[2])
            EE.tensor_add(Av[:, c, :, :, jl:jh], u3, u1)
            EE.tensor_add(Av[:, c, 3, :, jl:jh], Av[:, c, 3, :, jl:jh],
                          Av[:, c - 1, 3, :, jl:jh])

    # close inner pools (LIFO): Tp/up, then c3, dd, trig1
    x_T.close()
    x_c3.close()
    x_dd.close()
    x_trig1.close()

    # ---------------- P8: sidechain seeds ----------------
    x_seed = ExitStack()
    p_sd = x_seed.enter_context(tc.tile_pool(name="seedp", bufs=1))

    def rsv(t):
        return t[:].rearrange("p (m r j) -> p m r j", m=3, r=N_RES, j=J)

    AEX = [p_sd.tile([P, 3 * 49 * J], F16, tag=f"AEX{m}", name=f"AEX{m}")
           for m in range(3)]

    def aexv(t):
        return t[:].rearrange("p (m r j) -> p m r j", m=3, r=49, j=J)

    for m in range(3):
        dst = aexv(AEX[m])
        for cq in range(12):
            nc.scalar.copy(
                dst[:, :, 4 * cq:4 * cq + 4, :],
                Av[:, cq, m].unsqueeze(2).broadcast_to([P, 3, 4, J]))
        nc.scalar.copy(dst[:, :, 48, :], Av[:, 12, m])

    w1g = wv(WC1)[:, :, 1:146:3, :]
    w2g = wv(WC2)[:, :, 1:146:3, :]
    C2AW = p_sd.tile([P, 3 * 49 * J], F16, tag="C2AW")
    C3AW = p_sd.tile([P, 3 * 49 * J], F16, tag="C3AW")
    c2awv, c3awv = aexv(C2AW), aexv(C3AW)
    kj = lambda t: t[:].rearrange("p (k j) -> p k j", k=KP, j=J)
    cpb = _bcast_m(kj(CP)[:, 2:147:3, :])
    spb = _bcast_m(kj(SP)[:, 2:147:3, :])
    rs1a, rs2a, rs3a = rsv(RS_a[0]), rsv(RS_a[1]), rsv(RS_a[2])
    RSD = 39  # gather-index split: [0:RSD] DVE, [RSD:49] GPSIMD

    for EE, lo, hi, sfx in ((V3, 0, RSD, "d"), (GP, RSD, 49, "g")):
        n = hi - lo

        def sdt(tag):
            return p_sd.tile([P, 3 * n * J], F16, tag=tag + sfx,
                             name=tag + sfx)[:].rearrange(
                "p (m r j) -> p m r j", m=3, r=n, j=J)

        def rs(ap):
            return ap[:, :, lo:hi, :]

        def rsm(ap, m):
            return ap[:, m, lo:hi, :].unsqueeze(1).broadcast_to(
                [P, 3, n, J])

        sAv, sBv = sdt("sA"), sdt("sB")

        def apply_R(wg, outv):
            EE.tensor_mul(sAv, rs(aexv(AEX[0])), rsm(wg, 0))
            EE.tensor_mul(sBv, rs(aexv(AEX[1])), rsm(wg, 1))
            EE.tensor_add(sAv, sAv, sBv)
            EE.tensor_mul(sBv, rs(aexv(AEX[2])), rsm(wg, 2))
            EE.tensor_add(outv, sAv, sBv)

        apply_R(w1g, rs(rs1a[:, :, 1:, :]))
        apply_R(w2g, rs(c2awv))
        for mo in range(3):
            m1, m2 = (mo + 1) % 3, (mo + 2) % 3
            EE.tensor_mul(sAv[:, 0], rs1a[:, m1, 1 + lo:1 + hi, :],
                          c2awv[:, m2, lo:hi, :])
            EE.tensor_mul(sBv[:, 0], rs1a[:, m2, 1 + lo:1 + hi, :],
                          c2awv[:, m1, lo:hi, :])
            EE.tensor_sub(c3awv[:, mo, lo:hi, :], sAv[:, 0], sBv[:, 0])
        EE.tensor_mul(sAv, rs(cpb), rs(c2awv))
        EE.tensor_mul(sBv, rs(spb), rs(c3awv))
        EE.tensor_add(rs(rs2a[:, :, 1:, :]), sAv, sBv)
        EE.tensor_mul(sAv, rs(cpb), rs(c3awv))
        EE.tensor_mul(sBv, rs(spb), rs(c2awv))
        EE.tensor_sub(rs(rs3a[:, :, 1:, :]), sAv, sBv)

    for m in range(3):
        rm = rsv(RS_a[m])
        V3.memset(rm[:, :, 0, :], 0.0)
        V3.memset(rm[:, m, 0, :], 1.0)
    x_seed.close()

    # ---------------- P4: apply -> backbone AoS ----------------
    x_bbaos = ExitStack()
    p_bb = x_bbaos.enter_context(tc.tile_pool(name="bbaos", bufs=1))
    BBAOS = p_bb.tile([P, J * NA * 3], F32, tag="BBAOS")

    def bbv():
        return BBAOS[:].rearrange("p (j a m) -> p j a m", j=J, a=NA, m=3)

    # ---------------- P5: atoms 0,1,2 ----------------
    bb = bbv()
    V3.memset(bb[:, :, 0:2, :], 0.0)
    nc.scalar.copy(bb[:, :, 1, 0], cd0)
    V3.memset(bb[:, :, 2, 2], 0.0)
    nc.scalar.copy(bb[:, :, 2, 0], Av[:, 0, 3, 0])
    nc.scalar.copy(bb[:, :, 2, 1], Av[:, 0, 3, 1])


    x_apply = ExitStack()
    p_ap = x_apply.enter_context(tc.tile_pool(name="applyp", bufs=4))
    wtv = WT[:].rearrange("p (m c l j) -> p m c l j", m=3, c=C, l=L, j=J)
    for c in range(C):
        # [p, comp, L, J] per chunk (3 free dims: walrus TENSOR3D limit)
        aw = [Av[:, c, m].unsqueeze(2).broadcast_to([P, 3, L, J])
              for m in range(3)]
        at = Av[:, c, 3].unsqueeze(2).broadcast_to([P, 3, L, J])
        wtm = [wtv[:, m, c].unsqueeze(1).broadcast_to([P, 3, L, J])
               for m in range(3)]

        EE = GP if c >= 10 else V3

        def qq(tag):
            return p_ap.tile([P, 3 * L * J], F16, tag=tag,
                             name=f"{tag}_{c}")[:].rearrange(
                "p (m l j) -> p m l j", m=3, l=L, j=J)

        q1, q2 = qq("q1"), qq("q2")
        EE.tensor_mul(q1, aw[0], wtm[0])
        EE.tensor_mul(q2, aw[1], wtm[1])
        EE.tensor_add(q1, q1, q2)
        q2 = qq("q2")
        EE.tensor_mul(q2, aw[2], wtm[2])
        EE.tensor_add(q1, q1, q2)
        outv = bbv()[:, :, 3 + c * L:3 + (c + 1) * L, :].transpose(
            [0, 3, 2, 1])
        EE.tensor_add(outv, q1, at)
    x_apply.close()

    # ---------------- P6: backbone DMA out + T0 gather ----------------
    outdv = OUT_d.ap().rearrange("p (j a m) -> p j a m", j=J, a=400, m=3)
    nc.sync.dma_start(outdv[:, :, 0:87, :], bb[:, :, 0:87, :])
    nc.sync.dma_start(outdv[:, :, 87:NB, :], bb[:, :, 87:NB, :])
    t0v = T0[:].rearrange("p (m r j) -> p m r j", m=3, r=N_RES, j=J)
    nc.scalar.copy(t0v, bb[:, :, 1:NB:3, :].transpose([0, 3, 2, 1]))
    x_bbaos.close()
    x_wt.close()
    x_w.close()


    # ---------------- P7: sidechain inputs + trig ----------------
    x_side2 = ExitStack()
    p_side2 = x_side2.enter_context(tc.tile_pool(name="side2", bufs=1))
    RS_b = [p_side2.tile([P, 3 * N_RES * J], F16, tag=f"RSb{i}",
                         name=f"RSb{i}") for i in range(3)]
    SIDEAOS = p_side2.tile([P, J * NS * 3], F32, tag="SIDEAOS")

    # ---------------- P9: sidechain serial chain ----------------
    sav = SIDEAOS[:].rearrange("p (j r v m) -> p j r v m",
                               j=J, r=N_RES, v=5, m=3)
    p_st = x_side2.enter_context(tc.tile_pool(name="sidetmp", bufs=3))

    def sl(t, v):
        s = t[:].rearrange("p (v r j) -> p v r j", v=5, r=N_RES, j=J)[:, v]
        return _bcast_m(s)

    cur, nxt = RS_a, RS_b
    RD = 40  # residues 0..RD-1 on DVE, RD..49 on GPSIMD (independent chains)
    for v in range(5):
        cpl, spl = sl(SCP, v), sl(SSP, v)
        cal, sal = sl(SCA, v), sl(SSA, v)
        sdl = sl(SD, v)
        r1p, r2p, r3p = rsv(cur[0]), rsv(cur[1]), rsv(cur[2])
        r1c, r2c, r3c = rsv(nxt[0]), rsv(nxt[1]), rsv(nxt[2])
        out = sav[:, :, :, v, :].transpose([0, 3, 2, 1])
        prev = (t0v if v == 0
                else sav[:, :, :, v - 1, :].transpose([0, 3, 2, 1]))

        for EE, lo, hi, sfx in ((V3, 0, RD, "d"), (GP, RD, N_RES, "g")):
            n = hi - lo

            def stp(tag):
                return p_st.tile([P, 3 * n * J], F16, tag=tag + sfx,
                                 name=tag + sfx)[:].rearrange(
                    "p (m r j) -> p m r j", m=3, r=n, j=J)

            def rs(ap):
                return ap[:, :, lo:hi, :]

            t1, t2, c2x = stp("s1"), stp("s2"), stp("sc2x")
            EE.tensor_mul(t1, rs(cpl), rs(r2p))
            EE.tensor_mul(t2, rs(spl), rs(r3p))
            EE.tensor_add(c2x, t1, t2)
            if v < 4:
                t1, t2 = stp("s1"), stp("s2")
                EE.tensor_mul(t1, rs(cpl), rs(r3p))
                EE.tensor_mul(t2, rs(spl), rs(r2p))
                EE.tensor_sub(rs(r3c), t1, t2)
            t1, t2 = stp("s1"), stp("s2")
            EE.tensor_mul(t1, rs(cal), rs(r1p))
            EE.tensor_mul(t2, rs(sal), c2x)
            EE.tensor_add(rs(r1c), t1, t2)
            if v < 4:
                t1, t2 = stp("s1"), stp("s2")
                EE.tensor_mul(t1, rs(cal), c2x)
                EE.tensor_mul(t2, rs(sal), rs(r1p))
                EE.tensor_sub(rs(r2c), t1, t2)
            if v == 4 and sfx == "d":
                q = n // 4
                for a, b in ((0, q), (q, 2 * q), (2 * q, 3 * q),
                             (3 * q, n)):
                    t9h = p_st.tile([P, 3 * (b - a) * J], F16,
                                    tag="st9" + sfx, name=f"st9h{a}")[:]                        .rearrange("p (m r j) -> p m r j",
                                   m=3, r=b - a, j=J)
                    EE.tensor_mul(t9h, sdl[:, :, lo + a:lo + b, :],
                                  r1c[:, :, lo + a:lo + b, :])
                    EE.tensor_add(out[:, :, lo + a:lo + b, :],
                                  prev[:, :, lo + a:lo + b, :], t9h)
            else:
                t9 = stp("st9")
                EE.tensor_mul(t9, rs(sdl), rs(r1c))
                EE.tensor_add(rs(out), rs(prev), t9)
        cur, nxt = nxt, cur

    # ---------------- P10: sidechain DMA out ----------------
    sflat = SIDEAOS[:].rearrange("p (j x) -> p j x", j=J, x=NS * 3)
    dflat = OUT_d.ap().rearrange("p (j a m) -> p j a m", j=J, a=400, m=3)[
        :, :, NB:400, :].rearrange("p j a m -> p j (a m)")
    nc.sync.dma_start(dflat[:, :, 600:750], sflat[:, :, 600:750])
    for x0 in (0, 150, 300, 450):
        nc.sync.dma_start(dflat[:, :, x0:x0 + 150], sflat[:, :, x0:x0 + 150])

    x_side2.close()
    es.close()


def _build():
    if "nc" in _cache:
        return _cache["nc"]
    nc = bacc.Bacc("TRN2", target_bir_lowering=False, debug=False,
                   num_devices=NCORES)
    io = {
        "TH": nc.dram_tensor("TH", [P, KP * J], F16, kind="ExternalInput"),
        "PH": nc.dram_tensor("PH", [P, KP * J], F16, kind="ExternalInput"),
        "DD": nc.dram_tensor("DD", [P, KP * J], F16, kind="ExternalInput"),
        "SEED": nc.dram_tensor("SEED", [P, 48], F32, kind="ExternalInput"),
        "SD": nc.dram_tensor("SD", [P, 5 * N_RES * J], F16,
                             kind="ExternalInput"),
        "SA": nc.dram_tensor("SA", [P, 5 * N_RES * J], F16,
                             kind="ExternalInput"),
        "SPH": nc.dram_tensor("SPH", [P, 5 * N_RES * J], F16,
                              kind="ExternalInput"),
        "OUT": nc.dram_tensor("OUT", [P, J * 400 * 3], F32,
                              kind="ExternalOutput"),
    }
    with tile.TileContext(nc) as tc:
        _emit(tc, io)
    nc.compile()
    _cache["nc"] = nc
    return nc


def _marshal_core(cd, ca, ct, sd, sa, st):
    """Per-core (S samples) numpy repack into [128, free] layouts."""
    n = cd.shape[0]

    def tp(arr, ncol):  # (n, ncol) -> [128, ncol*16] k-major
        return np.ascontiguousarray(
            arr.reshape(P, J, ncol).transpose(0, 2, 1).reshape(P, ncol * J))

    th = np.full((n, KP), PI, np.float32)
    th[:, :K] = ca[:, 1:148]
    ph = np.zeros((n, KP), np.float32)
    ph[:, :K] = ct[:, :K]
    dd = np.zeros((n, KP), np.float32)
    dd[:, :K] = cd[:, 2:149]
    seed = np.stack([cd[:, 0], cd[:, 1], ca[:, 0]], axis=1)

    def tps(arr):  # (n, 50, 5) -> [128, 4000] (v, r, j)
        return np.ascontiguousarray(
            arr.reshape(P, J, N_RES, 5).transpose(0, 3, 2, 1).reshape(P, 4000))

    sph = np.empty((n, N_RES, 5), np.float32)
    sph[:, :, 0] = CHI0
    sph[:, :, 1:] = st.reshape(n, N_RES, V)
    return {
        "TH": tp(th, KP).astype(np.float16),
        "PH": tp(ph, KP).astype(np.float16),
        "DD": tp(dd, KP).astype(np.float16),
        "SEED": tp(seed, 3),
        "SD": tps(sd.reshape(n, N_RES, 5)).astype(np.float16),
        "SA": tps(sa.reshape(n, N_RES, 5)).astype(np.float16),
        "SPH": tps(sph).astype(np.float16),
    }


def kernel(central_distances, central_angles, central_dihedrals,
           side_distances, side_angles, side_dihedrals, **kw):
    nc = _build()
    in_maps = []
    for i in range(NCORES):
        sl_ = slice(i * S, (i + 1) * S)
        in_maps.append(_marshal_core(
            central_distances[sl_], central_angles[sl_],
            central_dihedrals[sl_], side_distances[sl_],
            side_angles[sl_], side_dihedrals[sl_]))
    res = run_bass_kernel_spmd(nc, in_maps, core_ids=list(range(NCORES)),
                               trace=TRACE, stitch_traces=False)
    _cache["last_results"] = res
    outs = []
    for i in range(NCORES):
        o = res.results[i]["OUT"].reshape(P, J, 400, 3).reshape(S, 400, 3)
        outs.append(o)
    return np.ascontiguousarray(np.concatenate(outs, axis=0))
